# revision 2
# baseline (speedup 1.0000x reference)
"""AWQ 4-bit quantized linear (out = x @ dequant(qweight).T + bias), 8-core
tensor-parallel on TRN2.

Sharding: out_features split 8 ways (O' = 1024 per core); x replicated; each
core computes out[:, c*1024:(c+1)*1024] and the host concatenates.  No
device collectives.

Per-core layout (same trick as the 1-core baseline, O-sliced): qweight u16
rows are permuted so SBUF tile i holds u16-rows {16p + i : p in [0,128)}.
Input column c = 4r + k = 64p + (4i + k); the AWQ group of column c is
g = p//2, independent of (i,k), so one [128, O'] scale tile
s128[p,o] = scales[o, p//2] serves every dequant op, and the x-transposes
are plain stride-64 column slices of x.

Dequant per tile/plane: nib = qw & (15<<4k) (DVE tensor_scalar, 4x mode),
w = nib * s128 (tensor_tensor, 2x mode).  The 2^{4k} nibble-position factor
is compensated on the x side during PSUM->SBUF eviction (ACT).  A subset of
the w-multiplies runs on the GPSIMD (Pool) engine to unload DVE, which is
otherwise the bottleneck engine.

Zeros + bias fold into one extra K=128 matmul: R[p,t] = sum of raw x over
columns [64p, 64p+64) (recovered from the scaled transposes via identity-
matmul accumulation and a 16^k recombination), paired with
bmat[p,o] = -(s*z)[p//2, o]; row 127 of R is forced to 1.0 and bmat[127] =
bias (row 127's half-group sum is folded into row 126 by matrix A).

Benchmarking: the per-execute axon dispatch costs ~1 ms/core and swamps
wall-clock timing of single executions.  bench() therefore builds a second
Bass module with the whole per-iteration body unrolled K_HI times (each
iteration re-DMAs all inputs from DRAM and re-writes the output) and
measures the slope between the K_LO- and K_HI-iteration programs: the
marginal cost of one extra full computation on-device, i.e. the steady-state
HW execution time per iteration.
"""

import numpy as np
from contextlib import ExitStack

import concourse.bass as bass
import concourse.mybir as mybir
import concourse.tile as tile
from concourse.bass_utils import run_bass_kernel_spmd
from concourse.masks import make_identity

dt = mybir.dt

N_CORES = 8
I = 8192                    # in_features
O = 8192                    # out_features
OSH = O // N_CORES          # 1024 out-features per core
T = 128                     # batch*seq = 4*32
NG = 64                     # groups (group_size 128)
NR = 64                     # residue tiles (r64 = 4i + k)
NS = OSH // 512             # 512-wide matmul chunks per out block (2)

# (i, k) pairs whose dequant multiply runs on the Pool (GPSIMD) engine
# instead of DVE.  k=3 needs no AND (full-u16 plane) so it is the cheapest
# to offload; a few k=2 multiplies rebalance the remainder.
_POOL_MULS = {(i, 3) for i in range(16)} | {(i, 2) for i in (3, 7, 11, 15)}

_CACHE = {}


def _emit_iteration(nc, tc, P):
    """Emit one full computation: load everything from DRAM, dequantize,
    matmul, correct zeros+bias, store out."""
    qw_d, x_d, aux_d, out_d = P["qw_d"], P["x_d"], P["aux_d"], P["out_d"]
    ident = P["ident"]

    x_sb = P["x_p"].tile([T, I], dt.float16, tag="x")
    nc.scalar.dma_start(x_sb[:], x_d[:, :])
    s128 = P["cst_p"].tile([128, OSH], dt.float16, tag="s128")
    nc.scalar.dma_start(s128[:], aux_d[:, 0:OSH])
    bmat = P["cst_p"].tile([128, OSH], dt.float16, tag="bmat")
    nc.scalar.dma_start(bmat[:], aux_d[:, OSH : 2 * OSH])
    amat = P["sm_p"].tile([128, 128], dt.float16, tag="amat")
    nc.scalar.dma_start(amat[:], aux_d[:, 2 * OSH : 2 * OSH + 128])
    cmat = P["sm_p"].tile([128, T], dt.float16, tag="cmat")
    nc.scalar.dma_start(cmat[:], aux_d[:, 2 * OSH + 128 : 2 * OSH + 256])

    # ---- preamble: 64 transposes, batched 4-per-PSUM-tile ----
    x_r = x_sb.rearrange("t (p r) -> t r p", r=NR)
    xts, yts = {}, {}
    for k in range(4):
        for q in range(4):
            ps = P["pst_p"].tile([128, 4 * T], dt.float16, tag="tp")
            for m in range(4):
                r64 = 4 * (4 * q + m) + k
                nc.tensor.transpose(
                    ps[:, T * m : T * (m + 1)], x_r[:, r64, :], ident[:]
                )
            xt = P["xt_p"].tile([128, 4 * T], dt.float16, tag=f"xt{k}_{q}")
            if k == 0:
                nc.scalar.copy(xt[:], ps[:])
            else:
                nc.scalar.mul(xt[:], ps[:], float(2.0 ** (-4 * k)))
            xts[(k, q)] = xt

    # V-plane trick: plane k=3 is the FULL u16 value (w = v*s, no AND),
    # paired with a3; planes k<3 pair with y_k = a_k - a_3 so the extra
    # terms telescope away exactly.
    for k in range(3):
        for q in range(4):
            y = P["xt_p"].tile([128, 4 * T], dt.float16, tag=f"y{k}_{q}")
            nc.vector.tensor_tensor(
                out=y[:], in0=xts[(k, q)][:], in1=xts[(3, q)][:],
                op=mybir.AluOpType.subtract,
            )
            yts[(k, q)] = y

    # ---- half-group sums of raw x -> R ----
    psum_x = P["psx_p"].tile([128, 4 * T], dt.float32, tag="psx")
    for k in range(4):
        for q in range(4):
            for m in range(4):
                nc.tensor.matmul(
                    psum_x[:, T * k : T * (k + 1)],
                    amat[:],
                    xts[(k, q)][:, T * m : T * (m + 1)],
                    start=(q == 0 and m == 0),
                    stop=(q == 3 and m == 3),
                )
    t0 = P["sm_p"].tile([128, T], dt.float32, tag="t0")
    t1 = P["sm_p"].tile([128, T], dt.float32, tag="t1")
    t2 = P["sm_p"].tile([128, T], dt.float32, tag="t2")
    t3 = P["sm_p"].tile([128, T], dt.float32, tag="t3")
    rmat = P["sm_p"].tile([128, T], dt.float16, tag="rmat")
    nc.scalar.copy(t0[:], psum_x[:, 0:T])
    nc.vector.scalar_tensor_tensor(
        out=t1[:], in0=psum_x[:, T : 2 * T], scalar=16.0, in1=t0[:],
        op0=mybir.AluOpType.mult, op1=mybir.AluOpType.add,
    )
    nc.vector.scalar_tensor_tensor(
        out=t2[:], in0=psum_x[:, 2 * T : 3 * T], scalar=256.0, in1=t1[:],
        op0=mybir.AluOpType.mult, op1=mybir.AluOpType.add,
    )
    nc.vector.scalar_tensor_tensor(
        out=t3[:], in0=psum_x[:, 3 * T : 4 * T], scalar=4096.0, in1=t2[:],
        op0=mybir.AluOpType.mult, op1=mybir.AluOpType.add,
    )
    # row 127 (zeroed by A) becomes the bias row: rmat = t3 + C
    nc.vector.tensor_tensor(
        out=rmat[:], in0=t3[:], in1=cmat[:], op=mybir.AluOpType.add
    )

    # ---- main: 16 qw tiles x 4 nibble planes -> one [T, OSH] out block ----
    out_ps = P["pso_p"].tile([T, OSH], dt.float32, tag="out")
    for i in range(16):
        qwt = P["qwt_p"].tile([128, OSH], dt.uint16, tag="qwt")
        nc.sync.dma_start(qwt[:], qw_d[128 * i : 128 * (i + 1), :])
        q, m = i // 4, i % 4
        for k in range(4):
            w = P["w_p"].tile([128, OSH], dt.float16, tag="w")
            eng = nc.gpsimd if (i, k) in _POOL_MULS else nc.vector
            if k == 3:
                eng.tensor_tensor(
                    out=w[:], in0=qwt[:], in1=s128[:],
                    op=mybir.AluOpType.mult,
                )
                xt = xts[(3, q)]
            else:
                nib = P["nib_p"].tile([128, OSH], dt.uint16, tag="nib")
                nc.vector.tensor_scalar(
                    out=nib[:], in0=qwt[:], scalar1=15 << (4 * k),
                    scalar2=None, op0=mybir.AluOpType.bitwise_and,
                )
                eng.tensor_tensor(
                    out=w[:], in0=nib[:], in1=s128[:],
                    op=mybir.AluOpType.mult,
                )
                xt = yts[(k, q)]
            for ns in range(NS):
                nc.tensor.matmul(
                    out_ps[:, 512 * ns : 512 * (ns + 1)],
                    xt[:, T * m : T * (m + 1)],
                    w[:, 512 * ns : 512 * (ns + 1)],
                    start=(i == 0 and k == 0),
                    stop=False,
                )
    # zeros + bias correction
    for ns in range(NS):
        nc.tensor.matmul(
            out_ps[:, 512 * ns : 512 * (ns + 1)],
            rmat[:],
            bmat[:, 512 * ns : 512 * (ns + 1)],
            start=False, stop=(ns == NS - 1),
        )
    osb = P["osb_p"].tile([T, OSH], dt.float32, tag="osb")
    nc.scalar.copy(osb[:], out_ps[:])
    nc.scalar.dma_start(out_d[:, :], osb[:])


def _build_nc(n_iters=1):
    nc = bass.Bass()
    qw_d = nc.dram_tensor("qw", [2048, OSH], dt.uint16, kind="ExternalInput")
    x_d = nc.dram_tensor("xin", [T, I], dt.float16, kind="ExternalInput")
    # aux cols: 0:OSH s128 | OSH:2*OSH bmat | +128 amat | +128 cmat
    aux_d = nc.dram_tensor("aux", [128, 2 * OSH + 256], dt.float16,
                           kind="ExternalInput")
    out_d = nc.dram_tensor("out", [T, OSH], dt.float32, kind="ExternalOutput")

    with tile.TileContext(nc) as tc:
        with ExitStack() as ctx:
            P = {
                "qw_d": qw_d, "x_d": x_d, "aux_d": aux_d, "out_d": out_d,
                "x_p": ctx.enter_context(tc.tile_pool(name="x", bufs=2)),
                "cst_p": ctx.enter_context(tc.tile_pool(name="cst", bufs=2)),
                "sm_p": ctx.enter_context(tc.tile_pool(name="sm", bufs=2)),
                "xt_p": ctx.enter_context(tc.tile_pool(name="xt", bufs=2)),
                "qwt_p": ctx.enter_context(tc.tile_pool(name="qwt", bufs=6)),
                "nib_p": ctx.enter_context(tc.tile_pool(name="nib", bufs=4)),
                "w_p": ctx.enter_context(tc.tile_pool(name="w", bufs=6)),
                "osb_p": ctx.enter_context(tc.tile_pool(name="osb", bufs=2)),
                "sgl_p": ctx.enter_context(tc.tile_pool(name="sgl", bufs=1)),
                "pst_p": ctx.enter_context(
                    tc.tile_pool(name="pst", bufs=3, space="PSUM")),
                "psx_p": ctx.enter_context(
                    tc.tile_pool(name="psx", bufs=1, space="PSUM")),
                "pso_p": ctx.enter_context(
                    tc.tile_pool(name="pso", bufs=1, space="PSUM")),
            }
            ident = P["sgl_p"].tile([128, 128], dt.float16, tag="ident")
            make_identity(nc, ident[:])
            P["ident"] = ident
            for _ in range(n_iters):
                _emit_iteration(nc, tc, P)

    _split_excess_waits(nc)
    nc.finalize()
    return nc


_SPLIT_TYPES = {
    "InstTensorScalarPtr", "InstTensorTensor", "InstActivation", "InstMatmult",
    "InstDMACopy", "InstDmaTransposeAnt", "InstMemSet", "InstTensorCopy",
    "InstTensorReduce", "InstDrain", "InstMemset", "InstNoOp",
}

_ENG_MAP = {
    "DVE": "vector", "Activation": "scalar", "PE": "tensor",
    "Pool": "gpsimd", "SP": "sync",
}


def _split_excess_waits(nc):
    """walrus accepts at most one sync-wait per (non-drain) instruction in
    this build; move excess waits onto same-engine ENGINE_NOPs inserted just
    before the instruction."""
    for bb in nc.main_func.blocks:
        insts = list(bb.instructions)
        need = []  # (idx, inst, extra_waits)
        for idx, ins in enumerate(insts):
            if type(ins).__name__ not in _SPLIT_TYPES:
                continue
            si = ins.sync_info
            w = list(si.on_wait) if si else []
            if len(w) > 1:
                need.append((idx, ins, w))
        if not need:
            continue
        created = {}
        for idx, ins, w in need:
            eng = _ENG_MAP.get(ins.engine.name if ins.engine else "", "vector")
            nops = []
            for extra in w[:-1]:
                bi = getattr(nc, eng).nop()
                nop = bi.ins
                nop.sync_info = mybir.SyncInfo(on_wait=[extra], on_update=[])
                nops.append(nop)
            ins.sync_info = mybir.SyncInfo(
                on_wait=[w[-1]], on_update=list(ins.sync_info.on_update))
            created[idx] = nops
        nop_names = {n.name for nops in created.values() for n in nops}
        for bb2 in nc.main_func.blocks:
            cur = [i for i in bb2.instructions if i.name not in nop_names]
            if bb2.name == bb.name:
                out = []
                for idx, ins in enumerate(insts):
                    if idx in created:
                        out.extend(created[idx])
                    out.append(ins)
                bb2.instructions = out
            elif len(cur) != len(list(bb2.instructions)):
                bb2.instructions = cur


def _prep_in_maps(x, qweight, scales, qzeros, bias):
    """Host prep: repack qweight to the permuted-u16 layout and build the
    per-core aux tables.  Cached on input identity."""
    key = (id(x), id(qweight), id(scales), id(qzeros), id(bias))
    cached = _CACHE.get("prep")
    if cached is not None and cached[0] == key:
        return cached[1]

    x = np.asarray(x)
    qweight = np.asarray(qweight)
    scales = np.asarray(scales)
    qzeros = np.asarray(qzeros)
    bias = np.asarray(bias)

    x2 = np.ascontiguousarray(x.reshape(T, I))
    if x2.dtype != np.float16:
        x2 = x2.astype(np.float16)

    # qw tile i, partition p <- u16 row 16p+i of qweight.view(u16).T
    qw16 = qweight.view(np.uint16)              # [O, 2048] (nibble c=4r+k in row o)
    qwp = np.ascontiguousarray(
        qw16.reshape(O, 128, 16).transpose(2, 1, 0)
    ).reshape(2048, O)

    scT = scales.T.astype(np.float16)           # [64, O]
    rep = np.arange(128) // 2
    s128 = np.ascontiguousarray(scT[rep])       # [128, O]

    # zeros: znib[g, o] = nibble g%4 of u16 word g//4 of qzeros row o
    qz16 = qzeros.view(np.uint16)               # [O, 16]
    g = np.arange(NG)
    znib = (qz16.T[g // 4] >> (4 * (g % 4))[:, None]).astype(np.uint16) & 15
    bm_half = -(scT.astype(np.float32) * znib.astype(np.float32))  # [64, O]
    bmat = bm_half[rep].astype(np.float16)
    bmat[127] = bias.astype(np.float16)

    # A matrix: identity, col 126 also sums row 127, col 127 dead
    amat = np.eye(128, dtype=np.float16)
    amat[127, 127] = 0.0
    amat[127, 126] = 1.0
    # C: row 127 ones (bias row of R)
    cmat = np.zeros((128, 128), np.float16)
    cmat[127, :T] = 1.0

    maps = []
    for c in range(N_CORES):
        osl = slice(OSH * c, OSH * (c + 1))
        aux = np.concatenate(
            [s128[:, osl], bmat[:, osl], amat, cmat], axis=1
        ).astype(np.float16)
        maps.append({
            "qw": np.ascontiguousarray(qwp[:, osl]),
            "xin": x2,
            "aux": np.ascontiguousarray(aux),
        })
    _CACHE["prep"] = (key, maps)
    return maps


def _get_nc(n_iters=1):
    key = ("nc", n_iters)
    if key not in _CACHE:
        _CACHE[key] = _build_nc(n_iters)
    return _CACHE[key]


def _gather(results):
    out = np.concatenate(
        [np.asarray(results[c]["out"]) for c in range(N_CORES)], axis=1
    )
    return np.ascontiguousarray(out.reshape(4, 32, O).astype(np.float32))


def run(inputs, trace=False, trace_cores=None):
    nc = _get_nc(1)
    maps = _prep_in_maps(**inputs)
    res = run_bass_kernel_spmd(nc, maps, list(range(N_CORES)), trace=trace,
                               trace_cores=trace_cores)
    return _gather(res.results), res


def kernel(**inputs) -> np.ndarray:
    out, _ = run(inputs, trace=False)
    return out


K_LO = 1
K_HI = 9


def _build_sharded(nc):
    import jax
    from jax.sharding import Mesh, PartitionSpec
    from jax.experimental.shard_map import shard_map
    from concourse import bass2jax
    import concourse.mybir as mb

    partition_name = nc.partition_id_tensor.name if nc.partition_id_tensor else None
    in_names, out_names, out_avals, zero_outs = [], [], [], []
    for alloc in nc.m.functions[0].allocations:
        if not isinstance(alloc, mb.MemoryLocationSet):
            continue
        name = alloc.memorylocations[0].name
        if alloc.kind == "ExternalInput":
            if name != partition_name:
                in_names.append(name)
        elif alloc.kind == "ExternalOutput":
            out_names.append(name)
            shape = tuple(alloc.tensor_shape)
            dtype = mb.dt.np(alloc.dtype)
            out_avals.append(jax.core.ShapedArray(shape, dtype))
            zero_outs.append(np.zeros(shape, dtype))
    n_params = len(in_names)
    in_names_all = in_names + out_names
    if partition_name is not None:
        in_names_all.append(partition_name)

    def _body(*args):
        operands = list(args)
        if partition_name is not None:
            operands.append(bass2jax.partition_id_tensor())
        outs = bass2jax._bass_exec_p.bind(
            *operands,
            out_avals=tuple(out_avals),
            in_names=tuple(in_names_all),
            out_names=tuple(out_names),
            lowering_input_output_aliases=(),
            sim_require_finite=True,
            sim_require_nnan=True,
            nc=nc,
        )
        return tuple(outs)

    devices = jax.devices()[:N_CORES]
    mesh = Mesh(np.asarray(devices), ("core",))
    n_outs = len(out_names)
    sharded = jax.jit(
        shard_map(
            _body, mesh=mesh,
            in_specs=(PartitionSpec("core"),) * (n_params + n_outs),
            out_specs=(PartitionSpec("core"),) * n_outs,
            check_rep=False,
        ),
        keep_unused=True,
    )
    return sharded, in_names, zero_outs


def bench(inputs, n_rep=8):
    """Time the K_LO- and K_HI-iteration unrolled programs; the slope
    isolates steady-state per-iteration device time from the (large, noisy)
    per-execute axon dispatch constant."""
    import time
    import jax
    from concourse import bass2jax

    bass2jax.install_neuronx_cc_hook()
    maps = _prep_in_maps(**inputs)

    runners = {}
    for k_it in (K_LO, K_HI):
        nc = _get_nc(k_it)
        sharded, in_names, zero_outs = _build_sharded(nc)
        concat_in = [
            np.concatenate([np.asarray(maps[c][nm]) for c in range(N_CORES)],
                           axis=0)
            for nm in in_names
        ]
        concat_zeros = [
            np.zeros((N_CORES * z.shape[0], *z.shape[1:]), z.dtype)
            for z in zero_outs
        ]
        args_dev = [jax.device_put(a) for a in concat_in + concat_zeros]
        outs = sharded(*args_dev)
        jax.block_until_ready(outs)
        runners[k_it] = (sharded, args_dev, outs)

    def timed(k_it):
        sharded, args_dev, _ = runners[k_it]
        o = sharded(*args_dev)
        jax.block_until_ready(o)
        t0 = time.time()
        o = sharded(*args_dev)
        jax.block_until_ready(o)
        return time.time() - t0

    lo = [timed(K_LO) for _ in range(n_rep)]
    hi = [timed(K_HI) for _ in range(n_rep)]
    per_iter_ns = (min(hi) - min(lo)) / (K_HI - K_LO) * 1e9

    outs = runners[K_HI][2]
    full = np.asarray(outs[0])          # [N_CORES*T, OSH] concat along axis0
    parts = [full[c * T : (c + 1) * T] for c in range(N_CORES)]
    out = np.concatenate(parts, axis=1).reshape(4, 32, O).astype(np.float32)
    return per_iter_ns, out, (min(lo), min(hi))


# revision 7
# speedup vs baseline: 1.5550x; 1.5550x over previous
"""AWQ 4-bit quantized linear (out = x @ dequant(qweight).T + bias), 8-core
tensor-parallel on TRN2.

Sharding: out_features split 8 ways (O' = 1024 per core); x replicated; each
core computes out[:, c*1024:(c+1)*1024] and the host concatenates.  No
device collectives.

Per-core layout (same trick as the 1-core baseline, O-sliced): qweight u16
rows are permuted so SBUF tile i holds u16-rows {16p + i : p in [0,128)}.
Input column c = 4r + k = 64p + (4i + k); the AWQ group of column c is
g = p//2, independent of (i,k), so one [128, O'] scale tile
s128[p,o] = scales[o, p//2] serves every dequant op, and the x-transposes
are plain stride-64 column slices of x.

Dequant per tile/plane: nib = qw & (15<<4k) (DVE tensor_scalar, 4x mode),
w = nib * s128 (tensor_tensor, 2x mode).  The 2^{4k} nibble-position factor
is compensated on the x side during PSUM->SBUF eviction (ACT).  A subset of
the w-multiplies runs on the GPSIMD (Pool) engine to unload DVE, which is
otherwise the bottleneck engine.

Zeros + bias fold into one extra K=128 matmul: R[p,t] = sum of raw x over
columns [64p, 64p+64) (recovered from the scaled transposes via identity-
matmul accumulation and a 16^k recombination), paired with
bmat[p,o] = -(s*z)[p//2, o]; row 127 of R is forced to 1.0 and bmat[127] =
bias (row 127's half-group sum is folded into row 126 by matrix A).

Benchmarking: the per-execute axon dispatch costs ~1 ms/core and swamps
wall-clock timing of single executions.  bench() therefore builds a second
Bass module with the whole per-iteration body unrolled K_HI times (each
iteration re-DMAs all inputs from DRAM and re-writes the output) and
measures the slope between the K_LO- and K_HI-iteration programs: the
marginal cost of one extra full computation on-device, i.e. the steady-state
HW execution time per iteration.
"""

import numpy as np
from contextlib import ExitStack

import concourse.bass as bass
import concourse.mybir as mybir
import concourse.tile as tile
from concourse.bass_utils import run_bass_kernel_spmd
from concourse.masks import make_identity

dt = mybir.dt

N_CORES = 8
I = 8192                    # in_features
O = 8192                    # out_features
OSH = O // N_CORES          # 1024 out-features per core
T = 128                     # batch*seq = 4*32
NG = 64                     # groups (group_size 128)
NR = 64                     # residue tiles (r64 = 4i + k)
NS = OSH // 512             # 512-wide matmul chunks per out block (2)

# (i, k) pairs whose dequant multiply runs on the Pool (GPSIMD) engine
# instead of DVE.  k=3 needs no AND (full-u16 plane) so it is the cheapest
# to offload; a few k=2 multiplies rebalance the remainder.
_POOL_MULS = {(i, 3) for i in range(16)} | {(i, 2) for i in (3, 7, 11, 15)}

_CACHE = {}


def _emit_iteration(nc, tc, P):
    """Emit one full computation: load everything from DRAM, dequantize,
    matmul, correct zeros+bias, store out."""
    qw_d, x_d, aux_d, out_d = P["qw_d"], P["x_d"], P["aux_d"], P["out_d"]
    ident = P["ident"]

    x_sb = P["x_p"].tile([T, I], dt.float16, tag="x")
    nc.scalar.dma_start(x_sb[:], x_d[:, :])
    s128 = P["cst_p"].tile([128, OSH], dt.float16, tag="s128")
    nc.scalar.dma_start(s128[:], aux_d[:, 0:OSH])
    bmat = P["cst_p"].tile([128, OSH], dt.float16, tag="bmat")
    nc.scalar.dma_start(bmat[:], aux_d[:, OSH : 2 * OSH])
    amat = P["sm_p"].tile([128, 128], dt.float16, tag="amat")
    nc.scalar.dma_start(amat[:], aux_d[:, 2 * OSH : 2 * OSH + 128])
    cmat = P["sm_p"].tile([128, T], dt.float16, tag="cmat")
    nc.scalar.dma_start(cmat[:], aux_d[:, 2 * OSH + 128 : 2 * OSH + 256])

    # ---- preamble: 64 transposes, batched 4-per-PSUM-tile ----
    x_r = x_sb.rearrange("t (p r) -> t r p", r=NR)
    xts, yts = {}, {}
    for k in range(4):
        for q in range(4):
            ps = P["pst_p"].tile([128, 4 * T], dt.float16, tag="tp")
            for m in range(4):
                r64 = 4 * (4 * q + m) + k
                nc.tensor.transpose(
                    ps[:, T * m : T * (m + 1)], x_r[:, r64, :], ident[:]
                )
            xt = P["xt_p"].tile([128, 4 * T], dt.float16, tag=f"xt{k}_{q}")
            if k == 0:
                nc.scalar.copy(xt[:], ps[:])
            else:
                nc.scalar.mul(xt[:], ps[:], float(2.0 ** (-4 * k)))
            xts[(k, q)] = xt

    # V-plane trick: plane k=3 is the FULL u16 value (w = v*s, no AND),
    # paired with a3; planes k<3 pair with y_k = a_k - a_3 so the extra
    # terms telescope away exactly.
    for k in range(3):
        for q in range(4):
            y = P["xt_p"].tile([128, 4 * T], dt.float16, tag=f"y{k}_{q}")
            nc.vector.tensor_tensor(
                out=y[:], in0=xts[(k, q)][:], in1=xts[(3, q)][:],
                op=mybir.AluOpType.subtract,
            )
            yts[(k, q)] = y

    # ---- half-group sums of raw x -> R ----
    psum_x = P["psx_p"].tile([128, 4 * T], dt.float32, tag="psx")
    for k in range(4):
        for q in range(4):
            for m in range(4):
                nc.tensor.matmul(
                    psum_x[:, T * k : T * (k + 1)],
                    amat[:],
                    xts[(k, q)][:, T * m : T * (m + 1)],
                    start=(q == 0 and m == 0),
                    stop=(q == 3 and m == 3),
                )
    t0 = P["sm_p"].tile([128, T], dt.float32, tag="t0")
    t1 = P["sm_p"].tile([128, T], dt.float32, tag="t1")
    t2 = P["sm_p"].tile([128, T], dt.float32, tag="t2")
    t3 = P["sm_p"].tile([128, T], dt.float32, tag="t3")
    rmat = P["sm_p"].tile([128, T], dt.float16, tag="rmat")
    nc.scalar.copy(t0[:], psum_x[:, 0:T])
    nc.vector.scalar_tensor_tensor(
        out=t1[:], in0=psum_x[:, T : 2 * T], scalar=16.0, in1=t0[:],
        op0=mybir.AluOpType.mult, op1=mybir.AluOpType.add,
    )
    nc.vector.scalar_tensor_tensor(
        out=t2[:], in0=psum_x[:, 2 * T : 3 * T], scalar=256.0, in1=t1[:],
        op0=mybir.AluOpType.mult, op1=mybir.AluOpType.add,
    )
    nc.vector.scalar_tensor_tensor(
        out=t3[:], in0=psum_x[:, 3 * T : 4 * T], scalar=4096.0, in1=t2[:],
        op0=mybir.AluOpType.mult, op1=mybir.AluOpType.add,
    )
    # row 127 (zeroed by A) becomes the bias row: rmat = t3 + C
    nc.vector.tensor_tensor(
        out=rmat[:], in0=t3[:], in1=cmat[:], op=mybir.AluOpType.add
    )

    # ---- main: 16 qw tiles x 4 nibble planes -> one [T, OSH] out block ----
    out_ps = P["pso_p"].tile([T, OSH], dt.float32, tag="out")
    for i in range(16):
        qwt = P["qwt_p"].tile([128, OSH], dt.uint16, tag="qwt")
        nc.sync.dma_start(qwt[:], qw_d[128 * i : 128 * (i + 1), :])
        q, m = i // 4, i % 4
        for k in range(4):
            w = P["w_p"].tile([128, OSH], dt.float16, tag="w")
            eng = nc.gpsimd if (i, k) in _POOL_MULS else nc.vector
            if k == 3:
                eng.tensor_tensor(
                    out=w[:], in0=qwt[:], in1=s128[:],
                    op=mybir.AluOpType.mult,
                )
                xt = xts[(3, q)]
            else:
                nib = P["nib_p"].tile([128, OSH], dt.uint16, tag="nib")
                nc.vector.tensor_scalar(
                    out=nib[:], in0=qwt[:], scalar1=15 << (4 * k),
                    scalar2=None, op0=mybir.AluOpType.bitwise_and,
                )
                eng.tensor_tensor(
                    out=w[:], in0=nib[:], in1=s128[:],
                    op=mybir.AluOpType.mult,
                )
                xt = yts[(k, q)]
            for ns in range(NS):
                nc.tensor.matmul(
                    out_ps[:, 512 * ns : 512 * (ns + 1)],
                    xt[:, T * m : T * (m + 1)],
                    w[:, 512 * ns : 512 * (ns + 1)],
                    start=(i == 0 and k == 0),
                    stop=False,
                )
    # zeros + bias correction
    for ns in range(NS):
        nc.tensor.matmul(
            out_ps[:, 512 * ns : 512 * (ns + 1)],
            rmat[:],
            bmat[:, 512 * ns : 512 * (ns + 1)],
            start=False, stop=(ns == NS - 1),
        )
    osb = P["osb_p"].tile([T, OSH], dt.float32, tag="osb")
    nc.scalar.copy(osb[:], out_ps[:])
    nc.scalar.dma_start(out_d[:, :], osb[:])


def _build_nc(n_iters=1, hw_loop=False):
    nc = bass.Bass()
    qw_d = nc.dram_tensor("qw", [2048, OSH], dt.uint16, kind="ExternalInput")
    x_d = nc.dram_tensor("xin", [T, I], dt.float16, kind="ExternalInput")
    # aux cols: 0:OSH s128 | OSH:2*OSH bmat | +128 amat | +128 cmat
    aux_d = nc.dram_tensor("aux", [128, 2 * OSH + 256], dt.float16,
                           kind="ExternalInput")
    out_d = nc.dram_tensor("out", [T, OSH], dt.float32, kind="ExternalOutput")

    with tile.TileContext(nc) as tc:
        with ExitStack() as ctx:
            P = {
                "qw_d": qw_d, "x_d": x_d, "aux_d": aux_d, "out_d": out_d,
                "x_p": ctx.enter_context(tc.tile_pool(name="x", bufs=2)),
                "cst_p": ctx.enter_context(tc.tile_pool(name="cst", bufs=2)),
                "sm_p": ctx.enter_context(tc.tile_pool(name="sm", bufs=2)),
                "xt_p": ctx.enter_context(tc.tile_pool(name="xt", bufs=2)),
                "qwt_p": ctx.enter_context(tc.tile_pool(name="qwt", bufs=6)),
                "nib_p": ctx.enter_context(tc.tile_pool(name="nib", bufs=4)),
                "w_p": ctx.enter_context(tc.tile_pool(name="w", bufs=6)),
                "osb_p": ctx.enter_context(tc.tile_pool(name="osb", bufs=2)),
                "sgl_p": ctx.enter_context(tc.tile_pool(name="sgl", bufs=1)),
                "pst_p": ctx.enter_context(
                    tc.tile_pool(name="pst", bufs=3, space="PSUM")),
                "psx_p": ctx.enter_context(
                    tc.tile_pool(name="psx", bufs=1, space="PSUM")),
                "pso_p": ctx.enter_context(
                    tc.tile_pool(name="pso", bufs=1, space="PSUM")),
            }
            ident = P["sgl_p"].tile([128, 128], dt.float16, tag="ident")
            make_identity(nc, ident[:])
            P["ident"] = ident
            if hw_loop:
                with tc.For_i(0, n_iters):
                    _emit_iteration(nc, tc, P)
            else:
                for _ in range(n_iters):
                    _emit_iteration(nc, tc, P)

    _split_excess_waits(nc)
    nc.finalize()
    return nc


_SPLIT_TYPES = {
    "InstTensorScalarPtr", "InstTensorTensor", "InstActivation", "InstMatmult",
    "InstDMACopy", "InstDmaTransposeAnt", "InstMemSet", "InstTensorCopy",
    "InstTensorReduce", "InstDrain", "InstMemset", "InstNoOp",
}

_ENG_MAP = {
    "DVE": "vector", "Activation": "scalar", "PE": "tensor",
    "Pool": "gpsimd", "SP": "sync",
}


def _split_excess_waits(nc):
    """walrus accepts at most one sync-wait per (non-drain) instruction in
    this build; move excess waits onto same-engine ENGINE_NOPs inserted just
    before the instruction."""
    for bb in nc.main_func.blocks:
        insts = list(bb.instructions)
        need = []  # (idx, inst, extra_waits)
        for idx, ins in enumerate(insts):
            if type(ins).__name__ not in _SPLIT_TYPES:
                continue
            si = ins.sync_info
            w = list(si.on_wait) if si else []
            if len(w) > 1:
                need.append((idx, ins, w))
        if not need:
            continue
        created = {}
        for idx, ins, w in need:
            eng = _ENG_MAP.get(ins.engine.name if ins.engine else "", "vector")
            nops = []
            for extra in w[:-1]:
                bi = getattr(nc, eng).nop()
                nop = bi.ins
                nop.sync_info = mybir.SyncInfo(on_wait=[extra], on_update=[])
                nops.append(nop)
            ins.sync_info = mybir.SyncInfo(
                on_wait=[w[-1]], on_update=list(ins.sync_info.on_update))
            created[idx] = nops
        nop_names = {n.name for nops in created.values() for n in nops}
        for bb2 in nc.main_func.blocks:
            cur = [i for i in bb2.instructions if i.name not in nop_names]
            if bb2.name == bb.name:
                out = []
                for idx, ins in enumerate(insts):
                    if idx in created:
                        out.extend(created[idx])
                    out.append(ins)
                bb2.instructions = out
            elif len(cur) != len(list(bb2.instructions)):
                bb2.instructions = cur


def _prep_in_maps(x, qweight, scales, qzeros, bias):
    """Host prep: repack qweight to the permuted-u16 layout and build the
    per-core aux tables.  Cached on input identity."""
    key = (id(x), id(qweight), id(scales), id(qzeros), id(bias))
    cached = _CACHE.get("prep")
    if cached is not None and cached[0] == key:
        return cached[1]

    x = np.asarray(x)
    qweight = np.asarray(qweight)
    scales = np.asarray(scales)
    qzeros = np.asarray(qzeros)
    bias = np.asarray(bias)

    x2 = np.ascontiguousarray(x.reshape(T, I))
    if x2.dtype != np.float16:
        x2 = x2.astype(np.float16)

    # qw tile i, partition p <- u16 row 16p+i of qweight.view(u16).T
    qw16 = qweight.view(np.uint16)              # [O, 2048] (nibble c=4r+k in row o)
    qwp = np.ascontiguousarray(
        qw16.reshape(O, 128, 16).transpose(2, 1, 0)
    ).reshape(2048, O)

    scT = scales.T.astype(np.float16)           # [64, O]
    rep = np.arange(128) // 2
    s128 = np.ascontiguousarray(scT[rep])       # [128, O]

    # zeros: znib[g, o] = nibble g%4 of u16 word g//4 of qzeros row o
    qz16 = qzeros.view(np.uint16)               # [O, 16]
    g = np.arange(NG)
    znib = (qz16.T[g // 4] >> (4 * (g % 4))[:, None]).astype(np.uint16) & 15
    bm_half = -(scT.astype(np.float32) * znib.astype(np.float32))  # [64, O]
    bmat = bm_half[rep].astype(np.float16)
    bmat[127] = bias.astype(np.float16)

    # A matrix: identity, col 126 also sums row 127, col 127 dead
    amat = np.eye(128, dtype=np.float16)
    amat[127, 127] = 0.0
    amat[127, 126] = 1.0
    # C: row 127 ones (bias row of R)
    cmat = np.zeros((128, 128), np.float16)
    cmat[127, :T] = 1.0

    maps = []
    for c in range(N_CORES):
        osl = slice(OSH * c, OSH * (c + 1))
        aux = np.concatenate(
            [s128[:, osl], bmat[:, osl], amat, cmat], axis=1
        ).astype(np.float16)
        maps.append({
            "qw": np.ascontiguousarray(qwp[:, osl]),
            "xin": x2,
            "aux": np.ascontiguousarray(aux),
        })
    _CACHE["prep"] = (key, maps)
    return maps


def _get_nc(n_iters=1, hw_loop=False):
    key = ("nc", n_iters, hw_loop)
    if key not in _CACHE:
        _CACHE[key] = _build_nc(n_iters, hw_loop)
    return _CACHE[key]


def _gather(results):
    out = np.concatenate(
        [np.asarray(results[c]["out"]) for c in range(N_CORES)], axis=1
    )
    return np.ascontiguousarray(out.reshape(4, 32, O).astype(np.float32))


def run(inputs, trace=False, trace_cores=None):
    nc = _get_nc(1)
    maps = _prep_in_maps(**inputs)
    res = run_bass_kernel_spmd(nc, maps, list(range(N_CORES)), trace=trace,
                               trace_cores=trace_cores)
    return _gather(res.results), res


def kernel(**inputs) -> np.ndarray:
    out, _ = run(inputs, trace=False)
    return out


K_LO = 8
K_HI = 808


def _build_sharded(nc):
    import jax
    from jax.sharding import Mesh, PartitionSpec
    from jax.experimental.shard_map import shard_map
    from concourse import bass2jax
    import concourse.mybir as mb

    partition_name = nc.partition_id_tensor.name if nc.partition_id_tensor else None
    in_names, out_names, out_avals, zero_outs = [], [], [], []
    for alloc in nc.m.functions[0].allocations:
        if not isinstance(alloc, mb.MemoryLocationSet):
            continue
        name = alloc.memorylocations[0].name
        if alloc.kind == "ExternalInput":
            if name != partition_name:
                in_names.append(name)
        elif alloc.kind == "ExternalOutput":
            out_names.append(name)
            shape = tuple(alloc.tensor_shape)
            dtype = mb.dt.np(alloc.dtype)
            out_avals.append(jax.core.ShapedArray(shape, dtype))
            zero_outs.append(np.zeros(shape, dtype))
    n_params = len(in_names)
    in_names_all = in_names + out_names
    if partition_name is not None:
        in_names_all.append(partition_name)

    def _body(*args):
        operands = list(args)
        if partition_name is not None:
            operands.append(bass2jax.partition_id_tensor())
        outs = bass2jax._bass_exec_p.bind(
            *operands,
            out_avals=tuple(out_avals),
            in_names=tuple(in_names_all),
            out_names=tuple(out_names),
            lowering_input_output_aliases=(),
            sim_require_finite=True,
            sim_require_nnan=True,
            nc=nc,
        )
        return tuple(outs)

    devices = jax.devices()[:N_CORES]
    mesh = Mesh(np.asarray(devices), ("core",))
    n_outs = len(out_names)
    sharded = jax.jit(
        shard_map(
            _body, mesh=mesh,
            in_specs=(PartitionSpec("core"),) * (n_params + n_outs),
            out_specs=(PartitionSpec("core"),) * n_outs,
            check_rep=False,
        ),
        keep_unused=True,
    )
    return sharded, in_names, zero_outs


def bench(inputs, n_rep=8):
    """Time the K_LO- and K_HI-iteration unrolled programs; the slope
    isolates steady-state per-iteration device time from the (large, noisy)
    per-execute axon dispatch constant."""
    import time
    import jax
    from concourse import bass2jax

    bass2jax.install_neuronx_cc_hook()
    maps = _prep_in_maps(**inputs)

    runners = {}
    for k_it in (K_LO, K_HI):
        nc = _get_nc(k_it, hw_loop=True)
        sharded, in_names, zero_outs = _build_sharded(nc)
        concat_in = [
            np.concatenate([np.asarray(maps[c][nm]) for c in range(N_CORES)],
                           axis=0)
            for nm in in_names
        ]
        concat_zeros = [
            np.zeros((N_CORES * z.shape[0], *z.shape[1:]), z.dtype)
            for z in zero_outs
        ]
        args_dev = [jax.device_put(a) for a in concat_in + concat_zeros]
        outs = sharded(*args_dev)
        jax.block_until_ready(outs)
        runners[k_it] = (sharded, args_dev, outs)

    def timed(k_it):
        sharded, args_dev, _ = runners[k_it]
        o = sharded(*args_dev)
        jax.block_until_ready(o)
        t0 = time.time()
        o = sharded(*args_dev)
        jax.block_until_ready(o)
        return time.time() - t0

    lo = [timed(K_LO) for _ in range(n_rep)]
    hi = [timed(K_HI) for _ in range(n_rep)]
    per_iter_ns = (min(hi) - min(lo)) / (K_HI - K_LO) * 1e9

    outs = runners[K_HI][2]
    full = np.asarray(outs[0])          # [N_CORES*T, OSH] concat along axis0
    parts = [full[c * T : (c + 1) * T] for c in range(N_CORES)]
    out = np.concatenate(parts, axis=1).reshape(4, 32, O).astype(np.float32)
    return per_iter_ns, out, (min(lo), min(hi))


# revision 9
# speedup vs baseline: 2.1250x; 1.3665x over previous
"""AWQ 4-bit quantized linear (out = x @ dequant(qweight).T + bias), 8-core
tensor-parallel on TRN2.

Sharding: out_features split 8 ways (O' = 1024 per core); x replicated; each
core computes out[:, c*1024:(c+1)*1024] and the host concatenates.  No
device collectives.

Per-core layout (same trick as the 1-core baseline, O-sliced): qweight u16
rows are permuted so SBUF tile i holds u16-rows {16p + i : p in [0,128)}.
Input column c = 4r + k = 64p + (4i + k); the AWQ group of column c is
g = p//2, independent of (i,k), so one [128, O'] scale tile
s128[p,o] = scales[o, p//2] serves every dequant op, and the x-transposes
are plain stride-64 column slices of x.

Dequant per tile/plane: nib = qw & (15<<4k) (DVE tensor_scalar, 4x mode),
w = nib * s128 (tensor_tensor, 2x mode).  The 2^{4k} nibble-position factor
is compensated on the x side during PSUM->SBUF eviction (ACT).  A subset of
the w-multiplies runs on the GPSIMD (Pool) engine to unload DVE, which is
otherwise the bottleneck engine.

Zeros + bias fold into one extra K=128 matmul: R[p,t] = sum of raw x over
columns [64p, 64p+64) (recovered from the scaled transposes via identity-
matmul accumulation and a 16^k recombination), paired with
bmat[p,o] = -(s*z)[p//2, o]; row 127 of R is forced to 1.0 and bmat[127] =
bias (row 127's half-group sum is folded into row 126 by matrix A).

Benchmarking: the per-execute axon dispatch costs ~1 ms/core and swamps
wall-clock timing of single executions.  bench() therefore builds a second
Bass module with the whole per-iteration body unrolled K_HI times (each
iteration re-DMAs all inputs from DRAM and re-writes the output) and
measures the slope between the K_LO- and K_HI-iteration programs: the
marginal cost of one extra full computation on-device, i.e. the steady-state
HW execution time per iteration.
"""

import numpy as np
from contextlib import ExitStack

import concourse.bass as bass
import concourse.mybir as mybir
import concourse.tile as tile
from concourse.bass_utils import run_bass_kernel_spmd
from concourse.masks import make_identity

dt = mybir.dt

N_CORES = 8
I = 8192                    # in_features
O = 8192                    # out_features
OSH = O // N_CORES          # 1024 out-features per core
T = 128                     # batch*seq = 4*32
NG = 64                     # groups (group_size 128)
NR = 64                     # residue tiles (r64 = 4i + k)
NS = OSH // 512             # 512-wide matmul chunks per out block (2)

_CACHE = {}

_KORD = (3, 0, 1, 2)        # k=3 (v-plane, no AND) first so DVE starts early


def _emit_iteration(nc, tc, P):
    """Emit one full computation: load everything from DRAM, dequantize,
    matmul, correct zeros+bias, store out.

    qw tiles are processed in PAIRS (one [128, 2*OSH] SBUF tile covering
    original tiles 2j and 2j+1) so each DVE dequant op handles 2*OSH
    elements, halving per-instruction overhead.  The scale operand is a
    doubled tile s128d = s128|s128.  DVE emission order is chosen to avoid
    head-of-line blocking on the in-order engine: the k=3 multiplies of the
    first two pairs (which need only qw + scales) run before the y-plane
    subtracts (which need the x transposes)."""
    qw_d, x_d, aux_d, out_d = P["qw_d"], P["x_d"], P["aux_d"], P["out_d"]
    ident = P["ident"]

    x_sb = P["x_p"].tile([T, I], dt.float16, tag="x")
    nc.scalar.dma_start(x_sb[:], x_d[:, :])
    s128d = P["cst_p"].tile([128, 2 * OSH], dt.float16, tag="s128d")
    nc.scalar.dma_start(s128d[:, 0:OSH], aux_d[:, 0:OSH])
    nc.scalar.dma_start(s128d[:, OSH : 2 * OSH], aux_d[:, 0:OSH])
    bmat = P["cst_p"].tile([128, OSH], dt.float16, tag="bmat")
    nc.scalar.dma_start(bmat[:], aux_d[:, OSH : 2 * OSH])
    amat = P["sm_p"].tile([128, 128], dt.float16, tag="amat")
    nc.scalar.dma_start(amat[:], aux_d[:, 2 * OSH : 2 * OSH + 128])
    cmat = P["sm_p"].tile([128, T], dt.float16, tag="cmat")
    nc.scalar.dma_start(cmat[:], aux_d[:, 2 * OSH + 128 : 2 * OSH + 256])

    # ---- early DVE work: k=3 dequant of pairs 0..1 needs only qw+scales ----
    qwts, w3s = {}, {}
    for j in range(2):
        qwt = P["qwt_p"].tile([128, 2 * OSH], dt.uint16, tag="qwt")
        nc.sync.dma_start(qwt[:, 0:OSH], qw_d[256 * j : 256 * j + 128, :])
        nc.sync.dma_start(qwt[:, OSH : 2 * OSH],
                          qw_d[256 * j + 128 : 256 * j + 256, :])
        qwts[j] = qwt
        w3 = P["w_p"].tile([128, 2 * OSH], dt.float16, tag="w")
        nc.vector.tensor_tensor(out=w3[:], in0=qwt[:], in1=s128d[:],
                                op=mybir.AluOpType.mult)
        w3s[j] = w3

    # ---- preamble: 64 transposes, 4-per-PSUM-tile, evicted into one
    # [128, 2048] tile per nibble plane k (quarter q at 512*q) ----
    x_r = x_sb.rearrange("t (p r) -> t r p", r=NR)
    xts, yts = {}, {}
    for k in range(4):
        xt = P["xt_p"].tile([128, 4 * 4 * T], dt.float16, tag=f"xt{k}")
        for q in range(4):
            ps = P["pst_p"].tile([128, 4 * T], dt.float16, tag="tp")
            for m in range(4):
                r64 = 4 * (4 * q + m) + k
                nc.tensor.transpose(
                    ps[:, T * m : T * (m + 1)], x_r[:, r64, :], ident[:]
                )
            if k == 0:
                nc.scalar.copy(xt[:, 512 * q : 512 * (q + 1)], ps[:])
            else:
                nc.scalar.mul(xt[:, 512 * q : 512 * (q + 1)], ps[:],
                              float(2.0 ** (-4 * k)))
        xts[k] = xt

    # V-plane trick: plane k=3 is the FULL u16 value (w = v*s, no AND),
    # paired with a3; planes k<3 pair with y_k = a_k - a_3 so the extra
    # terms telescope away exactly.
    for k in range(3):
        y = P["xt_p"].tile([128, 4 * 4 * T], dt.float16, tag=f"y{k}")
        nc.vector.tensor_tensor(out=y[:], in0=xts[k][:], in1=xts[3][:],
                                op=mybir.AluOpType.subtract)
        yts[k] = y

    def xsl(k, i):
        q, m = i // 4, i % 4
        src = xts[3] if k == 3 else yts[k]
        return src[:, 512 * q + T * m : 512 * q + T * (m + 1)]

    # ---- half-group sums of raw x -> R (PE) ----
    psum_x = P["psx_p"].tile([128, 4 * T], dt.float32, tag="psx")
    for k in range(4):
        for q in range(4):
            for m in range(4):
                nc.tensor.matmul(
                    psum_x[:, T * k : T * (k + 1)],
                    amat[:],
                    xts[k][:, 512 * q + T * m : 512 * q + T * (m + 1)],
                    start=(q == 0 and m == 0),
                    stop=(q == 3 and m == 3),
                )

    # ---- main: 8 qw pairs x 4 nibble planes -> one [T, OSH] out block ----
    out_ps = P["pso_p"].tile([T, OSH], dt.float32, tag="out")

    def emit_pair(j):
        if j in qwts:
            qwt = qwts[j]
        else:
            qwt = P["qwt_p"].tile([128, 2 * OSH], dt.uint16, tag="qwt")
            nc.sync.dma_start(qwt[:, 0:OSH], qw_d[256 * j : 256 * j + 128, :])
            nc.sync.dma_start(qwt[:, OSH : 2 * OSH],
                              qw_d[256 * j + 128 : 256 * j + 256, :])
        for k in _KORD:
            if k == 3:
                if j in w3s:
                    w = w3s[j]
                else:
                    w = P["w_p"].tile([128, 2 * OSH], dt.float16, tag="w")
                    nc.vector.tensor_tensor(out=w[:], in0=qwt[:],
                                            in1=s128d[:],
                                            op=mybir.AluOpType.mult)
            else:
                nib = P["nib_p"].tile([128, 2 * OSH], dt.uint16, tag="nib")
                nc.vector.tensor_scalar(
                    out=nib[:], in0=qwt[:], scalar1=15 << (4 * k),
                    scalar2=None, op0=mybir.AluOpType.bitwise_and,
                )
                w = P["w_p"].tile([128, 2 * OSH], dt.float16, tag="w")
                nc.vector.tensor_tensor(out=w[:], in0=nib[:], in1=s128d[:],
                                        op=mybir.AluOpType.mult)
            for h in range(2):
                i = 2 * j + h
                for ns in range(NS):
                    nc.tensor.matmul(
                        out_ps[:, 512 * ns : 512 * (ns + 1)],
                        xsl(k, i),
                        w[:, OSH * h + 512 * ns : OSH * h + 512 * (ns + 1)],
                        start=(j == 0 and k == 3 and h == 0),
                        stop=False,
                    )

    for j in range(5):
        emit_pair(j)

    # ---- R recombination (DVE, emitted mid-stream so rmat is ready well
    # before the correction matmul without blocking early dequant) ----
    t0 = P["sm_p"].tile([128, T], dt.float32, tag="t0")
    t1 = P["sm_p"].tile([128, T], dt.float32, tag="t1")
    t2 = P["sm_p"].tile([128, T], dt.float32, tag="t2")
    t3 = P["sm_p"].tile([128, T], dt.float32, tag="t3")
    rmat = P["sm_p"].tile([128, T], dt.float16, tag="rmat")
    nc.scalar.copy(t0[:], psum_x[:, 0:T])
    nc.vector.scalar_tensor_tensor(
        out=t1[:], in0=psum_x[:, T : 2 * T], scalar=16.0, in1=t0[:],
        op0=mybir.AluOpType.mult, op1=mybir.AluOpType.add,
    )
    nc.vector.scalar_tensor_tensor(
        out=t2[:], in0=psum_x[:, 2 * T : 3 * T], scalar=256.0, in1=t1[:],
        op0=mybir.AluOpType.mult, op1=mybir.AluOpType.add,
    )
    nc.vector.scalar_tensor_tensor(
        out=t3[:], in0=psum_x[:, 3 * T : 4 * T], scalar=4096.0, in1=t2[:],
        op0=mybir.AluOpType.mult, op1=mybir.AluOpType.add,
    )
    # row 127 (zeroed by A) becomes the bias row: rmat = t3 + C
    nc.vector.tensor_tensor(
        out=rmat[:], in0=t3[:], in1=cmat[:], op=mybir.AluOpType.add
    )

    for j in range(5, 8):
        emit_pair(j)

    # zeros + bias correction
    for ns in range(NS):
        nc.tensor.matmul(
            out_ps[:, 512 * ns : 512 * (ns + 1)],
            rmat[:],
            bmat[:, 512 * ns : 512 * (ns + 1)],
            start=False, stop=(ns == NS - 1),
        )
    osb = P["osb_p"].tile([T, OSH], dt.float32, tag="osb")
    nc.scalar.copy(osb[:], out_ps[:])
    nc.scalar.dma_start(out_d[:, :], osb[:])


def _build_nc(n_iters=1, hw_loop=False):
    nc = bass.Bass()
    qw_d = nc.dram_tensor("qw", [2048, OSH], dt.uint16, kind="ExternalInput")
    x_d = nc.dram_tensor("xin", [T, I], dt.float16, kind="ExternalInput")
    # aux cols: 0:OSH s128 | OSH:2*OSH bmat | +128 amat | +128 cmat
    aux_d = nc.dram_tensor("aux", [128, 2 * OSH + 256], dt.float16,
                           kind="ExternalInput")
    out_d = nc.dram_tensor("out", [T, OSH], dt.float32, kind="ExternalOutput")

    with tile.TileContext(nc) as tc:
        with ExitStack() as ctx:
            P = {
                "qw_d": qw_d, "x_d": x_d, "aux_d": aux_d, "out_d": out_d,
                "x_p": ctx.enter_context(tc.tile_pool(name="x", bufs=2)),
                "cst_p": ctx.enter_context(tc.tile_pool(name="cst", bufs=2)),
                "sm_p": ctx.enter_context(tc.tile_pool(name="sm", bufs=2)),
                "xt_p": ctx.enter_context(tc.tile_pool(name="xt", bufs=2)),
                "qwt_p": ctx.enter_context(tc.tile_pool(name="qwt", bufs=3)),
                "nib_p": ctx.enter_context(tc.tile_pool(name="nib", bufs=3)),
                "w_p": ctx.enter_context(tc.tile_pool(name="w", bufs=5)),
                "osb_p": ctx.enter_context(tc.tile_pool(name="osb", bufs=2)),
                "sgl_p": ctx.enter_context(tc.tile_pool(name="sgl", bufs=1)),
                "pst_p": ctx.enter_context(
                    tc.tile_pool(name="pst", bufs=3, space="PSUM")),
                "psx_p": ctx.enter_context(
                    tc.tile_pool(name="psx", bufs=1, space="PSUM")),
                "pso_p": ctx.enter_context(
                    tc.tile_pool(name="pso", bufs=1, space="PSUM")),
            }
            ident = P["sgl_p"].tile([128, 128], dt.float16, tag="ident")
            make_identity(nc, ident[:])
            P["ident"] = ident
            if hw_loop:
                with tc.For_i(0, n_iters):
                    _emit_iteration(nc, tc, P)
            else:
                for _ in range(n_iters):
                    _emit_iteration(nc, tc, P)

    _split_excess_waits(nc)
    nc.finalize()
    return nc


_SPLIT_TYPES = {
    "InstTensorScalarPtr", "InstTensorTensor", "InstActivation", "InstMatmult",
    "InstDMACopy", "InstDmaTransposeAnt", "InstMemSet", "InstTensorCopy",
    "InstTensorReduce", "InstDrain", "InstMemset", "InstNoOp",
}

_ENG_MAP = {
    "DVE": "vector", "Activation": "scalar", "PE": "tensor",
    "Pool": "gpsimd", "SP": "sync",
}


def _split_excess_waits(nc):
    """walrus accepts at most one sync-wait per (non-drain) instruction in
    this build; move excess waits onto same-engine ENGINE_NOPs inserted just
    before the instruction."""
    for bb in nc.main_func.blocks:
        insts = list(bb.instructions)
        need = []  # (idx, inst, extra_waits)
        for idx, ins in enumerate(insts):
            if type(ins).__name__ not in _SPLIT_TYPES:
                continue
            si = ins.sync_info
            w = list(si.on_wait) if si else []
            if len(w) > 1:
                need.append((idx, ins, w))
        if not need:
            continue
        created = {}
        for idx, ins, w in need:
            eng = _ENG_MAP.get(ins.engine.name if ins.engine else "", "vector")
            nops = []
            for extra in w[:-1]:
                bi = getattr(nc, eng).nop()
                nop = bi.ins
                nop.sync_info = mybir.SyncInfo(on_wait=[extra], on_update=[])
                nops.append(nop)
            ins.sync_info = mybir.SyncInfo(
                on_wait=[w[-1]], on_update=list(ins.sync_info.on_update))
            created[idx] = nops
        nop_names = {n.name for nops in created.values() for n in nops}
        for bb2 in nc.main_func.blocks:
            cur = [i for i in bb2.instructions if i.name not in nop_names]
            if bb2.name == bb.name:
                out = []
                for idx, ins in enumerate(insts):
                    if idx in created:
                        out.extend(created[idx])
                    out.append(ins)
                bb2.instructions = out
            elif len(cur) != len(list(bb2.instructions)):
                bb2.instructions = cur


def _prep_in_maps(x, qweight, scales, qzeros, bias):
    """Host prep: repack qweight to the permuted-u16 layout and build the
    per-core aux tables.  Cached on input identity."""
    key = (id(x), id(qweight), id(scales), id(qzeros), id(bias))
    cached = _CACHE.get("prep")
    if cached is not None and cached[0] == key:
        return cached[1]

    x = np.asarray(x)
    qweight = np.asarray(qweight)
    scales = np.asarray(scales)
    qzeros = np.asarray(qzeros)
    bias = np.asarray(bias)

    x2 = np.ascontiguousarray(x.reshape(T, I))
    if x2.dtype != np.float16:
        x2 = x2.astype(np.float16)

    # qw tile i, partition p <- u16 row 16p+i of qweight.view(u16).T
    qw16 = qweight.view(np.uint16)              # [O, 2048] (nibble c=4r+k in row o)
    qwp = np.ascontiguousarray(
        qw16.reshape(O, 128, 16).transpose(2, 1, 0)
    ).reshape(2048, O)

    scT = scales.T.astype(np.float16)           # [64, O]
    rep = np.arange(128) // 2
    s128 = np.ascontiguousarray(scT[rep])       # [128, O]

    # zeros: znib[g, o] = nibble g%4 of u16 word g//4 of qzeros row o
    qz16 = qzeros.view(np.uint16)               # [O, 16]
    g = np.arange(NG)
    znib = (qz16.T[g // 4] >> (4 * (g % 4))[:, None]).astype(np.uint16) & 15
    bm_half = -(scT.astype(np.float32) * znib.astype(np.float32))  # [64, O]
    bmat = bm_half[rep].astype(np.float16)
    bmat[127] = bias.astype(np.float16)

    # A matrix: identity, col 126 also sums row 127, col 127 dead
    amat = np.eye(128, dtype=np.float16)
    amat[127, 127] = 0.0
    amat[127, 126] = 1.0
    # C: row 127 ones (bias row of R)
    cmat = np.zeros((128, 128), np.float16)
    cmat[127, :T] = 1.0

    maps = []
    for c in range(N_CORES):
        osl = slice(OSH * c, OSH * (c + 1))
        aux = np.concatenate(
            [s128[:, osl], bmat[:, osl], amat, cmat], axis=1
        ).astype(np.float16)
        maps.append({
            "qw": np.ascontiguousarray(qwp[:, osl]),
            "xin": x2,
            "aux": np.ascontiguousarray(aux),
        })
    _CACHE["prep"] = (key, maps)
    return maps


def _get_nc(n_iters=1, hw_loop=False):
    key = ("nc", n_iters, hw_loop)
    if key not in _CACHE:
        _CACHE[key] = _build_nc(n_iters, hw_loop)
    return _CACHE[key]


def _gather(results):
    out = np.concatenate(
        [np.asarray(results[c]["out"]) for c in range(N_CORES)], axis=1
    )
    return np.ascontiguousarray(out.reshape(4, 32, O).astype(np.float32))


def run(inputs, trace=False, trace_cores=None):
    nc = _get_nc(1)
    maps = _prep_in_maps(**inputs)
    res = run_bass_kernel_spmd(nc, maps, list(range(N_CORES)), trace=trace,
                               trace_cores=trace_cores)
    return _gather(res.results), res


def kernel(**inputs) -> np.ndarray:
    out, _ = run(inputs, trace=False)
    return out


K_LO = 8
K_HI = 808


def _build_sharded(nc):
    import jax
    from jax.sharding import Mesh, PartitionSpec
    from jax.experimental.shard_map import shard_map
    from concourse import bass2jax
    import concourse.mybir as mb

    partition_name = nc.partition_id_tensor.name if nc.partition_id_tensor else None
    in_names, out_names, out_avals, zero_outs = [], [], [], []
    for alloc in nc.m.functions[0].allocations:
        if not isinstance(alloc, mb.MemoryLocationSet):
            continue
        name = alloc.memorylocations[0].name
        if alloc.kind == "ExternalInput":
            if name != partition_name:
                in_names.append(name)
        elif alloc.kind == "ExternalOutput":
            out_names.append(name)
            shape = tuple(alloc.tensor_shape)
            dtype = mb.dt.np(alloc.dtype)
            out_avals.append(jax.core.ShapedArray(shape, dtype))
            zero_outs.append(np.zeros(shape, dtype))
    n_params = len(in_names)
    in_names_all = in_names + out_names
    if partition_name is not None:
        in_names_all.append(partition_name)

    def _body(*args):
        operands = list(args)
        if partition_name is not None:
            operands.append(bass2jax.partition_id_tensor())
        outs = bass2jax._bass_exec_p.bind(
            *operands,
            out_avals=tuple(out_avals),
            in_names=tuple(in_names_all),
            out_names=tuple(out_names),
            lowering_input_output_aliases=(),
            sim_require_finite=True,
            sim_require_nnan=True,
            nc=nc,
        )
        return tuple(outs)

    devices = jax.devices()[:N_CORES]
    mesh = Mesh(np.asarray(devices), ("core",))
    n_outs = len(out_names)
    sharded = jax.jit(
        shard_map(
            _body, mesh=mesh,
            in_specs=(PartitionSpec("core"),) * (n_params + n_outs),
            out_specs=(PartitionSpec("core"),) * n_outs,
            check_rep=False,
        ),
        keep_unused=True,
    )
    return sharded, in_names, zero_outs


def bench(inputs, n_rep=8):
    """Time the K_LO- and K_HI-iteration unrolled programs; the slope
    isolates steady-state per-iteration device time from the (large, noisy)
    per-execute axon dispatch constant."""
    import time
    import jax
    from concourse import bass2jax

    bass2jax.install_neuronx_cc_hook()
    maps = _prep_in_maps(**inputs)

    runners = {}
    for k_it in (K_LO, K_HI):
        nc = _get_nc(k_it, hw_loop=True)
        sharded, in_names, zero_outs = _build_sharded(nc)
        concat_in = [
            np.concatenate([np.asarray(maps[c][nm]) for c in range(N_CORES)],
                           axis=0)
            for nm in in_names
        ]
        concat_zeros = [
            np.zeros((N_CORES * z.shape[0], *z.shape[1:]), z.dtype)
            for z in zero_outs
        ]
        args_dev = [jax.device_put(a) for a in concat_in + concat_zeros]
        outs = sharded(*args_dev)
        jax.block_until_ready(outs)
        runners[k_it] = (sharded, args_dev, outs)

    def timed(k_it):
        sharded, args_dev, _ = runners[k_it]
        o = sharded(*args_dev)
        jax.block_until_ready(o)
        t0 = time.time()
        o = sharded(*args_dev)
        jax.block_until_ready(o)
        return time.time() - t0

    lo = [timed(K_LO) for _ in range(n_rep)]
    hi = [timed(K_HI) for _ in range(n_rep)]
    per_iter_ns = (min(hi) - min(lo)) / (K_HI - K_LO) * 1e9

    outs = runners[K_HI][2]
    full = np.asarray(outs[0])          # [N_CORES*T, OSH] concat along axis0
    parts = [full[c * T : (c + 1) * T] for c in range(N_CORES)]
    out = np.concatenate(parts, axis=1).reshape(4, 32, O).astype(np.float32)
    return per_iter_ns, out, (min(lo), min(hi))


# revision 13
# speedup vs baseline: 2.8197x; 1.3269x over previous
"""AWQ 4-bit quantized linear (out = x @ dequant(qweight).T + bias), 8-core
tensor-parallel on TRN2.

Sharding: out_features split 8 ways (O' = 1024 per core); x replicated; each
core computes out[:, c*1024:(c+1)*1024] and the host concatenates.  No
device collectives.

Per-core layout (same trick as the 1-core baseline, O-sliced): qweight u16
rows are permuted so SBUF tile i holds u16-rows {16p + i : p in [0,128)}.
Input column c = 4r + k = 64p + (4i + k); the AWQ group of column c is
g = p//2, independent of (i,k), so one [128, O'] scale tile
s128[p,o] = scales[o, p//2] serves every dequant op, and the x-transposes
are plain stride-64 column slices of x.

Dequant per tile/plane: nib = qw & (15<<4k) (DVE tensor_scalar, 4x mode),
w = nib * s128 (tensor_tensor, 2x mode).  The 2^{4k} nibble-position factor
is compensated on the x side during PSUM->SBUF eviction (ACT).  A subset of
the w-multiplies runs on the GPSIMD (Pool) engine to unload DVE, which is
otherwise the bottleneck engine.

Zeros + bias fold into one extra K=128 matmul: R[p,t] = sum of raw x over
columns [64p, 64p+64) (recovered from the scaled transposes via identity-
matmul accumulation and a 16^k recombination), paired with
bmat[p,o] = -(s*z)[p//2, o]; row 127 of R is forced to 1.0 and bmat[127] =
bias (row 127's half-group sum is folded into row 126 by matrix A).

Benchmarking: the per-execute axon dispatch costs ~1 ms/core and swamps
wall-clock timing of single executions.  bench() therefore builds a second
Bass module with the whole per-iteration body unrolled K_HI times (each
iteration re-DMAs all inputs from DRAM and re-writes the output) and
measures the slope between the K_LO- and K_HI-iteration programs: the
marginal cost of one extra full computation on-device, i.e. the steady-state
HW execution time per iteration.
"""

import numpy as np
from contextlib import ExitStack

import concourse.bass as bass
import concourse.mybir as mybir
import concourse.tile as tile
from concourse.bass_utils import run_bass_kernel_spmd
from concourse.masks import make_identity

dt = mybir.dt

N_CORES = 8
I = 8192                    # in_features
O = 8192                    # out_features
OSH = O // N_CORES          # 1024 out-features per core
T = 128                     # batch*seq = 4*32
NG = 64                     # groups (group_size 128)
NR = 64                     # residue tiles (r64 = 4i + k)
NS = OSH // 512             # 512-wide matmul chunks per out block (2)

_CACHE = {}

_KORD = (3, 0, 1, 2)        # k=3 (v-plane, no AND) first so DVE starts early


def _emit_iteration(nc, tc, P):
    """Emit one full computation: load everything from DRAM, dequantize,
    matmul, correct zeros+bias, store out.

    qw tiles are processed in PAIRS (one [128, 2*OSH] SBUF tile covering
    original tiles 2j and 2j+1) so each DVE dequant op handles 2*OSH
    elements, halving per-instruction overhead.  The scale operand is a
    doubled tile s128d = s128|s128.  DVE emission order is chosen to avoid
    head-of-line blocking on the in-order engine: the k=3 multiplies of the
    first two pairs (which need only qw + scales) run before the y-plane
    subtracts (which need the x transposes)."""
    qw_d, x_d, aux_d, out_d = P["qw_d"], P["x_d"], P["aux_d"], P["out_d"]
    ident = P["ident"]

    x_sb = P["x_p"].tile([T, I], dt.float16, tag="x")
    nc.scalar.dma_start(x_sb[:], x_d[:, :])
    s128d = P["cst_p"].tile([128, 2 * OSH], dt.float16, tag="s128d")
    nc.scalar.dma_start(s128d[:, 0:OSH], aux_d[:, 0:OSH])
    nc.scalar.dma_start(s128d[:, OSH : 2 * OSH], aux_d[:, 0:OSH])
    bmat = P["cst_p"].tile([128, OSH], dt.float16, tag="bmat")
    nc.scalar.dma_start(bmat[:], aux_d[:, OSH : 2 * OSH])
    amat = P["sm_p"].tile([128, 128], dt.float16, tag="amat")
    nc.scalar.dma_start(amat[:], aux_d[:, 2 * OSH : 2 * OSH + 128])
    cmat = P["sm_p"].tile([128, T], dt.float16, tag="cmat")
    nc.scalar.dma_start(cmat[:], aux_d[:, 2 * OSH + 128 : 2 * OSH + 256])

    # ---- early DVE work: k=3 dequant of pairs 0..1 needs only qw+scales ----
    qwts, w3s = {}, {}
    for j in range(2):
        qwt = P["qwt_p"].tile([128, 2 * OSH], dt.uint16, tag="qwt")
        nc.sync.dma_start(qwt[:, 0:OSH], qw_d[256 * j : 256 * j + 128, :])
        nc.sync.dma_start(qwt[:, OSH : 2 * OSH],
                          qw_d[256 * j + 128 : 256 * j + 256, :])
        qwts[j] = qwt
        w3 = P["w_p"].tile([128, 2 * OSH], dt.float16, tag="w")
        nc.vector.tensor_tensor(out=w3[:], in0=qwt[:], in1=s128d[:],
                                op=mybir.AluOpType.mult)
        w3s[j] = w3

    # ---- preamble: 64 transposes, 4-per-PSUM-tile, evicted into one
    # [128, 2048] tile per nibble plane k (quarter q at 512*q) ----
    x_r = x_sb.rearrange("t (p r) -> t r p", r=NR)
    xts, yts = {}, {}
    for k in range(4):
        xt = P["xt_p"].tile([128, 4 * 4 * T], dt.float16, tag=f"xt{k}")
        for q in range(4):
            ps = P["pst_p"].tile([128, 4 * T], dt.float16, tag="tp")
            for m in range(4):
                r64 = 4 * (4 * q + m) + k
                nc.tensor.transpose(
                    ps[:, T * m : T * (m + 1)], x_r[:, r64, :], ident[:]
                )
            if k == 0:
                nc.scalar.copy(xt[:, 512 * q : 512 * (q + 1)], ps[:])
            else:
                nc.scalar.mul(xt[:, 512 * q : 512 * (q + 1)], ps[:],
                              float(2.0 ** (-4 * k)))
        xts[k] = xt

    # V-plane trick: plane k=3 is the FULL u16 value (w = v*s, no AND),
    # paired with a3; planes k<3 pair with y_k = a_k - a_3 so the extra
    # terms telescope away exactly.
    for k in range(3):
        y = P["xt_p"].tile([128, 4 * 4 * T], dt.float16, tag=f"y{k}")
        nc.vector.tensor_tensor(out=y[:], in0=xts[k][:], in1=xts[3][:],
                                op=mybir.AluOpType.subtract)
        yts[k] = y

    def xsl(k, i):
        q, m = i // 4, i % 4
        src = xts[3] if k == 3 else yts[k]
        return src[:, 512 * q + T * m : 512 * q + T * (m + 1)]

    # ---- half-group sums of raw x -> R (PE) ----
    psum_x = P["psx_p"].tile([128, 4 * T], dt.float32, tag="psx")
    for k in range(4):
        for q in range(4):
            for m in range(4):
                nc.tensor.matmul(
                    psum_x[:, T * k : T * (k + 1)],
                    amat[:],
                    xts[k][:, 512 * q + T * m : 512 * q + T * (m + 1)],
                    start=(q == 0 and m == 0),
                    stop=(q == 3 and m == 3),
                )

    # ---- main: 8 qw pairs x 4 nibble planes -> one [T, OSH] out block ----
    out_ps = P["pso_p"].tile([T, OSH], dt.float32, tag="out")

    def emit_pair(j):
        if j in qwts:
            qwt = qwts[j]
        else:
            qwt = P["qwt_p"].tile([128, 2 * OSH], dt.uint16, tag="qwt")
            nc.sync.dma_start(qwt[:, 0:OSH], qw_d[256 * j : 256 * j + 128, :])
            nc.sync.dma_start(qwt[:, OSH : 2 * OSH],
                              qw_d[256 * j + 128 : 256 * j + 256, :])
        for k in _KORD:
            if k == 3:
                if j in w3s:
                    w = w3s[j]
                else:
                    w = P["w_p"].tile([128, 2 * OSH], dt.float16, tag="w")
                    nc.vector.tensor_tensor(out=w[:], in0=qwt[:],
                                            in1=s128d[:],
                                            op=mybir.AluOpType.mult)
            else:
                nib = P["nib_p"].tile([128, 2 * OSH], dt.uint16, tag="nib")
                nc.vector.tensor_scalar(
                    out=nib[:], in0=qwt[:], scalar1=15 << (4 * k),
                    scalar2=None, op0=mybir.AluOpType.bitwise_and,
                )
                w = P["w_p"].tile([128, 2 * OSH], dt.float16, tag="w")
                nc.vector.tensor_tensor(out=w[:], in0=nib[:], in1=s128d[:],
                                        op=mybir.AluOpType.mult)
            for h in range(2):
                i = 2 * j + h
                for ns in range(NS):
                    nc.tensor.matmul(
                        out_ps[:, 512 * ns : 512 * (ns + 1)],
                        xsl(k, i),
                        w[:, OSH * h + 512 * ns : OSH * h + 512 * (ns + 1)],
                        start=(j == 0 and k == 3 and h == 0),
                        stop=False,
                    )

    for j in range(5):
        emit_pair(j)

    # ---- R recombination (DVE, emitted mid-stream so rmat is ready well
    # before the correction matmul without blocking early dequant) ----
    t0 = P["sm_p"].tile([128, T], dt.float32, tag="t0")
    t1 = P["sm_p"].tile([128, T], dt.float32, tag="t1")
    t2 = P["sm_p"].tile([128, T], dt.float32, tag="t2")
    t3 = P["sm_p"].tile([128, T], dt.float32, tag="t3")
    rmat = P["sm_p"].tile([128, T], dt.float16, tag="rmat")
    nc.scalar.copy(t0[:], psum_x[:, 0:T])
    nc.vector.scalar_tensor_tensor(
        out=t1[:], in0=psum_x[:, T : 2 * T], scalar=16.0, in1=t0[:],
        op0=mybir.AluOpType.mult, op1=mybir.AluOpType.add,
    )
    nc.vector.scalar_tensor_tensor(
        out=t2[:], in0=psum_x[:, 2 * T : 3 * T], scalar=256.0, in1=t1[:],
        op0=mybir.AluOpType.mult, op1=mybir.AluOpType.add,
    )
    nc.vector.scalar_tensor_tensor(
        out=t3[:], in0=psum_x[:, 3 * T : 4 * T], scalar=4096.0, in1=t2[:],
        op0=mybir.AluOpType.mult, op1=mybir.AluOpType.add,
    )
    # row 127 (zeroed by A) becomes the bias row: rmat = t3 + C
    nc.vector.tensor_tensor(
        out=rmat[:], in0=t3[:], in1=cmat[:], op=mybir.AluOpType.add
    )

    for j in range(5, 8):
        emit_pair(j)

    # zeros + bias correction
    for ns in range(NS):
        nc.tensor.matmul(
            out_ps[:, 512 * ns : 512 * (ns + 1)],
            rmat[:],
            bmat[:, 512 * ns : 512 * (ns + 1)],
            start=False, stop=(ns == NS - 1),
        )
    osb = P["osb_p"].tile([T, OSH], dt.float32, tag="osb")
    nc.scalar.copy(osb[:], out_ps[:])
    nc.scalar.dma_start(out_d[:, :], osb[:])


def _build_nc(n_iters=1, hw_loop=False, unroll=1):
    nc = bass.Bass()
    qw_d = nc.dram_tensor("qw", [2048, OSH], dt.uint16, kind="ExternalInput")
    x_d = nc.dram_tensor("xin", [T, I], dt.float16, kind="ExternalInput")
    # aux cols: 0:OSH s128 | OSH:2*OSH bmat | +128 amat | +128 cmat
    aux_d = nc.dram_tensor("aux", [128, 2 * OSH + 256], dt.float16,
                           kind="ExternalInput")
    out_d = nc.dram_tensor("out", [T, OSH], dt.float32, kind="ExternalOutput")

    with tile.TileContext(nc) as tc:
        with ExitStack() as ctx:
            P = {
                "qw_d": qw_d, "x_d": x_d, "aux_d": aux_d, "out_d": out_d,
                "x_p": ctx.enter_context(tc.tile_pool(name="x", bufs=2)),
                "cst_p": ctx.enter_context(tc.tile_pool(name="cst", bufs=2)),
                "sm_p": ctx.enter_context(tc.tile_pool(name="sm", bufs=2)),
                "xt_p": ctx.enter_context(tc.tile_pool(name="xt", bufs=2)),
                "qwt_p": ctx.enter_context(tc.tile_pool(name="qwt", bufs=3)),
                "nib_p": ctx.enter_context(tc.tile_pool(name="nib", bufs=3)),
                "w_p": ctx.enter_context(tc.tile_pool(name="w", bufs=5)),
                "osb_p": ctx.enter_context(tc.tile_pool(name="osb", bufs=2)),
                "sgl_p": ctx.enter_context(tc.tile_pool(name="sgl", bufs=1)),
                "pst_p": ctx.enter_context(
                    tc.tile_pool(name="pst", bufs=3, space="PSUM")),
                "psx_p": ctx.enter_context(
                    tc.tile_pool(name="psx", bufs=1, space="PSUM")),
                "pso_p": ctx.enter_context(
                    tc.tile_pool(name="pso", bufs=1, space="PSUM")),
            }
            ident = P["sgl_p"].tile([128, 128], dt.float16, tag="ident")
            make_identity(nc, ident[:])
            P["ident"] = ident
            if hw_loop:
                assert n_iters % unroll == 0
                with tc.For_i(0, n_iters // unroll):
                    for _ in range(unroll):
                        _emit_iteration(nc, tc, P)
            else:
                for _ in range(n_iters):
                    _emit_iteration(nc, tc, P)

    _split_excess_waits(nc)
    nc.finalize()
    return nc


_SPLIT_TYPES = {
    "InstTensorScalarPtr", "InstTensorTensor", "InstActivation", "InstMatmult",
    "InstDMACopy", "InstDmaTransposeAnt", "InstMemSet", "InstTensorCopy",
    "InstTensorReduce", "InstDrain", "InstMemset", "InstNoOp",
}

_ENG_MAP = {
    "DVE": "vector", "Activation": "scalar", "PE": "tensor",
    "Pool": "gpsimd", "SP": "sync",
}


def _split_excess_waits(nc):
    """walrus accepts at most one sync-wait per (non-drain) instruction in
    this build; move excess waits onto same-engine ENGINE_NOPs inserted just
    before the instruction."""
    for bb in nc.main_func.blocks:
        insts = list(bb.instructions)
        need = []  # (idx, inst, extra_waits)
        for idx, ins in enumerate(insts):
            if type(ins).__name__ not in _SPLIT_TYPES:
                continue
            si = ins.sync_info
            w = list(si.on_wait) if si else []
            if len(w) > 1:
                need.append((idx, ins, w))
        if not need:
            continue
        created = {}
        for idx, ins, w in need:
            eng = _ENG_MAP.get(ins.engine.name if ins.engine else "", "vector")
            nops = []
            for extra in w[:-1]:
                bi = getattr(nc, eng).nop()
                nop = bi.ins
                nop.sync_info = mybir.SyncInfo(on_wait=[extra], on_update=[])
                nops.append(nop)
            ins.sync_info = mybir.SyncInfo(
                on_wait=[w[-1]], on_update=list(ins.sync_info.on_update))
            created[idx] = nops
        nop_names = {n.name for nops in created.values() for n in nops}
        for bb2 in nc.main_func.blocks:
            cur = [i for i in bb2.instructions if i.name not in nop_names]
            if bb2.name == bb.name:
                out = []
                for idx, ins in enumerate(insts):
                    if idx in created:
                        out.extend(created[idx])
                    out.append(ins)
                bb2.instructions = out
            elif len(cur) != len(list(bb2.instructions)):
                bb2.instructions = cur


def _prep_in_maps(x, qweight, scales, qzeros, bias):
    """Host prep: repack qweight to the permuted-u16 layout and build the
    per-core aux tables.  Cached on input identity."""
    key = (id(x), id(qweight), id(scales), id(qzeros), id(bias))
    cached = _CACHE.get("prep")
    if cached is not None and cached[0] == key:
        return cached[1]

    x = np.asarray(x)
    qweight = np.asarray(qweight)
    scales = np.asarray(scales)
    qzeros = np.asarray(qzeros)
    bias = np.asarray(bias)

    x2 = np.ascontiguousarray(x.reshape(T, I))
    if x2.dtype != np.float16:
        x2 = x2.astype(np.float16)

    # qw tile i, partition p <- u16 row 16p+i of qweight.view(u16).T
    qw16 = qweight.view(np.uint16)              # [O, 2048] (nibble c=4r+k in row o)
    qwp = np.ascontiguousarray(
        qw16.reshape(O, 128, 16).transpose(2, 1, 0)
    ).reshape(2048, O)

    scT = scales.T.astype(np.float16)           # [64, O]
    rep = np.arange(128) // 2
    s128 = np.ascontiguousarray(scT[rep])       # [128, O]

    # zeros: znib[g, o] = nibble g%4 of u16 word g//4 of qzeros row o
    qz16 = qzeros.view(np.uint16)               # [O, 16]
    g = np.arange(NG)
    znib = (qz16.T[g // 4] >> (4 * (g % 4))[:, None]).astype(np.uint16) & 15
    bm_half = -(scT.astype(np.float32) * znib.astype(np.float32))  # [64, O]
    bmat = bm_half[rep].astype(np.float16)
    bmat[127] = bias.astype(np.float16)

    # A matrix: identity, col 126 also sums row 127, col 127 dead
    amat = np.eye(128, dtype=np.float16)
    amat[127, 127] = 0.0
    amat[127, 126] = 1.0
    # C: row 127 ones (bias row of R)
    cmat = np.zeros((128, 128), np.float16)
    cmat[127, :T] = 1.0

    maps = []
    for c in range(N_CORES):
        osl = slice(OSH * c, OSH * (c + 1))
        aux = np.concatenate(
            [s128[:, osl], bmat[:, osl], amat, cmat], axis=1
        ).astype(np.float16)
        maps.append({
            "qw": np.ascontiguousarray(qwp[:, osl]),
            "xin": x2,
            "aux": np.ascontiguousarray(aux),
        })
    _CACHE["prep"] = (key, maps)
    return maps


def _get_nc(n_iters=1, hw_loop=False, unroll=1):
    key = ("nc", n_iters, hw_loop, unroll)
    if key not in _CACHE:
        _CACHE[key] = _build_nc(n_iters, hw_loop, unroll)
    return _CACHE[key]


def _gather(results):
    out = np.concatenate(
        [np.asarray(results[c]["out"]) for c in range(N_CORES)], axis=1
    )
    return np.ascontiguousarray(out.reshape(4, 32, O).astype(np.float32))


def run(inputs, trace=False, trace_cores=None):
    nc = _get_nc(1)
    maps = _prep_in_maps(**inputs)
    res = run_bass_kernel_spmd(nc, maps, list(range(N_CORES)), trace=trace,
                               trace_cores=trace_cores)
    return _gather(res.results), res


def kernel(**inputs) -> np.ndarray:
    out, _ = run(inputs, trace=False)
    return out


K_LO = 8
K_HI = 808


def _build_sharded(nc):
    import jax
    from jax.sharding import Mesh, PartitionSpec
    from jax.experimental.shard_map import shard_map
    from concourse import bass2jax
    import concourse.mybir as mb

    partition_name = nc.partition_id_tensor.name if nc.partition_id_tensor else None
    in_names, out_names, out_avals, zero_outs = [], [], [], []
    for alloc in nc.m.functions[0].allocations:
        if not isinstance(alloc, mb.MemoryLocationSet):
            continue
        name = alloc.memorylocations[0].name
        if alloc.kind == "ExternalInput":
            if name != partition_name:
                in_names.append(name)
        elif alloc.kind == "ExternalOutput":
            out_names.append(name)
            shape = tuple(alloc.tensor_shape)
            dtype = mb.dt.np(alloc.dtype)
            out_avals.append(jax.core.ShapedArray(shape, dtype))
            zero_outs.append(np.zeros(shape, dtype))
    n_params = len(in_names)
    in_names_all = in_names + out_names
    if partition_name is not None:
        in_names_all.append(partition_name)

    def _body(*args):
        operands = list(args)
        if partition_name is not None:
            operands.append(bass2jax.partition_id_tensor())
        outs = bass2jax._bass_exec_p.bind(
            *operands,
            out_avals=tuple(out_avals),
            in_names=tuple(in_names_all),
            out_names=tuple(out_names),
            lowering_input_output_aliases=(),
            sim_require_finite=True,
            sim_require_nnan=True,
            nc=nc,
        )
        return tuple(outs)

    devices = jax.devices()[:N_CORES]
    mesh = Mesh(np.asarray(devices), ("core",))
    n_outs = len(out_names)
    sharded = jax.jit(
        shard_map(
            _body, mesh=mesh,
            in_specs=(PartitionSpec("core"),) * (n_params + n_outs),
            out_specs=(PartitionSpec("core"),) * n_outs,
            check_rep=False,
        ),
        keep_unused=True,
    )
    return sharded, in_names, zero_outs


def bench(inputs, n_rep=8):
    """Time the K_LO- and K_HI-iteration unrolled programs; the slope
    isolates steady-state per-iteration device time from the (large, noisy)
    per-execute axon dispatch constant."""
    import time
    import jax
    from concourse import bass2jax

    bass2jax.install_neuronx_cc_hook()
    maps = _prep_in_maps(**inputs)

    runners = {}
    for k_it in (K_LO, K_HI):
        nc = _get_nc(k_it, hw_loop=True, unroll=8)
        sharded, in_names, zero_outs = _build_sharded(nc)
        concat_in = [
            np.concatenate([np.asarray(maps[c][nm]) for c in range(N_CORES)],
                           axis=0)
            for nm in in_names
        ]
        concat_zeros = [
            np.zeros((N_CORES * z.shape[0], *z.shape[1:]), z.dtype)
            for z in zero_outs
        ]
        args_dev = [jax.device_put(a) for a in concat_in + concat_zeros]
        outs = sharded(*args_dev)
        jax.block_until_ready(outs)
        runners[k_it] = (sharded, args_dev, outs)

    def timed(k_it):
        sharded, args_dev, _ = runners[k_it]
        o = sharded(*args_dev)
        jax.block_until_ready(o)
        t0 = time.time()
        o = sharded(*args_dev)
        jax.block_until_ready(o)
        return time.time() - t0

    lo = [timed(K_LO) for _ in range(n_rep)]
    hi = [timed(K_HI) for _ in range(n_rep)]
    per_iter_ns = (min(hi) - min(lo)) / (K_HI - K_LO) * 1e9

    outs = runners[K_HI][2]
    full = np.asarray(outs[0])          # [N_CORES*T, OSH] concat along axis0
    parts = [full[c * T : (c + 1) * T] for c in range(N_CORES)]
    out = np.concatenate(parts, axis=1).reshape(4, 32, O).astype(np.float32)
    return per_iter_ns, out, (min(lo), min(hi))


# revision 17
# speedup vs baseline: 2.8365x; 1.0060x over previous
"""AWQ 4-bit quantized linear (out = x @ dequant(qweight).T + bias), 8-core
tensor-parallel on TRN2.

Sharding: out_features split 8 ways (O' = 1024 per core); x replicated; each
core computes out[:, c*1024:(c+1)*1024] and the host concatenates.  No
device collectives.

Per-core layout (same trick as the 1-core baseline, O-sliced): qweight u16
rows are permuted so SBUF tile i holds u16-rows {16p + i : p in [0,128)}.
Input column c = 4r + k = 64p + (4i + k); the AWQ group of column c is
g = p//2, independent of (i,k), so one [128, O'] scale tile
s128[p,o] = scales[o, p//2] serves every dequant op, and the x-transposes
are plain stride-64 column slices of x.

Dequant per tile/plane: nib = qw & (15<<4k) (DVE tensor_scalar, 4x mode),
w = nib * s128 (tensor_tensor, 2x mode).  The 2^{4k} nibble-position factor
is compensated on the x side during PSUM->SBUF eviction (ACT).  A subset of
the w-multiplies runs on the GPSIMD (Pool) engine to unload DVE, which is
otherwise the bottleneck engine.

Zeros + bias fold into one extra K=128 matmul: R[p,t] = sum of raw x over
columns [64p, 64p+64) (recovered from the scaled transposes via identity-
matmul accumulation and a 16^k recombination), paired with
bmat[p,o] = -(s*z)[p//2, o]; row 127 of R is forced to 1.0 and bmat[127] =
bias (row 127's half-group sum is folded into row 126 by matrix A).

Benchmarking: the per-execute axon dispatch costs ~1 ms/core and swamps
wall-clock timing of single executions.  bench() therefore builds a second
Bass module with the whole per-iteration body unrolled K_HI times (each
iteration re-DMAs all inputs from DRAM and re-writes the output) and
measures the slope between the K_LO- and K_HI-iteration programs: the
marginal cost of one extra full computation on-device, i.e. the steady-state
HW execution time per iteration.
"""

import numpy as np
from contextlib import ExitStack

import concourse.bass as bass
import concourse.mybir as mybir
import concourse.tile as tile
from concourse.bass_utils import run_bass_kernel_spmd
from concourse.masks import make_identity

dt = mybir.dt

N_CORES = 8
I = 8192                    # in_features
O = 8192                    # out_features
OSH = O // N_CORES          # 1024 out-features per core
T = 128                     # batch*seq = 4*32
NG = 64                     # groups (group_size 128)
NR = 64                     # residue tiles (r64 = 4i + k)
NS = OSH // 512             # 512-wide matmul chunks per out block (2)

_CACHE = {}

_KORD = (3, 0, 1, 2)        # k=3 (v-plane, no AND) first so DVE starts early


def _emit_iteration(nc, tc, P):
    """Emit one full computation: load everything from DRAM, dequantize,
    matmul, correct zeros+bias, store out.

    qw tiles are processed in PAIRS (one [128, 2*OSH] SBUF tile covering
    original tiles 2j and 2j+1) so each DVE dequant op handles 2*OSH
    elements, halving per-instruction overhead.  The scale operand is a
    doubled tile s128d = s128|s128.  DVE emission order is chosen to avoid
    head-of-line blocking on the in-order engine: the k=3 multiplies of the
    first two pairs (which need only qw + scales) run before the y-plane
    subtracts (which need the x transposes)."""
    qw_d, x_d, aux_d, out_d = P["qw_d"], P["x_d"], P["aux_d"], P["out_d"]
    ident = P["ident"]

    x_sb = P["x_p"].tile([T, I], dt.float16, tag="x")
    nc.scalar.dma_start(x_sb[:], x_d[:, :])
    s128d = P["cst_p"].tile([128, 2 * OSH], dt.float16, tag="s128d")
    nc.scalar.dma_start(s128d[:, 0:OSH], aux_d[:, 0:OSH])
    nc.scalar.dma_start(s128d[:, OSH : 2 * OSH], aux_d[:, 0:OSH])
    bmat = P["cst_p"].tile([128, OSH], dt.float16, tag="bmat")
    nc.scalar.dma_start(bmat[:], aux_d[:, OSH : 2 * OSH])
    amat = P["sm_p"].tile([128, 128], dt.float16, tag="amat")
    nc.scalar.dma_start(amat[:], aux_d[:, 2 * OSH : 2 * OSH + 128])
    cmat = P["sm_p"].tile([128, T], dt.float16, tag="cmat")
    nc.scalar.dma_start(cmat[:], aux_d[:, 2 * OSH + 128 : 2 * OSH + 256])

    # ---- early DVE work: k=3 dequant of ALL pairs needs only qw+scales,
    # so DVE is busy while the x transposes are still in flight ----
    qwts, w3s = {}, {}
    for j in range(8):
        qwt = P["qwt_p"].tile([128, 2 * OSH], dt.uint16, tag="qwt")
        nc.sync.dma_start(qwt[:, 0:OSH], qw_d[256 * j : 256 * j + 128, :])
        nc.sync.dma_start(qwt[:, OSH : 2 * OSH],
                          qw_d[256 * j + 128 : 256 * j + 256, :])
        qwts[j] = qwt
        w3 = P["w3_p"].tile([128, 2 * OSH], dt.float16, tag="w3")
        nc.vector.tensor_tensor(out=w3[:], in0=qwt[:], in1=s128d[:],
                                op=mybir.AluOpType.mult)
        w3s[j] = w3

    # ---- preamble: 64 transposes, 4-per-PSUM-tile, evicted into one
    # [128, 2048] tile per nibble plane k (quarter q at 512*q).
    # k=3 first: its transposes feed both the y-subtracts and the early
    # k=3 matmuls. ----
    x_r = x_sb.rearrange("t (p r) -> t r p", r=NR)
    xts, yts = {}, {}
    for k in _KORD:
        xt = P["xt_p"].tile([128, 4 * 4 * T], dt.float16, tag=f"xt{k}")
        for q in range(4):
            ps = P["pst_p"].tile([128, 4 * T], dt.float16, tag="tp")
            for m in range(4):
                r64 = 4 * (4 * q + m) + k
                nc.tensor.transpose(
                    ps[:, T * m : T * (m + 1)], x_r[:, r64, :], ident[:]
                )
            if k == 0:
                nc.scalar.copy(xt[:, 512 * q : 512 * (q + 1)], ps[:])
            else:
                nc.scalar.mul(xt[:, 512 * q : 512 * (q + 1)], ps[:],
                              float(2.0 ** (-4 * k)))
        xts[k] = xt

    # V-plane trick: plane k=3 is the FULL u16 value (w = v*s, no AND),
    # paired with a3; planes k<3 pair with y_k = a_k - a_3 so the extra
    # terms telescope away exactly.
    for k in range(3):
        y = P["xt_p"].tile([128, 4 * 4 * T], dt.float16, tag=f"y{k}")
        nc.vector.tensor_tensor(out=y[:], in0=xts[k][:], in1=xts[3][:],
                                op=mybir.AluOpType.subtract)
        yts[k] = y

    def xsl(k, i):
        q, m = i // 4, i % 4
        src = xts[3] if k == 3 else yts[k]
        return src[:, 512 * q + T * m : 512 * q + T * (m + 1)]

    # ---- all k=3 matmuls up front (w3 + xts[3] are ready first) ----
    out_ps = P["pso_p"].tile([T, OSH], dt.float32, tag="out")
    for j in range(8):
        for h in range(2):
            i = 2 * j + h
            for ns in range(NS):
                nc.tensor.matmul(
                    out_ps[:, 512 * ns : 512 * (ns + 1)],
                    xsl(3, i),
                    w3s[j][:, OSH * h + 512 * ns : OSH * h + 512 * (ns + 1)],
                    start=(j == 0 and h == 0),
                    stop=False,
                )

    # ---- half-group sums of raw x -> R (PE) ----
    psum_x = P["psx_p"].tile([128, 4 * T], dt.float32, tag="psx")
    for k in range(4):
        for q in range(4):
            for m in range(4):
                nc.tensor.matmul(
                    psum_x[:, T * k : T * (k + 1)],
                    amat[:],
                    xts[k][:, 512 * q + T * m : 512 * q + T * (m + 1)],
                    start=(q == 0 and m == 0),
                    stop=(q == 3 and m == 3),
                )

    def emit_pair(j):
        qwt = qwts[j]
        for k in (0, 1, 2):
            nib = P["nib_p"].tile([128, 2 * OSH], dt.uint16, tag="nib")
            nc.vector.tensor_scalar(
                out=nib[:], in0=qwt[:], scalar1=15 << (4 * k),
                scalar2=None, op0=mybir.AluOpType.bitwise_and,
            )
            w = P["w_p"].tile([128, 2 * OSH], dt.float16, tag="w")
            nc.vector.tensor_tensor(out=w[:], in0=nib[:], in1=s128d[:],
                                    op=mybir.AluOpType.mult)
            for h in range(2):
                i = 2 * j + h
                for ns in range(NS):
                    nc.tensor.matmul(
                        out_ps[:, 512 * ns : 512 * (ns + 1)],
                        xsl(k, i),
                        w[:, OSH * h + 512 * ns : OSH * h + 512 * (ns + 1)],
                        start=False,
                        stop=False,
                    )

    for j in range(5):
        emit_pair(j)

    # ---- R recombination (DVE, emitted mid-stream so rmat is ready well
    # before the correction matmul without blocking early dequant) ----
    t0 = P["sm_p"].tile([128, T], dt.float32, tag="t0")
    t1 = P["sm_p"].tile([128, T], dt.float32, tag="t1")
    t2 = P["sm_p"].tile([128, T], dt.float32, tag="t2")
    t3 = P["sm_p"].tile([128, T], dt.float32, tag="t3")
    rmat = P["sm_p"].tile([128, T], dt.float16, tag="rmat")
    nc.scalar.copy(t0[:], psum_x[:, 0:T])
    nc.vector.scalar_tensor_tensor(
        out=t1[:], in0=psum_x[:, T : 2 * T], scalar=16.0, in1=t0[:],
        op0=mybir.AluOpType.mult, op1=mybir.AluOpType.add,
    )
    nc.vector.scalar_tensor_tensor(
        out=t2[:], in0=psum_x[:, 2 * T : 3 * T], scalar=256.0, in1=t1[:],
        op0=mybir.AluOpType.mult, op1=mybir.AluOpType.add,
    )
    nc.vector.scalar_tensor_tensor(
        out=t3[:], in0=psum_x[:, 3 * T : 4 * T], scalar=4096.0, in1=t2[:],
        op0=mybir.AluOpType.mult, op1=mybir.AluOpType.add,
    )
    # row 127 (zeroed by A) becomes the bias row: rmat = t3 + C
    nc.vector.tensor_tensor(
        out=rmat[:], in0=t3[:], in1=cmat[:], op=mybir.AluOpType.add
    )

    for j in range(5, 8):
        emit_pair(j)

    # zeros + bias correction
    for ns in range(NS):
        nc.tensor.matmul(
            out_ps[:, 512 * ns : 512 * (ns + 1)],
            rmat[:],
            bmat[:, 512 * ns : 512 * (ns + 1)],
            start=False, stop=(ns == NS - 1),
        )
    osb = P["osb_p"].tile([T, OSH], dt.float32, tag="osb")
    nc.scalar.copy(osb[:], out_ps[:])
    nc.scalar.dma_start(out_d[:, :], osb[:])


def _build_nc(n_iters=1, hw_loop=False, unroll=1):
    nc = bass.Bass()
    qw_d = nc.dram_tensor("qw", [2048, OSH], dt.uint16, kind="ExternalInput")
    x_d = nc.dram_tensor("xin", [T, I], dt.float16, kind="ExternalInput")
    # aux cols: 0:OSH s128 | OSH:2*OSH bmat | +128 amat | +128 cmat
    aux_d = nc.dram_tensor("aux", [128, 2 * OSH + 256], dt.float16,
                           kind="ExternalInput")
    out_d = nc.dram_tensor("out", [T, OSH], dt.float32, kind="ExternalOutput")

    with tile.TileContext(nc) as tc:
        with ExitStack() as ctx:
            P = {
                "qw_d": qw_d, "x_d": x_d, "aux_d": aux_d, "out_d": out_d,
                "x_p": ctx.enter_context(tc.tile_pool(name="x", bufs=2)),
                "cst_p": ctx.enter_context(tc.tile_pool(name="cst", bufs=2)),
                "sm_p": ctx.enter_context(tc.tile_pool(name="sm", bufs=2)),
                "xt_p": ctx.enter_context(tc.tile_pool(name="xt", bufs=2)),
                "qwt_p": ctx.enter_context(tc.tile_pool(name="qwt", bufs=9)),
                "nib_p": ctx.enter_context(tc.tile_pool(name="nib", bufs=2)),
                "w_p": ctx.enter_context(tc.tile_pool(name="w", bufs=3)),
                "w3_p": ctx.enter_context(tc.tile_pool(name="w3", bufs=8)),
                "osb_p": ctx.enter_context(tc.tile_pool(name="osb", bufs=2)),
                "sgl_p": ctx.enter_context(tc.tile_pool(name="sgl", bufs=1)),
                "pst_p": ctx.enter_context(
                    tc.tile_pool(name="pst", bufs=3, space="PSUM")),
                "psx_p": ctx.enter_context(
                    tc.tile_pool(name="psx", bufs=1, space="PSUM")),
                "pso_p": ctx.enter_context(
                    tc.tile_pool(name="pso", bufs=1, space="PSUM")),
            }
            ident = P["sgl_p"].tile([128, 128], dt.float16, tag="ident")
            make_identity(nc, ident[:])
            P["ident"] = ident
            if hw_loop:
                assert n_iters % unroll == 0
                with tc.For_i(0, n_iters // unroll):
                    for _ in range(unroll):
                        _emit_iteration(nc, tc, P)
            else:
                for _ in range(n_iters):
                    _emit_iteration(nc, tc, P)

    _split_excess_waits(nc)
    nc.finalize()
    return nc


_SPLIT_TYPES = {
    "InstTensorScalarPtr", "InstTensorTensor", "InstActivation", "InstMatmult",
    "InstDMACopy", "InstDmaTransposeAnt", "InstMemSet", "InstTensorCopy",
    "InstTensorReduce", "InstDrain", "InstMemset", "InstNoOp",
}

_ENG_MAP = {
    "DVE": "vector", "Activation": "scalar", "PE": "tensor",
    "Pool": "gpsimd", "SP": "sync",
}


def _split_excess_waits(nc):
    """walrus accepts at most one sync-wait per (non-drain) instruction in
    this build; move excess waits onto same-engine ENGINE_NOPs inserted just
    before the instruction."""
    for bb in nc.main_func.blocks:
        insts = list(bb.instructions)
        need = []  # (idx, inst, extra_waits)
        for idx, ins in enumerate(insts):
            if type(ins).__name__ not in _SPLIT_TYPES:
                continue
            si = ins.sync_info
            w = list(si.on_wait) if si else []
            if len(w) > 1:
                need.append((idx, ins, w))
        if not need:
            continue
        created = {}
        for idx, ins, w in need:
            eng = _ENG_MAP.get(ins.engine.name if ins.engine else "", "vector")
            nops = []
            for extra in w[:-1]:
                bi = getattr(nc, eng).nop()
                nop = bi.ins
                nop.sync_info = mybir.SyncInfo(on_wait=[extra], on_update=[])
                nops.append(nop)
            ins.sync_info = mybir.SyncInfo(
                on_wait=[w[-1]], on_update=list(ins.sync_info.on_update))
            created[idx] = nops
        nop_names = {n.name for nops in created.values() for n in nops}
        for bb2 in nc.main_func.blocks:
            cur = [i for i in bb2.instructions if i.name not in nop_names]
            if bb2.name == bb.name:
                out = []
                for idx, ins in enumerate(insts):
                    if idx in created:
                        out.extend(created[idx])
                    out.append(ins)
                bb2.instructions = out
            elif len(cur) != len(list(bb2.instructions)):
                bb2.instructions = cur


def _prep_in_maps(x, qweight, scales, qzeros, bias):
    """Host prep: repack qweight to the permuted-u16 layout and build the
    per-core aux tables.  Cached on input identity."""
    key = (id(x), id(qweight), id(scales), id(qzeros), id(bias))
    cached = _CACHE.get("prep")
    if cached is not None and cached[0] == key:
        return cached[1]

    x = np.asarray(x)
    qweight = np.asarray(qweight)
    scales = np.asarray(scales)
    qzeros = np.asarray(qzeros)
    bias = np.asarray(bias)

    x2 = np.ascontiguousarray(x.reshape(T, I))
    if x2.dtype != np.float16:
        x2 = x2.astype(np.float16)

    # qw tile i, partition p <- u16 row 16p+i of qweight.view(u16).T
    qw16 = qweight.view(np.uint16)              # [O, 2048] (nibble c=4r+k in row o)
    qwp = np.ascontiguousarray(
        qw16.reshape(O, 128, 16).transpose(2, 1, 0)
    ).reshape(2048, O)

    scT = scales.T.astype(np.float16)           # [64, O]
    rep = np.arange(128) // 2
    s128 = np.ascontiguousarray(scT[rep])       # [128, O]

    # zeros: znib[g, o] = nibble g%4 of u16 word g//4 of qzeros row o
    qz16 = qzeros.view(np.uint16)               # [O, 16]
    g = np.arange(NG)
    znib = (qz16.T[g // 4] >> (4 * (g % 4))[:, None]).astype(np.uint16) & 15
    bm_half = -(scT.astype(np.float32) * znib.astype(np.float32))  # [64, O]
    bmat = bm_half[rep].astype(np.float16)
    bmat[127] = bias.astype(np.float16)

    # A matrix: identity, col 126 also sums row 127, col 127 dead
    amat = np.eye(128, dtype=np.float16)
    amat[127, 127] = 0.0
    amat[127, 126] = 1.0
    # C: row 127 ones (bias row of R)
    cmat = np.zeros((128, 128), np.float16)
    cmat[127, :T] = 1.0

    maps = []
    for c in range(N_CORES):
        osl = slice(OSH * c, OSH * (c + 1))
        aux = np.concatenate(
            [s128[:, osl], bmat[:, osl], amat, cmat], axis=1
        ).astype(np.float16)
        maps.append({
            "qw": np.ascontiguousarray(qwp[:, osl]),
            "xin": x2,
            "aux": np.ascontiguousarray(aux),
        })
    _CACHE["prep"] = (key, maps)
    return maps


def _get_nc(n_iters=1, hw_loop=False, unroll=1):
    key = ("nc", n_iters, hw_loop, unroll)
    if key not in _CACHE:
        _CACHE[key] = _build_nc(n_iters, hw_loop, unroll)
    return _CACHE[key]


def _gather(results):
    out = np.concatenate(
        [np.asarray(results[c]["out"]) for c in range(N_CORES)], axis=1
    )
    return np.ascontiguousarray(out.reshape(4, 32, O).astype(np.float32))


def run(inputs, trace=False, trace_cores=None):
    nc = _get_nc(1)
    maps = _prep_in_maps(**inputs)
    res = run_bass_kernel_spmd(nc, maps, list(range(N_CORES)), trace=trace,
                               trace_cores=trace_cores)
    return _gather(res.results), res


def kernel(**inputs) -> np.ndarray:
    out, _ = run(inputs, trace=False)
    return out


K_LO = 8
K_HI = 808


def _build_sharded(nc):
    import jax
    from jax.sharding import Mesh, PartitionSpec
    from jax.experimental.shard_map import shard_map
    from concourse import bass2jax
    import concourse.mybir as mb

    partition_name = nc.partition_id_tensor.name if nc.partition_id_tensor else None
    in_names, out_names, out_avals, zero_outs = [], [], [], []
    for alloc in nc.m.functions[0].allocations:
        if not isinstance(alloc, mb.MemoryLocationSet):
            continue
        name = alloc.memorylocations[0].name
        if alloc.kind == "ExternalInput":
            if name != partition_name:
                in_names.append(name)
        elif alloc.kind == "ExternalOutput":
            out_names.append(name)
            shape = tuple(alloc.tensor_shape)
            dtype = mb.dt.np(alloc.dtype)
            out_avals.append(jax.core.ShapedArray(shape, dtype))
            zero_outs.append(np.zeros(shape, dtype))
    n_params = len(in_names)
    in_names_all = in_names + out_names
    if partition_name is not None:
        in_names_all.append(partition_name)

    def _body(*args):
        operands = list(args)
        if partition_name is not None:
            operands.append(bass2jax.partition_id_tensor())
        outs = bass2jax._bass_exec_p.bind(
            *operands,
            out_avals=tuple(out_avals),
            in_names=tuple(in_names_all),
            out_names=tuple(out_names),
            lowering_input_output_aliases=(),
            sim_require_finite=True,
            sim_require_nnan=True,
            nc=nc,
        )
        return tuple(outs)

    devices = jax.devices()[:N_CORES]
    mesh = Mesh(np.asarray(devices), ("core",))
    n_outs = len(out_names)
    sharded = jax.jit(
        shard_map(
            _body, mesh=mesh,
            in_specs=(PartitionSpec("core"),) * (n_params + n_outs),
            out_specs=(PartitionSpec("core"),) * n_outs,
            check_rep=False,
        ),
        keep_unused=True,
    )
    return sharded, in_names, zero_outs


def bench(inputs, n_rep=8):
    """Time the K_LO- and K_HI-iteration unrolled programs; the slope
    isolates steady-state per-iteration device time from the (large, noisy)
    per-execute axon dispatch constant."""
    import time
    import jax
    from concourse import bass2jax

    bass2jax.install_neuronx_cc_hook()
    maps = _prep_in_maps(**inputs)

    runners = {}
    for k_it in (K_LO, K_HI):
        nc = _get_nc(k_it, hw_loop=True, unroll=8)
        sharded, in_names, zero_outs = _build_sharded(nc)
        concat_in = [
            np.concatenate([np.asarray(maps[c][nm]) for c in range(N_CORES)],
                           axis=0)
            for nm in in_names
        ]
        concat_zeros = [
            np.zeros((N_CORES * z.shape[0], *z.shape[1:]), z.dtype)
            for z in zero_outs
        ]
        args_dev = [jax.device_put(a) for a in concat_in + concat_zeros]
        outs = sharded(*args_dev)
        jax.block_until_ready(outs)
        runners[k_it] = (sharded, args_dev, outs)

    def timed(k_it):
        sharded, args_dev, _ = runners[k_it]
        o = sharded(*args_dev)
        jax.block_until_ready(o)
        t0 = time.time()
        o = sharded(*args_dev)
        jax.block_until_ready(o)
        return time.time() - t0

    lo = [timed(K_LO) for _ in range(n_rep)]
    hi = [timed(K_HI) for _ in range(n_rep)]
    per_iter_ns = (min(hi) - min(lo)) / (K_HI - K_LO) * 1e9

    outs = runners[K_HI][2]
    full = np.asarray(outs[0])          # [N_CORES*T, OSH] concat along axis0
    parts = [full[c * T : (c + 1) * T] for c in range(N_CORES)]
    out = np.concatenate(parts, axis=1).reshape(4, 32, O).astype(np.float32)
    return per_iter_ns, out, (min(lo), min(hi))


# revision 19
# speedup vs baseline: 2.8614x; 1.0088x over previous
"""AWQ 4-bit quantized linear (out = x @ dequant(qweight).T + bias), 8-core
tensor-parallel on TRN2.

Sharding: out_features split 8 ways (O' = 1024 per core); x replicated; each
core computes out[:, c*1024:(c+1)*1024] and the host concatenates.  No
device collectives.

Per-core layout (same trick as the 1-core baseline, O-sliced): qweight u16
rows are permuted so SBUF tile i holds u16-rows {16p + i : p in [0,128)}.
Input column c = 4r + k = 64p + (4i + k); the AWQ group of column c is
g = p//2, independent of (i,k), so one [128, O'] scale tile
s128[p,o] = scales[o, p//2] serves every dequant op, and the x-transposes
are plain stride-64 column slices of x.

Dequant per tile/plane: nib = qw & (15<<4k) (DVE tensor_scalar, 4x mode),
w = nib * s128 (tensor_tensor, 2x mode).  The 2^{4k} nibble-position factor
is compensated on the x side during PSUM->SBUF eviction (ACT).  A subset of
the w-multiplies runs on the GPSIMD (Pool) engine to unload DVE, which is
otherwise the bottleneck engine.

Zeros + bias fold into one extra K=128 matmul: R[p,t] = sum of raw x over
columns [64p, 64p+64) (recovered from the scaled transposes via identity-
matmul accumulation and a 16^k recombination), paired with
bmat[p,o] = -(s*z)[p//2, o]; row 127 of R is forced to 1.0 and bmat[127] =
bias (row 127's half-group sum is folded into row 126 by matrix A).

Benchmarking: the per-execute axon dispatch costs ~1 ms/core and swamps
wall-clock timing of single executions.  bench() therefore builds a second
Bass module with the whole per-iteration body unrolled K_HI times (each
iteration re-DMAs all inputs from DRAM and re-writes the output) and
measures the slope between the K_LO- and K_HI-iteration programs: the
marginal cost of one extra full computation on-device, i.e. the steady-state
HW execution time per iteration.
"""

import numpy as np
from contextlib import ExitStack

import concourse.bass as bass
import concourse.mybir as mybir
import concourse.tile as tile
from concourse.bass_utils import run_bass_kernel_spmd
from concourse.masks import make_identity

dt = mybir.dt

N_CORES = 8
I = 8192                    # in_features
O = 8192                    # out_features
OSH = O // N_CORES          # 1024 out-features per core
T = 128                     # batch*seq = 4*32
NG = 64                     # groups (group_size 128)
NR = 64                     # residue tiles (r64 = 4i + k)
NS = OSH // 512             # 512-wide matmul chunks per out block (2)

_CACHE = {}

_KORD = (3, 0, 1, 2)        # k=3 (v-plane, no AND) first so DVE starts early


def _emit_iteration(nc, tc, P):
    """Emit one full computation: load everything from DRAM, dequantize,
    matmul, correct zeros+bias, store out.

    qw tiles are processed in PAIRS (one [128, 2*OSH] SBUF tile covering
    original tiles 2j and 2j+1) so each DVE dequant op handles 2*OSH
    elements, halving per-instruction overhead.  The scale operand is a
    doubled tile s128d = s128|s128.  DVE emission order is chosen to avoid
    head-of-line blocking on the in-order engine: the k=3 multiplies of the
    first two pairs (which need only qw + scales) run before the y-plane
    subtracts (which need the x transposes)."""
    qw_d, x_d, aux_d, out_d = P["qw_d"], P["x_d"], P["aux_d"], P["out_d"]
    ident = P["ident"]

    # s128d first on the scalar queue: it unblocks the early k=3 DVE
    # multiplies; x follows (feeds the PE transposes); the bias/correction
    # tables are only needed late.
    s128d = P["cst_p"].tile([128, 2 * OSH], dt.float16, tag="s128d")
    nc.scalar.dma_start(s128d[:, 0:OSH], aux_d[:, 0:OSH])
    nc.scalar.dma_start(s128d[:, OSH : 2 * OSH], aux_d[:, 0:OSH])
    x_sb = P["x_p"].tile([T, I], dt.float16, tag="x")
    nc.scalar.dma_start(x_sb[:], x_d[:, :])
    amat = P["sm_p"].tile([128, 128], dt.float16, tag="amat")
    nc.scalar.dma_start(amat[:], aux_d[:, 2 * OSH : 2 * OSH + 128])
    cmat = P["sm_p"].tile([128, T], dt.float16, tag="cmat")
    nc.scalar.dma_start(cmat[:], aux_d[:, 2 * OSH + 128 : 2 * OSH + 256])
    bmat = P["cst_p"].tile([128, OSH], dt.float16, tag="bmat")
    nc.scalar.dma_start(bmat[:], aux_d[:, OSH : 2 * OSH])

    # ---- early DVE work: k=3 dequant of ALL pairs needs only qw+scales,
    # so DVE is busy while the x transposes are still in flight ----
    qwts, w3s = {}, {}
    for j in range(8):
        qwt = P["qwt_p"].tile([128, 2 * OSH], dt.uint16, tag="qwt")
        nc.sync.dma_start(qwt[:, 0:OSH], qw_d[256 * j : 256 * j + 128, :])
        nc.sync.dma_start(qwt[:, OSH : 2 * OSH],
                          qw_d[256 * j + 128 : 256 * j + 256, :])
        qwts[j] = qwt
        w3 = P["w3_p"].tile([128, 2 * OSH], dt.float16, tag="w3")
        nc.vector.tensor_tensor(out=w3[:], in0=qwt[:], in1=s128d[:],
                                op=mybir.AluOpType.mult)
        w3s[j] = w3

    # ---- preamble: 64 transposes, 4-per-PSUM-tile, evicted into one
    # [128, 2048] tile per nibble plane k (quarter q at 512*q).
    # k=3 first: its transposes feed both the y-subtracts and the early
    # k=3 matmuls. ----
    x_r = x_sb.rearrange("t (p r) -> t r p", r=NR)
    xts, yts = {}, {}
    for k in _KORD:
        xt = P["xt_p"].tile([128, 4 * 4 * T], dt.float16, tag=f"xt{k}")
        for q in range(4):
            ps = P["pst_p"].tile([128, 4 * T], dt.float16, tag="tp")
            for m in range(4):
                r64 = 4 * (4 * q + m) + k
                nc.tensor.transpose(
                    ps[:, T * m : T * (m + 1)], x_r[:, r64, :], ident[:]
                )
            if k == 0:
                nc.scalar.copy(xt[:, 512 * q : 512 * (q + 1)], ps[:])
            else:
                nc.scalar.mul(xt[:, 512 * q : 512 * (q + 1)], ps[:],
                              float(2.0 ** (-4 * k)))
        xts[k] = xt

    # V-plane trick: plane k=3 is the FULL u16 value (w = v*s, no AND),
    # paired with a3; planes k<3 pair with y_k = a_k - a_3 so the extra
    # terms telescope away exactly.
    for k in range(3):
        y = P["xt_p"].tile([128, 4 * 4 * T], dt.float16, tag=f"y{k}")
        nc.vector.tensor_tensor(out=y[:], in0=xts[k][:], in1=xts[3][:],
                                op=mybir.AluOpType.subtract)
        yts[k] = y

    def xsl(k, i):
        q, m = i // 4, i % 4
        src = xts[3] if k == 3 else yts[k]
        return src[:, 512 * q + T * m : 512 * q + T * (m + 1)]

    # ---- all k=3 matmuls up front (w3 + xts[3] are ready first) ----
    out_ps = P["pso_p"].tile([T, OSH], dt.float32, tag="out")
    for j in range(8):
        for h in range(2):
            i = 2 * j + h
            for ns in range(NS):
                nc.tensor.matmul(
                    out_ps[:, 512 * ns : 512 * (ns + 1)],
                    xsl(3, i),
                    w3s[j][:, OSH * h + 512 * ns : OSH * h + 512 * (ns + 1)],
                    start=(j == 0 and h == 0),
                    stop=False,
                )

    # ---- half-group sums of raw x -> R (PE) ----
    psum_x = P["psx_p"].tile([128, 4 * T], dt.float32, tag="psx")
    for k in range(4):
        for q in range(4):
            for m in range(4):
                nc.tensor.matmul(
                    psum_x[:, T * k : T * (k + 1)],
                    amat[:],
                    xts[k][:, 512 * q + T * m : 512 * q + T * (m + 1)],
                    start=(q == 0 and m == 0),
                    stop=(q == 3 and m == 3),
                )

    def emit_pair(j):
        qwt = qwts[j]
        for k in (0, 1, 2):
            nib = P["nib_p"].tile([128, 2 * OSH], dt.uint16, tag="nib")
            nc.vector.tensor_scalar(
                out=nib[:], in0=qwt[:], scalar1=15 << (4 * k),
                scalar2=None, op0=mybir.AluOpType.bitwise_and,
            )
            w = P["w_p"].tile([128, 2 * OSH], dt.float16, tag="w")
            nc.vector.tensor_tensor(out=w[:], in0=nib[:], in1=s128d[:],
                                    op=mybir.AluOpType.mult)
            for h in range(2):
                i = 2 * j + h
                for ns in range(NS):
                    nc.tensor.matmul(
                        out_ps[:, 512 * ns : 512 * (ns + 1)],
                        xsl(k, i),
                        w[:, OSH * h + 512 * ns : OSH * h + 512 * (ns + 1)],
                        start=False,
                        stop=False,
                    )

    for j in range(5):
        emit_pair(j)

    # ---- R recombination (DVE, emitted mid-stream so rmat is ready well
    # before the correction matmul without blocking early dequant) ----
    t0 = P["sm_p"].tile([128, T], dt.float32, tag="t0")
    t1 = P["sm_p"].tile([128, T], dt.float32, tag="t1")
    t2 = P["sm_p"].tile([128, T], dt.float32, tag="t2")
    t3 = P["sm_p"].tile([128, T], dt.float32, tag="t3")
    rmat = P["sm_p"].tile([128, T], dt.float16, tag="rmat")
    nc.scalar.copy(t0[:], psum_x[:, 0:T])
    nc.vector.scalar_tensor_tensor(
        out=t1[:], in0=psum_x[:, T : 2 * T], scalar=16.0, in1=t0[:],
        op0=mybir.AluOpType.mult, op1=mybir.AluOpType.add,
    )
    nc.vector.scalar_tensor_tensor(
        out=t2[:], in0=psum_x[:, 2 * T : 3 * T], scalar=256.0, in1=t1[:],
        op0=mybir.AluOpType.mult, op1=mybir.AluOpType.add,
    )
    nc.vector.scalar_tensor_tensor(
        out=t3[:], in0=psum_x[:, 3 * T : 4 * T], scalar=4096.0, in1=t2[:],
        op0=mybir.AluOpType.mult, op1=mybir.AluOpType.add,
    )
    # row 127 (zeroed by A) becomes the bias row: rmat = t3 + C
    nc.vector.tensor_tensor(
        out=rmat[:], in0=t3[:], in1=cmat[:], op=mybir.AluOpType.add
    )

    for j in range(5, 8):
        emit_pair(j)

    # zeros + bias correction
    for ns in range(NS):
        nc.tensor.matmul(
            out_ps[:, 512 * ns : 512 * (ns + 1)],
            rmat[:],
            bmat[:, 512 * ns : 512 * (ns + 1)],
            start=False, stop=(ns == NS - 1),
        )
    osb = P["osb_p"].tile([T, OSH], dt.float32, tag="osb")
    nc.scalar.copy(osb[:], out_ps[:])
    nc.scalar.dma_start(out_d[:, :], osb[:])


def _build_nc(n_iters=1, hw_loop=False, unroll=1):
    nc = bass.Bass()
    qw_d = nc.dram_tensor("qw", [2048, OSH], dt.uint16, kind="ExternalInput")
    x_d = nc.dram_tensor("xin", [T, I], dt.float16, kind="ExternalInput")
    # aux cols: 0:OSH s128 | OSH:2*OSH bmat | +128 amat | +128 cmat
    aux_d = nc.dram_tensor("aux", [128, 2 * OSH + 256], dt.float16,
                           kind="ExternalInput")
    out_d = nc.dram_tensor("out", [T, OSH], dt.float32, kind="ExternalOutput")

    with tile.TileContext(nc) as tc:
        with ExitStack() as ctx:
            P = {
                "qw_d": qw_d, "x_d": x_d, "aux_d": aux_d, "out_d": out_d,
                "x_p": ctx.enter_context(tc.tile_pool(name="x", bufs=2)),
                "cst_p": ctx.enter_context(tc.tile_pool(name="cst", bufs=2)),
                "sm_p": ctx.enter_context(tc.tile_pool(name="sm", bufs=2)),
                "xt_p": ctx.enter_context(tc.tile_pool(name="xt", bufs=2)),
                "qwt_p": ctx.enter_context(tc.tile_pool(name="qwt", bufs=9)),
                "nib_p": ctx.enter_context(tc.tile_pool(name="nib", bufs=2)),
                "w_p": ctx.enter_context(tc.tile_pool(name="w", bufs=3)),
                "w3_p": ctx.enter_context(tc.tile_pool(name="w3", bufs=8)),
                "osb_p": ctx.enter_context(tc.tile_pool(name="osb", bufs=2)),
                "sgl_p": ctx.enter_context(tc.tile_pool(name="sgl", bufs=1)),
                "pst_p": ctx.enter_context(
                    tc.tile_pool(name="pst", bufs=3, space="PSUM")),
                "psx_p": ctx.enter_context(
                    tc.tile_pool(name="psx", bufs=1, space="PSUM")),
                "pso_p": ctx.enter_context(
                    tc.tile_pool(name="pso", bufs=1, space="PSUM")),
            }
            ident = P["sgl_p"].tile([128, 128], dt.float16, tag="ident")
            make_identity(nc, ident[:])
            P["ident"] = ident
            if hw_loop:
                assert n_iters % unroll == 0
                with tc.For_i(0, n_iters // unroll):
                    for _ in range(unroll):
                        _emit_iteration(nc, tc, P)
            else:
                for _ in range(n_iters):
                    _emit_iteration(nc, tc, P)

    _split_excess_waits(nc)
    nc.finalize()
    return nc


_SPLIT_TYPES = {
    "InstTensorScalarPtr", "InstTensorTensor", "InstActivation", "InstMatmult",
    "InstDMACopy", "InstDmaTransposeAnt", "InstMemSet", "InstTensorCopy",
    "InstTensorReduce", "InstDrain", "InstMemset", "InstNoOp",
}

_ENG_MAP = {
    "DVE": "vector", "Activation": "scalar", "PE": "tensor",
    "Pool": "gpsimd", "SP": "sync",
}


def _split_excess_waits(nc):
    """walrus accepts at most one sync-wait per (non-drain) instruction in
    this build; move excess waits onto same-engine ENGINE_NOPs inserted just
    before the instruction."""
    for bb in nc.main_func.blocks:
        insts = list(bb.instructions)
        need = []  # (idx, inst, extra_waits)
        for idx, ins in enumerate(insts):
            if type(ins).__name__ not in _SPLIT_TYPES:
                continue
            si = ins.sync_info
            w = list(si.on_wait) if si else []
            if len(w) > 1:
                need.append((idx, ins, w))
        if not need:
            continue
        created = {}
        for idx, ins, w in need:
            eng = _ENG_MAP.get(ins.engine.name if ins.engine else "", "vector")
            nops = []
            for extra in w[:-1]:
                bi = getattr(nc, eng).nop()
                nop = bi.ins
                nop.sync_info = mybir.SyncInfo(on_wait=[extra], on_update=[])
                nops.append(nop)
            ins.sync_info = mybir.SyncInfo(
                on_wait=[w[-1]], on_update=list(ins.sync_info.on_update))
            created[idx] = nops
        nop_names = {n.name for nops in created.values() for n in nops}
        for bb2 in nc.main_func.blocks:
            cur = [i for i in bb2.instructions if i.name not in nop_names]
            if bb2.name == bb.name:
                out = []
                for idx, ins in enumerate(insts):
                    if idx in created:
                        out.extend(created[idx])
                    out.append(ins)
                bb2.instructions = out
            elif len(cur) != len(list(bb2.instructions)):
                bb2.instructions = cur


def _prep_in_maps(x, qweight, scales, qzeros, bias):
    """Host prep: repack qweight to the permuted-u16 layout and build the
    per-core aux tables.  Cached on input identity."""
    key = (id(x), id(qweight), id(scales), id(qzeros), id(bias))
    cached = _CACHE.get("prep")
    if cached is not None and cached[0] == key:
        return cached[1]

    x = np.asarray(x)
    qweight = np.asarray(qweight)
    scales = np.asarray(scales)
    qzeros = np.asarray(qzeros)
    bias = np.asarray(bias)

    x2 = np.ascontiguousarray(x.reshape(T, I))
    if x2.dtype != np.float16:
        x2 = x2.astype(np.float16)

    # qw tile i, partition p <- u16 row 16p+i of qweight.view(u16).T
    qw16 = qweight.view(np.uint16)              # [O, 2048] (nibble c=4r+k in row o)
    qwp = np.ascontiguousarray(
        qw16.reshape(O, 128, 16).transpose(2, 1, 0)
    ).reshape(2048, O)

    scT = scales.T.astype(np.float16)           # [64, O]
    rep = np.arange(128) // 2
    s128 = np.ascontiguousarray(scT[rep])       # [128, O]

    # zeros: znib[g, o] = nibble g%4 of u16 word g//4 of qzeros row o
    qz16 = qzeros.view(np.uint16)               # [O, 16]
    g = np.arange(NG)
    znib = (qz16.T[g // 4] >> (4 * (g % 4))[:, None]).astype(np.uint16) & 15
    bm_half = -(scT.astype(np.float32) * znib.astype(np.float32))  # [64, O]
    bmat = bm_half[rep].astype(np.float16)
    bmat[127] = bias.astype(np.float16)

    # A matrix: identity, col 126 also sums row 127, col 127 dead
    amat = np.eye(128, dtype=np.float16)
    amat[127, 127] = 0.0
    amat[127, 126] = 1.0
    # C: row 127 ones (bias row of R)
    cmat = np.zeros((128, 128), np.float16)
    cmat[127, :T] = 1.0

    maps = []
    for c in range(N_CORES):
        osl = slice(OSH * c, OSH * (c + 1))
        aux = np.concatenate(
            [s128[:, osl], bmat[:, osl], amat, cmat], axis=1
        ).astype(np.float16)
        maps.append({
            "qw": np.ascontiguousarray(qwp[:, osl]),
            "xin": x2,
            "aux": np.ascontiguousarray(aux),
        })
    _CACHE["prep"] = (key, maps)
    return maps


def _get_nc(n_iters=1, hw_loop=False, unroll=1):
    key = ("nc", n_iters, hw_loop, unroll)
    if key not in _CACHE:
        _CACHE[key] = _build_nc(n_iters, hw_loop, unroll)
    return _CACHE[key]


def _gather(results):
    out = np.concatenate(
        [np.asarray(results[c]["out"]) for c in range(N_CORES)], axis=1
    )
    return np.ascontiguousarray(out.reshape(4, 32, O).astype(np.float32))


def run(inputs, trace=False, trace_cores=None):
    nc = _get_nc(1)
    maps = _prep_in_maps(**inputs)
    res = run_bass_kernel_spmd(nc, maps, list(range(N_CORES)), trace=trace,
                               trace_cores=trace_cores)
    return _gather(res.results), res


def kernel(**inputs) -> np.ndarray:
    out, _ = run(inputs, trace=False)
    return out


K_LO = 8
K_HI = 808


def _build_sharded(nc):
    import jax
    from jax.sharding import Mesh, PartitionSpec
    from jax.experimental.shard_map import shard_map
    from concourse import bass2jax
    import concourse.mybir as mb

    partition_name = nc.partition_id_tensor.name if nc.partition_id_tensor else None
    in_names, out_names, out_avals, zero_outs = [], [], [], []
    for alloc in nc.m.functions[0].allocations:
        if not isinstance(alloc, mb.MemoryLocationSet):
            continue
        name = alloc.memorylocations[0].name
        if alloc.kind == "ExternalInput":
            if name != partition_name:
                in_names.append(name)
        elif alloc.kind == "ExternalOutput":
            out_names.append(name)
            shape = tuple(alloc.tensor_shape)
            dtype = mb.dt.np(alloc.dtype)
            out_avals.append(jax.core.ShapedArray(shape, dtype))
            zero_outs.append(np.zeros(shape, dtype))
    n_params = len(in_names)
    in_names_all = in_names + out_names
    if partition_name is not None:
        in_names_all.append(partition_name)

    def _body(*args):
        operands = list(args)
        if partition_name is not None:
            operands.append(bass2jax.partition_id_tensor())
        outs = bass2jax._bass_exec_p.bind(
            *operands,
            out_avals=tuple(out_avals),
            in_names=tuple(in_names_all),
            out_names=tuple(out_names),
            lowering_input_output_aliases=(),
            sim_require_finite=True,
            sim_require_nnan=True,
            nc=nc,
        )
        return tuple(outs)

    devices = jax.devices()[:N_CORES]
    mesh = Mesh(np.asarray(devices), ("core",))
    n_outs = len(out_names)
    sharded = jax.jit(
        shard_map(
            _body, mesh=mesh,
            in_specs=(PartitionSpec("core"),) * (n_params + n_outs),
            out_specs=(PartitionSpec("core"),) * n_outs,
            check_rep=False,
        ),
        keep_unused=True,
    )
    return sharded, in_names, zero_outs


def bench(inputs, n_rep=8):
    """Time the K_LO- and K_HI-iteration unrolled programs; the slope
    isolates steady-state per-iteration device time from the (large, noisy)
    per-execute axon dispatch constant."""
    import time
    import jax
    from concourse import bass2jax

    bass2jax.install_neuronx_cc_hook()
    maps = _prep_in_maps(**inputs)

    runners = {}
    for k_it in (K_LO, K_HI):
        nc = _get_nc(k_it, hw_loop=True, unroll=8)
        sharded, in_names, zero_outs = _build_sharded(nc)
        concat_in = [
            np.concatenate([np.asarray(maps[c][nm]) for c in range(N_CORES)],
                           axis=0)
            for nm in in_names
        ]
        concat_zeros = [
            np.zeros((N_CORES * z.shape[0], *z.shape[1:]), z.dtype)
            for z in zero_outs
        ]
        args_dev = [jax.device_put(a) for a in concat_in + concat_zeros]
        outs = sharded(*args_dev)
        jax.block_until_ready(outs)
        runners[k_it] = (sharded, args_dev, outs)

    def timed(k_it):
        sharded, args_dev, _ = runners[k_it]
        o = sharded(*args_dev)
        jax.block_until_ready(o)
        t0 = time.time()
        o = sharded(*args_dev)
        jax.block_until_ready(o)
        return time.time() - t0

    lo = [timed(K_LO) for _ in range(n_rep)]
    hi = [timed(K_HI) for _ in range(n_rep)]
    per_iter_ns = (min(hi) - min(lo)) / (K_HI - K_LO) * 1e9

    outs = runners[K_HI][2]
    full = np.asarray(outs[0])          # [N_CORES*T, OSH] concat along axis0
    parts = [full[c * T : (c + 1) * T] for c in range(N_CORES)]
    out = np.concatenate(parts, axis=1).reshape(4, 32, O).astype(np.float32)
    return per_iter_ns, out, (min(lo), min(hi))


# revision 26
# speedup vs baseline: 2.9665x; 1.0368x over previous
"""AWQ 4-bit quantized linear (out = x @ dequant(qweight).T + bias), 8-core
tensor-parallel on TRN2.

Sharding: out_features split 8 ways (O' = 1024 per core); x replicated; each
core computes out[:, c*1024:(c+1)*1024] and the host concatenates.  No
device collectives.

Per-core layout (same trick as the 1-core baseline, O-sliced): qweight u16
rows are permuted so SBUF tile i holds u16-rows {16p + i : p in [0,128)}.
Input column c = 4r + k = 64p + (4i + k); the AWQ group of column c is
g = p//2, independent of (i,k), so one [128, O'] scale tile
s128[p,o] = scales[o, p//2] serves every dequant op, and the x-transposes
are plain stride-64 column slices of x.

Dequant per tile/plane: nib = qw & (15<<4k) (DVE tensor_scalar, 4x mode),
w = nib * s128 (tensor_tensor, 2x mode).  The 2^{4k} nibble-position factor
is compensated on the x side during PSUM->SBUF eviction (ACT).  A subset of
the w-multiplies runs on the GPSIMD (Pool) engine to unload DVE, which is
otherwise the bottleneck engine.

Zeros + bias fold into one extra K=128 matmul: R[p,t] = sum of raw x over
columns [64p, 64p+64) (recovered from the scaled transposes via identity-
matmul accumulation and a 16^k recombination), paired with
bmat[p,o] = -(s*z)[p//2, o]; row 127 of R is forced to 1.0 and bmat[127] =
bias (row 127's half-group sum is folded into row 126 by matrix A).

Benchmarking: the per-execute axon dispatch costs ~1 ms/core and swamps
wall-clock timing of single executions.  bench() therefore builds a second
Bass module with the whole per-iteration body unrolled K_HI times (each
iteration re-DMAs all inputs from DRAM and re-writes the output) and
measures the slope between the K_LO- and K_HI-iteration programs: the
marginal cost of one extra full computation on-device, i.e. the steady-state
HW execution time per iteration.
"""

import numpy as np
from contextlib import ExitStack

import concourse.bass as bass
import concourse.mybir as mybir
import concourse.tile as tile
from concourse.bass_utils import run_bass_kernel_spmd
from concourse.masks import make_identity

dt = mybir.dt

N_CORES = 8
I = 8192                    # in_features
O = 8192                    # out_features
OSH = O // N_CORES          # 1024 out-features per core
T = 128                     # batch*seq = 4*32
NG = 64                     # groups (group_size 128)
NR = 64                     # residue tiles (r64 = 4i + k)
NS = OSH // 512             # 512-wide matmul chunks per out block (2)

_CACHE = {}

_KORD = (3, 0, 1, 2)        # k=3 (v-plane, no AND) first so DVE starts early


def _emit_iteration(nc, tc, P):
    """Emit one full computation: load everything from DRAM, dequantize,
    matmul, correct zeros+bias, store out.

    qw tiles are processed in PAIRS (one [128, 2*OSH] SBUF tile covering
    original tiles 2j and 2j+1) so each DVE dequant op handles 2*OSH
    elements, halving per-instruction overhead.  The scale operand is a
    doubled tile s128d = s128|s128.  DVE emission order is chosen to avoid
    head-of-line blocking on the in-order engine: the k=3 multiplies of the
    first two pairs (which need only qw + scales) run before the y-plane
    subtracts (which need the x transposes)."""
    qw_d, x_d, aux_d, out_d = P["qw_d"], P["x_d"], P["aux_d"], P["out_d"]
    ident = P["ident"]

    # s128d first on the scalar queue: it unblocks the early k=3 DVE
    # multiplies; x follows (feeds the PE transposes); the bias/correction
    # tables are only needed late.
    s128d = P["cst_p"].tile([128, 2 * OSH], dt.float16, tag="s128d")
    nc.scalar.dma_start(s128d[:, 0:OSH], aux_d[:, 0:OSH])
    nc.scalar.dma_start(s128d[:, OSH : 2 * OSH], aux_d[:, 0:OSH])
    x_sb = P["x_p"].tile([T, I], dt.float16, tag="x")
    nc.scalar.dma_start(x_sb[:], x_d[:, :])
    # amat4: four copies of the A matrix pre-scaled by 16^k; the R matmuls
    # for plane k use amat4[:, 128k:128(k+1)] so all 64 accumulate into ONE
    # psum slot and no 16^k recombination pass is needed.
    amat4 = P["sm_p"].tile([128, 512], dt.float16, tag="amat4")
    nc.scalar.dma_start(amat4[:], aux_d[:, 2 * OSH : 2 * OSH + 512])
    cmat = P["sm_p"].tile([128, T], dt.float16, tag="cmat")
    nc.scalar.dma_start(cmat[:], aux_d[:, 2 * OSH + 512 : 2 * OSH + 640])
    bmat = P["cst_p"].tile([128, OSH], dt.float16, tag="bmat")
    nc.scalar.dma_start(bmat[:], aux_d[:, OSH : 2 * OSH])

    # ---- early DVE work: k=3 dequant of ALL pairs needs only qw+scales,
    # so DVE is busy while the x transposes are still in flight ----
    qwts, w3s = {}, {}
    for j in range(8):
        qwt = P["qwt_p"].tile([128, 2 * OSH], dt.uint16, tag="qwt")
        nc.sync.dma_start(qwt[:, 0:OSH], qw_d[256 * j : 256 * j + 128, :])
        nc.sync.dma_start(qwt[:, OSH : 2 * OSH],
                          qw_d[256 * j + 128 : 256 * j + 256, :])
        qwts[j] = qwt
        w3 = P["w3_p"].tile([128, 2 * OSH], dt.float16, tag="w3")
        nc.vector.tensor_tensor(out=w3[:], in0=qwt[:], in1=s128d[:],
                                op=mybir.AluOpType.mult)
        w3s[j] = w3

    # ---- preamble: 64 transposes, 4-per-PSUM-tile, evicted into one
    # [128, 2048] tile per nibble plane k (quarter q at 512*q).
    # k=3 first: its transposes feed both the y-subtracts and the early
    # k=3 matmuls. ----
    x_r = x_sb.rearrange("t (p r) -> t r p", r=NR)
    xts, yts = {}, {}
    for k in _KORD:
        xt = P["xt_p"].tile([128, 4 * 4 * T], dt.float16, tag=f"xt{k}")
        for q in range(4):
            ps = P["pst_p"].tile([128, 4 * T], dt.float16, tag="tp")
            for m in range(4):
                r64 = 4 * (4 * q + m) + k
                nc.tensor.transpose(
                    ps[:, T * m : T * (m + 1)], x_r[:, r64, :], ident[:]
                )
            if k == 0:
                nc.scalar.copy(xt[:, 512 * q : 512 * (q + 1)], ps[:])
            else:
                nc.scalar.mul(xt[:, 512 * q : 512 * (q + 1)], ps[:],
                              float(2.0 ** (-4 * k)))
        xts[k] = xt

    # V-plane trick: plane k=3 is the FULL u16 value (w = v*s, no AND),
    # paired with a3; planes k<3 pair with y_k = a_k - a_3 so the extra
    # terms telescope away exactly.
    for k in range(3):
        y = P["xt_p"].tile([128, 4 * 4 * T], dt.float16, tag=f"y{k}")
        nc.vector.tensor_tensor(out=y[:], in0=xts[k][:], in1=xts[3][:],
                                op=mybir.AluOpType.subtract)
        yts[k] = y

    def xsl(k, i):
        q, m = i // 4, i % 4
        src = xts[3] if k == 3 else yts[k]
        return src[:, 512 * q + T * m : 512 * q + T * (m + 1)]

    # ---- all k=3 matmuls up front (w3 + xts[3] are ready first) ----
    out_ps = P["pso_p"].tile([T, OSH], dt.float32, tag="out")
    for j in range(8):
        for h in range(2):
            i = 2 * j + h
            for ns in range(NS):
                nc.tensor.matmul(
                    out_ps[:, 512 * ns : 512 * (ns + 1)],
                    xsl(3, i),
                    w3s[j][:, OSH * h + 512 * ns : OSH * h + 512 * (ns + 1)],
                    start=(j == 0 and h == 0),
                    stop=False,
                )

    # ---- half-group sums of raw x -> R (PE): R = sum_k 16^k A @ a_k ----
    psum_x = P["psx_p"].tile([128, T], dt.float32, tag="psx")
    for k in range(4):
        for q in range(4):
            for m in range(4):
                nc.tensor.matmul(
                    psum_x[:],
                    amat4[:, 128 * k : 128 * (k + 1)],
                    xts[k][:, 512 * q + T * m : 512 * q + T * (m + 1)],
                    start=(k == 0 and q == 0 and m == 0),
                    stop=(k == 3 and q == 3 and m == 3),
                )

    def emit_pair(j):
        qwt = qwts[j]
        for k in (0, 1, 2):
            nib = P["nib_p"].tile([128, 2 * OSH], dt.uint16, tag="nib")
            nc.vector.tensor_scalar(
                out=nib[:], in0=qwt[:], scalar1=15 << (4 * k),
                scalar2=None, op0=mybir.AluOpType.bitwise_and,
            )
            w = P["w_p"].tile([128, 2 * OSH], dt.float16, tag="w")
            nc.vector.tensor_tensor(out=w[:], in0=nib[:], in1=s128d[:],
                                    op=mybir.AluOpType.mult)
            for h in range(2):
                i = 2 * j + h
                for ns in range(NS):
                    nc.tensor.matmul(
                        out_ps[:, 512 * ns : 512 * (ns + 1)],
                        xsl(k, i),
                        w[:, OSH * h + 512 * ns : OSH * h + 512 * (ns + 1)],
                        start=False,
                        stop=False,
                    )

    for j in range(5):
        emit_pair(j)

    # row 127 (zeroed by A) becomes the bias row: rmat = psum_x + C
    rmat = P["sm_p"].tile([128, T], dt.float16, tag="rmat")
    nc.vector.tensor_tensor(
        out=rmat[:], in0=psum_x[:], in1=cmat[:], op=mybir.AluOpType.add
    )

    for j in range(5, 8):
        emit_pair(j)

    # zeros + bias correction
    for ns in range(NS):
        nc.tensor.matmul(
            out_ps[:, 512 * ns : 512 * (ns + 1)],
            rmat[:],
            bmat[:, 512 * ns : 512 * (ns + 1)],
            start=False, stop=(ns == NS - 1),
        )
    osb = P["osb_p"].tile([T, OSH], dt.float32, tag="osb")
    nc.scalar.copy(osb[:], out_ps[:])
    nc.scalar.dma_start(out_d[:, :], osb[:])


def _build_nc(n_iters=1, hw_loop=False, unroll=1):
    nc = bass.Bass()
    qw_d = nc.dram_tensor("qw", [2048, OSH], dt.uint16, kind="ExternalInput")
    x_d = nc.dram_tensor("xin", [T, I], dt.float16, kind="ExternalInput")
    # aux cols: 0:OSH s128 | OSH:2*OSH bmat | +512 amat4 | +128 cmat
    aux_d = nc.dram_tensor("aux", [128, 2 * OSH + 640], dt.float16,
                           kind="ExternalInput")
    out_d = nc.dram_tensor("out", [T, OSH], dt.float32, kind="ExternalOutput")

    with tile.TileContext(nc) as tc:
        with ExitStack() as ctx:
            P = {
                "qw_d": qw_d, "x_d": x_d, "aux_d": aux_d, "out_d": out_d,
                "x_p": ctx.enter_context(tc.tile_pool(name="x", bufs=2)),
                "cst_p": ctx.enter_context(tc.tile_pool(name="cst", bufs=2)),
                "sm_p": ctx.enter_context(tc.tile_pool(name="sm", bufs=2)),
                "xt_p": ctx.enter_context(tc.tile_pool(name="xt", bufs=2)),
                "qwt_p": ctx.enter_context(tc.tile_pool(name="qwt", bufs=9)),
                "nib_p": ctx.enter_context(tc.tile_pool(name="nib", bufs=2)),
                "w_p": ctx.enter_context(tc.tile_pool(name="w", bufs=3)),
                "w3_p": ctx.enter_context(tc.tile_pool(name="w3", bufs=8)),
                "osb_p": ctx.enter_context(tc.tile_pool(name="osb", bufs=2)),
                "sgl_p": ctx.enter_context(tc.tile_pool(name="sgl", bufs=1)),
                "pst_p": ctx.enter_context(
                    tc.tile_pool(name="pst", bufs=3, space="PSUM")),
                "psx_p": ctx.enter_context(
                    tc.tile_pool(name="psx", bufs=1, space="PSUM")),
                "pso_p": ctx.enter_context(
                    tc.tile_pool(name="pso", bufs=1, space="PSUM")),
            }
            ident = P["sgl_p"].tile([128, 128], dt.float16, tag="ident")
            make_identity(nc, ident[:])
            P["ident"] = ident
            if hw_loop:
                assert n_iters % unroll == 0
                with tc.For_i(0, n_iters // unroll):
                    for _ in range(unroll):
                        _emit_iteration(nc, tc, P)
            else:
                for _ in range(n_iters):
                    _emit_iteration(nc, tc, P)

    _split_excess_waits(nc)
    nc.finalize()
    return nc


_SPLIT_TYPES = {
    "InstTensorScalarPtr", "InstTensorTensor", "InstActivation", "InstMatmult",
    "InstDMACopy", "InstDmaTransposeAnt", "InstMemSet", "InstTensorCopy",
    "InstTensorReduce", "InstDrain", "InstMemset", "InstNoOp",
}

_ENG_MAP = {
    "DVE": "vector", "Activation": "scalar", "PE": "tensor",
    "Pool": "gpsimd", "SP": "sync",
}


def _split_excess_waits(nc):
    """walrus accepts at most one sync-wait per (non-drain) instruction in
    this build; move excess waits onto same-engine ENGINE_NOPs inserted just
    before the instruction."""
    for bb in nc.main_func.blocks:
        insts = list(bb.instructions)
        need = []  # (idx, inst, extra_waits)
        for idx, ins in enumerate(insts):
            if type(ins).__name__ not in _SPLIT_TYPES:
                continue
            si = ins.sync_info
            w = list(si.on_wait) if si else []
            if len(w) > 1:
                need.append((idx, ins, w))
        if not need:
            continue
        created = {}
        for idx, ins, w in need:
            eng = _ENG_MAP.get(ins.engine.name if ins.engine else "", "vector")
            nops = []
            for extra in w[:-1]:
                bi = getattr(nc, eng).nop()
                nop = bi.ins
                nop.sync_info = mybir.SyncInfo(on_wait=[extra], on_update=[])
                nops.append(nop)
            ins.sync_info = mybir.SyncInfo(
                on_wait=[w[-1]], on_update=list(ins.sync_info.on_update))
            created[idx] = nops
        nop_names = {n.name for nops in created.values() for n in nops}
        for bb2 in nc.main_func.blocks:
            cur = [i for i in bb2.instructions if i.name not in nop_names]
            if bb2.name == bb.name:
                out = []
                for idx, ins in enumerate(insts):
                    if idx in created:
                        out.extend(created[idx])
                    out.append(ins)
                bb2.instructions = out
            elif len(cur) != len(list(bb2.instructions)):
                bb2.instructions = cur


def _prep_in_maps(x, qweight, scales, qzeros, bias):
    """Host prep: repack qweight to the permuted-u16 layout and build the
    per-core aux tables.  Cached on input identity."""
    key = (id(x), id(qweight), id(scales), id(qzeros), id(bias))
    cached = _CACHE.get("prep")
    if cached is not None and cached[0] == key:
        return cached[1]

    x = np.asarray(x)
    qweight = np.asarray(qweight)
    scales = np.asarray(scales)
    qzeros = np.asarray(qzeros)
    bias = np.asarray(bias)

    x2 = np.ascontiguousarray(x.reshape(T, I))
    if x2.dtype != np.float16:
        x2 = x2.astype(np.float16)

    # qw tile i, partition p <- u16 row 16p+i of qweight.view(u16).T
    qw16 = qweight.view(np.uint16)              # [O, 2048] (nibble c=4r+k in row o)
    qwp = np.ascontiguousarray(
        qw16.reshape(O, 128, 16).transpose(2, 1, 0)
    ).reshape(2048, O)

    scT = scales.T.astype(np.float16)           # [64, O]
    rep = np.arange(128) // 2
    s128 = np.ascontiguousarray(scT[rep])       # [128, O]

    # zeros: znib[g, o] = nibble g%4 of u16 word g//4 of qzeros row o
    qz16 = qzeros.view(np.uint16)               # [O, 16]
    g = np.arange(NG)
    znib = (qz16.T[g // 4] >> (4 * (g % 4))[:, None]).astype(np.uint16) & 15
    bm_half = -(scT.astype(np.float32) * znib.astype(np.float32))  # [64, O]
    bmat = bm_half[rep].astype(np.float16)
    bmat[127] = bias.astype(np.float16)

    # A matrix: identity, col 126 also sums row 127, col 127 dead.
    # amat4[:, 128k:128(k+1)] = 16^k * A compensates the 2^-4k scaling of
    # the plane-k transposes, so the 64 R matmuls share one psum slot.
    amat = np.eye(128, dtype=np.float32)
    amat[127, 127] = 0.0
    amat[127, 126] = 1.0
    amat4 = np.concatenate(
        [amat * float(16 ** k) for k in range(4)], axis=1
    ).astype(np.float16)
    # C: row 127 ones (bias row of R)
    cmat = np.zeros((128, 128), np.float16)
    cmat[127, :T] = 1.0

    maps = []
    for c in range(N_CORES):
        osl = slice(OSH * c, OSH * (c + 1))
        aux = np.concatenate(
            [s128[:, osl], bmat[:, osl], amat4, cmat], axis=1
        ).astype(np.float16)
        maps.append({
            "qw": np.ascontiguousarray(qwp[:, osl]),
            "xin": x2,
            "aux": np.ascontiguousarray(aux),
        })
    _CACHE["prep"] = (key, maps)
    return maps


def _get_nc(n_iters=1, hw_loop=False, unroll=1):
    key = ("nc", n_iters, hw_loop, unroll)
    if key not in _CACHE:
        _CACHE[key] = _build_nc(n_iters, hw_loop, unroll)
    return _CACHE[key]


def _gather(results):
    out = np.concatenate(
        [np.asarray(results[c]["out"]) for c in range(N_CORES)], axis=1
    )
    return np.ascontiguousarray(out.reshape(4, 32, O).astype(np.float32))


def run(inputs, trace=False, trace_cores=None):
    nc = _get_nc(1)
    maps = _prep_in_maps(**inputs)
    res = run_bass_kernel_spmd(nc, maps, list(range(N_CORES)), trace=trace,
                               trace_cores=trace_cores)
    return _gather(res.results), res


def kernel(**inputs) -> np.ndarray:
    out, _ = run(inputs, trace=False)
    return out


K_LO = 16
K_HI = 816


def _build_sharded(nc):
    import jax
    from jax.sharding import Mesh, PartitionSpec
    from jax.experimental.shard_map import shard_map
    from concourse import bass2jax
    import concourse.mybir as mb

    partition_name = nc.partition_id_tensor.name if nc.partition_id_tensor else None
    in_names, out_names, out_avals, zero_outs = [], [], [], []
    for alloc in nc.m.functions[0].allocations:
        if not isinstance(alloc, mb.MemoryLocationSet):
            continue
        name = alloc.memorylocations[0].name
        if alloc.kind == "ExternalInput":
            if name != partition_name:
                in_names.append(name)
        elif alloc.kind == "ExternalOutput":
            out_names.append(name)
            shape = tuple(alloc.tensor_shape)
            dtype = mb.dt.np(alloc.dtype)
            out_avals.append(jax.core.ShapedArray(shape, dtype))
            zero_outs.append(np.zeros(shape, dtype))
    n_params = len(in_names)
    in_names_all = in_names + out_names
    if partition_name is not None:
        in_names_all.append(partition_name)

    def _body(*args):
        operands = list(args)
        if partition_name is not None:
            operands.append(bass2jax.partition_id_tensor())
        outs = bass2jax._bass_exec_p.bind(
            *operands,
            out_avals=tuple(out_avals),
            in_names=tuple(in_names_all),
            out_names=tuple(out_names),
            lowering_input_output_aliases=(),
            sim_require_finite=True,
            sim_require_nnan=True,
            nc=nc,
        )
        return tuple(outs)

    devices = jax.devices()[:N_CORES]
    mesh = Mesh(np.asarray(devices), ("core",))
    n_outs = len(out_names)
    sharded = jax.jit(
        shard_map(
            _body, mesh=mesh,
            in_specs=(PartitionSpec("core"),) * (n_params + n_outs),
            out_specs=(PartitionSpec("core"),) * n_outs,
            check_rep=False,
        ),
        keep_unused=True,
    )
    return sharded, in_names, zero_outs


def bench(inputs, n_rep=8):
    """Time the K_LO- and K_HI-iteration unrolled programs; the slope
    isolates steady-state per-iteration device time from the (large, noisy)
    per-execute axon dispatch constant."""
    import time
    import jax
    from concourse import bass2jax

    bass2jax.install_neuronx_cc_hook()
    maps = _prep_in_maps(**inputs)

    runners = {}
    for k_it in (K_LO, K_HI):
        nc = _get_nc(k_it, hw_loop=True, unroll=16)
        sharded, in_names, zero_outs = _build_sharded(nc)
        concat_in = [
            np.concatenate([np.asarray(maps[c][nm]) for c in range(N_CORES)],
                           axis=0)
            for nm in in_names
        ]
        concat_zeros = [
            np.zeros((N_CORES * z.shape[0], *z.shape[1:]), z.dtype)
            for z in zero_outs
        ]
        args_dev = [jax.device_put(a) for a in concat_in + concat_zeros]
        outs = sharded(*args_dev)
        jax.block_until_ready(outs)
        runners[k_it] = (sharded, args_dev, outs)

    def timed(k_it):
        sharded, args_dev, _ = runners[k_it]
        o = sharded(*args_dev)
        jax.block_until_ready(o)
        t0 = time.time()
        o = sharded(*args_dev)
        jax.block_until_ready(o)
        return time.time() - t0

    lo = [timed(K_LO) for _ in range(n_rep)]
    hi = [timed(K_HI) for _ in range(n_rep)]
    per_iter_ns = (min(hi) - min(lo)) / (K_HI - K_LO) * 1e9

    outs = runners[K_HI][2]
    full = np.asarray(outs[0])          # [N_CORES*T, OSH] concat along axis0
    parts = [full[c * T : (c + 1) * T] for c in range(N_CORES)]
    out = np.concatenate(parts, axis=1).reshape(4, 32, O).astype(np.float32)
    return per_iter_ns, out, (min(lo), min(hi))


# revision 33
# speedup vs baseline: 2.9672x; 1.0002x over previous
"""AWQ 4-bit quantized linear (out = x @ dequant(qweight).T + bias), 8-core
tensor-parallel on TRN2.

Sharding: out_features split 8 ways (O' = 1024 per core); x replicated; each
core computes out[:, c*1024:(c+1)*1024] and the host concatenates.  No
device collectives.

Per-core layout (same trick as the 1-core baseline, O-sliced): qweight u16
rows are permuted so SBUF tile i holds u16-rows {16p + i : p in [0,128)}.
Input column c = 4r + k = 64p + (4i + k); the AWQ group of column c is
g = p//2, independent of (i,k), so one [128, O'] scale tile
s128[p,o] = scales[o, p//2] serves every dequant op, and the x-transposes
are plain stride-64 column slices of x.

Dequant per tile/plane: nib = qw & (15<<4k) (DVE tensor_scalar, 4x mode),
w = nib * s128 (tensor_tensor, 2x mode).  The 2^{4k} nibble-position factor
is compensated on the x side during PSUM->SBUF eviction (ACT).  A subset of
the w-multiplies runs on the GPSIMD (Pool) engine to unload DVE, which is
otherwise the bottleneck engine.

Zeros + bias fold into one extra K=128 matmul: R[p,t] = sum of raw x over
columns [64p, 64p+64) (recovered from the scaled transposes via identity-
matmul accumulation and a 16^k recombination), paired with
bmat[p,o] = -(s*z)[p//2, o]; row 127 of R is forced to 1.0 and bmat[127] =
bias (row 127's half-group sum is folded into row 126 by matrix A).

Benchmarking: the per-execute axon dispatch costs ~1 ms/core and swamps
wall-clock timing of single executions.  bench() therefore builds a second
Bass module with the whole per-iteration body unrolled K_HI times (each
iteration re-DMAs all inputs from DRAM and re-writes the output) and
measures the slope between the K_LO- and K_HI-iteration programs: the
marginal cost of one extra full computation on-device, i.e. the steady-state
HW execution time per iteration.
"""

import numpy as np
from contextlib import ExitStack

import concourse.bass as bass
import concourse.mybir as mybir
import concourse.tile as tile
from concourse.bass_utils import run_bass_kernel_spmd
from concourse.masks import make_identity

dt = mybir.dt

N_CORES = 8
I = 8192                    # in_features
O = 8192                    # out_features
OSH = O // N_CORES          # 1024 out-features per core
T = 128                     # batch*seq = 4*32
NG = 64                     # groups (group_size 128)
NR = 64                     # residue tiles (r64 = 4i + k)
NS = OSH // 512             # 512-wide matmul chunks per out block (2)

_CACHE = {}

_KORD = (3, 0, 1, 2)        # k=3 (v-plane, no AND) first so DVE starts early


def _emit_iteration(nc, tc, P):
    """Emit one full computation: load everything from DRAM, dequantize,
    matmul, correct zeros+bias, store out.

    qw tiles are processed in PAIRS (one [128, 2*OSH] SBUF tile covering
    original tiles 2j and 2j+1) so each DVE dequant op handles 2*OSH
    elements, halving per-instruction overhead.  The scale operand is a
    doubled tile s128d = s128|s128.  DVE emission order is chosen to avoid
    head-of-line blocking on the in-order engine: the k=3 multiplies of the
    first two pairs (which need only qw + scales) run before the y-plane
    subtracts (which need the x transposes)."""
    qw_d, x_d, aux_d, out_d = P["qw_d"], P["x_d"], P["aux_d"], P["out_d"]
    ident = P["ident"]

    # s128d first on the scalar queue: it unblocks the early k=3 DVE
    # multiplies; x follows (feeds the PE transposes); the bias/correction
    # tables are only needed late.
    s128 = P["cst_p"].tile([128, OSH], dt.float16, tag="s128")
    nc.scalar.dma_start(s128[:], aux_d[:, 0:OSH])
    x_sb = P["x_p"].tile([T, I], dt.float16, tag="x")
    nc.scalar.dma_start(x_sb[:], x_d[:, :])
    # amat4: four copies of the A matrix pre-scaled by 16^k; the R matmuls
    # for plane k use amat4[:, 128k:128(k+1)] so all 64 accumulate into ONE
    # psum slot and no 16^k recombination pass is needed.
    amat4 = P["sm_p"].tile([128, 512], dt.float16, tag="amat4")
    nc.scalar.dma_start(amat4[:], aux_d[:, 2 * OSH : 2 * OSH + 512])
    cmat = P["sm_p"].tile([128, T], dt.float16, tag="cmat")
    nc.scalar.dma_start(cmat[:], aux_d[:, 2 * OSH + 512 : 2 * OSH + 640])
    bmat = P["cst_p"].tile([128, OSH], dt.float16, tag="bmat")
    nc.scalar.dma_start(bmat[:], aux_d[:, OSH : 2 * OSH])

    # ---- early DVE work: k=3 dequant of ALL quads needs only qw+scales,
    # so DVE is busy while the x transposes are still in flight ----
    def bq(ap):        # view a [128, 4*OSH] quad op as [128, 4, OSH]
        return ap.rearrange("p (a b) -> p a b", a=4)

    s128b = s128[:].unsqueeze(1).broadcast_to([128, 4, OSH])
    qwts, w3s = {}, {}
    for j in range(4):
        qwt = P["qwt_p"].tile([128, 4 * OSH], dt.uint16, tag="qwt")
        for h in range(4):
            r0 = 512 * j + 128 * h
            nc.sync.dma_start(qwt[:, OSH * h : OSH * (h + 1)],
                              qw_d[r0 : r0 + 128, :])
        qwts[j] = qwt
        w3 = P["w3_p"].tile([128, 4 * OSH], dt.float16, tag="w3")
        nc.vector.tensor_tensor(out=bq(w3[:]), in0=bq(qwt[:]), in1=s128b,
                                op=mybir.AluOpType.mult)
        w3s[j] = w3

    # ---- preamble: 64 transposes, 4-per-PSUM-tile, evicted into one
    # [128, 2048] tile per nibble plane k (quarter q at 512*q).
    # k=3 first: its transposes feed both the y-subtracts and the early
    # k=3 matmuls. ----
    x_r = x_sb.rearrange("t (p r) -> t r p", r=NR)
    xts, yts = {}, {}
    for k in _KORD:
        xt = P["xt_p"].tile([128, 4 * 4 * T], dt.float16, tag=f"xt{k}")
        for q in range(4):
            ps = P["pst_p"].tile([128, 4 * T], dt.float16, tag="tp")
            for m in range(4):
                r64 = 4 * (4 * q + m) + k
                nc.tensor.transpose(
                    ps[:, T * m : T * (m + 1)], x_r[:, r64, :], ident[:]
                )
            if k == 0:
                nc.scalar.copy(xt[:, 512 * q : 512 * (q + 1)], ps[:])
            else:
                nc.scalar.mul(xt[:, 512 * q : 512 * (q + 1)], ps[:],
                              float(2.0 ** (-4 * k)))
        xts[k] = xt

    # V-plane trick: plane k=3 is the FULL u16 value (w = v*s, no AND),
    # paired with a3; planes k<3 pair with y_k = a_k - a_3 so the extra
    # terms telescope away exactly.
    for k in range(3):
        y = P["xt_p"].tile([128, 4 * 4 * T], dt.float16, tag=f"y{k}")
        nc.vector.tensor_tensor(out=y[:], in0=xts[k][:], in1=xts[3][:],
                                op=mybir.AluOpType.subtract)
        yts[k] = y

    def xsl(k, i):
        q, m = i // 4, i % 4
        src = xts[3] if k == 3 else yts[k]
        return src[:, 512 * q + T * m : 512 * q + T * (m + 1)]

    # ---- all k=3 matmuls up front (w3 + xts[3] are ready first) ----
    out_ps = P["pso_p"].tile([T, OSH], dt.float32, tag="out")
    for j in range(4):
        for h in range(4):
            i = 4 * j + h
            for ns in range(NS):
                nc.tensor.matmul(
                    out_ps[:, 512 * ns : 512 * (ns + 1)],
                    xsl(3, i),
                    w3s[j][:, OSH * h + 512 * ns : OSH * h + 512 * (ns + 1)],
                    start=(j == 0 and h == 0),
                    stop=False,
                )

    # ---- half-group sums of raw x -> R (PE): R = sum_k 16^k A @ a_k ----
    psum_x = P["psx_p"].tile([128, T], dt.float32, tag="psx")
    for k in range(4):
        for q in range(4):
            for m in range(4):
                nc.tensor.matmul(
                    psum_x[:],
                    amat4[:, 128 * k : 128 * (k + 1)],
                    xts[k][:, 512 * q + T * m : 512 * q + T * (m + 1)],
                    start=(k == 0 and q == 0 and m == 0),
                    stop=(k == 3 and q == 3 and m == 3),
                )

    def emit_quad(j):
        qwt = qwts[j]
        for k in (0, 1, 2):
            nib = P["nib_p"].tile([128, 4 * OSH], dt.uint16, tag="nib")
            nc.vector.tensor_scalar(
                out=nib[:], in0=qwt[:], scalar1=15 << (4 * k),
                scalar2=None, op0=mybir.AluOpType.bitwise_and,
            )
            w = P["w_p"].tile([128, 4 * OSH], dt.float16, tag="w")
            nc.vector.tensor_tensor(out=bq(w[:]), in0=bq(nib[:]), in1=s128b,
                                    op=mybir.AluOpType.mult)
            for h in range(4):
                i = 4 * j + h
                for ns in range(NS):
                    nc.tensor.matmul(
                        out_ps[:, 512 * ns : 512 * (ns + 1)],
                        xsl(k, i),
                        w[:, OSH * h + 512 * ns : OSH * h + 512 * (ns + 1)],
                        start=False,
                        stop=False,
                    )

    for j in range(3):
        emit_quad(j)

    # row 127 (zeroed by A) becomes the bias row: rmat = psum_x + C
    rmat = P["sm_p"].tile([128, T], dt.float16, tag="rmat")
    nc.vector.tensor_tensor(
        out=rmat[:], in0=psum_x[:], in1=cmat[:], op=mybir.AluOpType.add
    )

    emit_quad(3)

    # zeros + bias correction
    for ns in range(NS):
        nc.tensor.matmul(
            out_ps[:, 512 * ns : 512 * (ns + 1)],
            rmat[:],
            bmat[:, 512 * ns : 512 * (ns + 1)],
            start=False, stop=(ns == NS - 1),
        )
    osb = P["osb_p"].tile([T, OSH], dt.float32, tag="osb")
    nc.scalar.copy(osb[:], out_ps[:])
    nc.scalar.dma_start(out_d[:, :], osb[:])


def _build_nc(n_iters=1, hw_loop=False, unroll=1):
    nc = bass.Bass()
    qw_d = nc.dram_tensor("qw", [2048, OSH], dt.uint16, kind="ExternalInput")
    x_d = nc.dram_tensor("xin", [T, I], dt.float16, kind="ExternalInput")
    # aux cols: 0:OSH s128 | OSH:2*OSH bmat | +512 amat4 | +128 cmat
    aux_d = nc.dram_tensor("aux", [128, 2 * OSH + 640], dt.float16,
                           kind="ExternalInput")
    out_d = nc.dram_tensor("out", [T, OSH], dt.float32, kind="ExternalOutput")

    with tile.TileContext(nc) as tc:
        with ExitStack() as ctx:
            P = {
                "qw_d": qw_d, "x_d": x_d, "aux_d": aux_d, "out_d": out_d,
                "x_p": ctx.enter_context(tc.tile_pool(name="x", bufs=2)),
                "cst_p": ctx.enter_context(tc.tile_pool(name="cst", bufs=2)),
                "sm_p": ctx.enter_context(tc.tile_pool(name="sm", bufs=2)),
                "xt_p": ctx.enter_context(tc.tile_pool(name="xt", bufs=1)),
                "qwt_p": ctx.enter_context(tc.tile_pool(name="qwt", bufs=5)),
                "nib_p": ctx.enter_context(tc.tile_pool(name="nib", bufs=2)),
                "w_p": ctx.enter_context(tc.tile_pool(name="w", bufs=3)),
                "w3_p": ctx.enter_context(tc.tile_pool(name="w3", bufs=5)),
                "osb_p": ctx.enter_context(tc.tile_pool(name="osb", bufs=2)),
                "sgl_p": ctx.enter_context(tc.tile_pool(name="sgl", bufs=1)),
                "pst_p": ctx.enter_context(
                    tc.tile_pool(name="pst", bufs=3, space="PSUM")),
                "psx_p": ctx.enter_context(
                    tc.tile_pool(name="psx", bufs=1, space="PSUM")),
                "pso_p": ctx.enter_context(
                    tc.tile_pool(name="pso", bufs=1, space="PSUM")),
            }
            ident = P["sgl_p"].tile([128, 128], dt.float16, tag="ident")
            make_identity(nc, ident[:])
            P["ident"] = ident
            if hw_loop:
                assert n_iters % unroll == 0
                with tc.For_i(0, n_iters // unroll):
                    for _ in range(unroll):
                        _emit_iteration(nc, tc, P)
            else:
                for _ in range(n_iters):
                    _emit_iteration(nc, tc, P)

    _split_excess_waits(nc)
    nc.finalize()
    return nc


_SPLIT_TYPES = {
    "InstTensorScalarPtr", "InstTensorTensor", "InstActivation", "InstMatmult",
    "InstDMACopy", "InstDmaTransposeAnt", "InstMemSet", "InstTensorCopy",
    "InstTensorReduce", "InstDrain", "InstMemset", "InstNoOp",
}

_ENG_MAP = {
    "DVE": "vector", "Activation": "scalar", "PE": "tensor",
    "Pool": "gpsimd", "SP": "sync",
}


def _split_excess_waits(nc):
    """walrus accepts at most one sync-wait per (non-drain) instruction in
    this build; move excess waits onto same-engine ENGINE_NOPs inserted just
    before the instruction."""
    for bb in nc.main_func.blocks:
        insts = list(bb.instructions)
        need = []  # (idx, inst, extra_waits)
        for idx, ins in enumerate(insts):
            if type(ins).__name__ not in _SPLIT_TYPES:
                continue
            si = ins.sync_info
            w = list(si.on_wait) if si else []
            if len(w) > 1:
                need.append((idx, ins, w))
        if not need:
            continue
        created = {}
        for idx, ins, w in need:
            eng = _ENG_MAP.get(ins.engine.name if ins.engine else "", "vector")
            nops = []
            for extra in w[:-1]:
                bi = getattr(nc, eng).nop()
                nop = bi.ins
                nop.sync_info = mybir.SyncInfo(on_wait=[extra], on_update=[])
                nops.append(nop)
            ins.sync_info = mybir.SyncInfo(
                on_wait=[w[-1]], on_update=list(ins.sync_info.on_update))
            created[idx] = nops
        nop_names = {n.name for nops in created.values() for n in nops}
        for bb2 in nc.main_func.blocks:
            cur = [i for i in bb2.instructions if i.name not in nop_names]
            if bb2.name == bb.name:
                out = []
                for idx, ins in enumerate(insts):
                    if idx in created:
                        out.extend(created[idx])
                    out.append(ins)
                bb2.instructions = out
            elif len(cur) != len(list(bb2.instructions)):
                bb2.instructions = cur


def _prep_in_maps(x, qweight, scales, qzeros, bias):
    """Host prep: repack qweight to the permuted-u16 layout and build the
    per-core aux tables.  Cached on input identity."""
    key = (id(x), id(qweight), id(scales), id(qzeros), id(bias))
    cached = _CACHE.get("prep")
    if cached is not None and cached[0] == key:
        return cached[1]

    x = np.asarray(x)
    qweight = np.asarray(qweight)
    scales = np.asarray(scales)
    qzeros = np.asarray(qzeros)
    bias = np.asarray(bias)

    x2 = np.ascontiguousarray(x.reshape(T, I))
    if x2.dtype != np.float16:
        x2 = x2.astype(np.float16)

    # qw tile i, partition p <- u16 row 16p+i of qweight.view(u16).T
    qw16 = qweight.view(np.uint16)              # [O, 2048] (nibble c=4r+k in row o)
    qwp = np.ascontiguousarray(
        qw16.reshape(O, 128, 16).transpose(2, 1, 0)
    ).reshape(2048, O)

    scT = scales.T.astype(np.float16)           # [64, O]
    rep = np.arange(128) // 2
    s128 = np.ascontiguousarray(scT[rep])       # [128, O]

    # zeros: znib[g, o] = nibble g%4 of u16 word g//4 of qzeros row o
    qz16 = qzeros.view(np.uint16)               # [O, 16]
    g = np.arange(NG)
    znib = (qz16.T[g // 4] >> (4 * (g % 4))[:, None]).astype(np.uint16) & 15
    bm_half = -(scT.astype(np.float32) * znib.astype(np.float32))  # [64, O]
    bmat = bm_half[rep].astype(np.float16)
    bmat[127] = bias.astype(np.float16)

    # A matrix: identity, col 126 also sums row 127, col 127 dead.
    # amat4[:, 128k:128(k+1)] = 16^k * A compensates the 2^-4k scaling of
    # the plane-k transposes, so the 64 R matmuls share one psum slot.
    amat = np.eye(128, dtype=np.float32)
    amat[127, 127] = 0.0
    amat[127, 126] = 1.0
    amat4 = np.concatenate(
        [amat * float(16 ** k) for k in range(4)], axis=1
    ).astype(np.float16)
    # C: row 127 ones (bias row of R)
    cmat = np.zeros((128, 128), np.float16)
    cmat[127, :T] = 1.0

    maps = []
    for c in range(N_CORES):
        osl = slice(OSH * c, OSH * (c + 1))
        aux = np.concatenate(
            [s128[:, osl], bmat[:, osl], amat4, cmat], axis=1
        ).astype(np.float16)
        maps.append({
            "qw": np.ascontiguousarray(qwp[:, osl]),
            "xin": x2,
            "aux": np.ascontiguousarray(aux),
        })
    _CACHE["prep"] = (key, maps)
    return maps


def _get_nc(n_iters=1, hw_loop=False, unroll=1):
    key = ("nc", n_iters, hw_loop, unroll)
    if key not in _CACHE:
        _CACHE[key] = _build_nc(n_iters, hw_loop, unroll)
    return _CACHE[key]


def _gather(results):
    out = np.concatenate(
        [np.asarray(results[c]["out"]) for c in range(N_CORES)], axis=1
    )
    return np.ascontiguousarray(out.reshape(4, 32, O).astype(np.float32))


def run(inputs, trace=False, trace_cores=None):
    nc = _get_nc(1)
    maps = _prep_in_maps(**inputs)
    res = run_bass_kernel_spmd(nc, maps, list(range(N_CORES)), trace=trace,
                               trace_cores=trace_cores)
    return _gather(res.results), res


def kernel(**inputs) -> np.ndarray:
    out, _ = run(inputs, trace=False)
    return out


K_LO = 16
K_HI = 816


def _build_sharded(nc):
    import jax
    from jax.sharding import Mesh, PartitionSpec
    from jax.experimental.shard_map import shard_map
    from concourse import bass2jax
    import concourse.mybir as mb

    partition_name = nc.partition_id_tensor.name if nc.partition_id_tensor else None
    in_names, out_names, out_avals, zero_outs = [], [], [], []
    for alloc in nc.m.functions[0].allocations:
        if not isinstance(alloc, mb.MemoryLocationSet):
            continue
        name = alloc.memorylocations[0].name
        if alloc.kind == "ExternalInput":
            if name != partition_name:
                in_names.append(name)
        elif alloc.kind == "ExternalOutput":
            out_names.append(name)
            shape = tuple(alloc.tensor_shape)
            dtype = mb.dt.np(alloc.dtype)
            out_avals.append(jax.core.ShapedArray(shape, dtype))
            zero_outs.append(np.zeros(shape, dtype))
    n_params = len(in_names)
    in_names_all = in_names + out_names
    if partition_name is not None:
        in_names_all.append(partition_name)

    def _body(*args):
        operands = list(args)
        if partition_name is not None:
            operands.append(bass2jax.partition_id_tensor())
        outs = bass2jax._bass_exec_p.bind(
            *operands,
            out_avals=tuple(out_avals),
            in_names=tuple(in_names_all),
            out_names=tuple(out_names),
            lowering_input_output_aliases=(),
            sim_require_finite=True,
            sim_require_nnan=True,
            nc=nc,
        )
        return tuple(outs)

    devices = jax.devices()[:N_CORES]
    mesh = Mesh(np.asarray(devices), ("core",))
    n_outs = len(out_names)
    sharded = jax.jit(
        shard_map(
            _body, mesh=mesh,
            in_specs=(PartitionSpec("core"),) * (n_params + n_outs),
            out_specs=(PartitionSpec("core"),) * n_outs,
            check_rep=False,
        ),
        keep_unused=True,
    )
    return sharded, in_names, zero_outs


def bench(inputs, n_rep=8):
    """Time the K_LO- and K_HI-iteration unrolled programs; the slope
    isolates steady-state per-iteration device time from the (large, noisy)
    per-execute axon dispatch constant."""
    import time
    import jax
    from concourse import bass2jax

    bass2jax.install_neuronx_cc_hook()
    maps = _prep_in_maps(**inputs)

    runners = {}
    for k_it in (K_LO, K_HI):
        nc = _get_nc(k_it, hw_loop=True, unroll=16)
        sharded, in_names, zero_outs = _build_sharded(nc)
        concat_in = [
            np.concatenate([np.asarray(maps[c][nm]) for c in range(N_CORES)],
                           axis=0)
            for nm in in_names
        ]
        concat_zeros = [
            np.zeros((N_CORES * z.shape[0], *z.shape[1:]), z.dtype)
            for z in zero_outs
        ]
        args_dev = [jax.device_put(a) for a in concat_in + concat_zeros]
        outs = sharded(*args_dev)
        jax.block_until_ready(outs)
        runners[k_it] = (sharded, args_dev, outs)

    def timed(k_it):
        sharded, args_dev, _ = runners[k_it]
        o = sharded(*args_dev)
        jax.block_until_ready(o)
        t0 = time.time()
        o = sharded(*args_dev)
        jax.block_until_ready(o)
        return time.time() - t0

    lo = [timed(K_LO) for _ in range(n_rep)]
    hi = [timed(K_HI) for _ in range(n_rep)]
    per_iter_ns = (min(hi) - min(lo)) / (K_HI - K_LO) * 1e9

    outs = runners[K_HI][2]
    full = np.asarray(outs[0])          # [N_CORES*T, OSH] concat along axis0
    parts = [full[c * T : (c + 1) * T] for c in range(N_CORES)]
    out = np.concatenate(parts, axis=1).reshape(4, 32, O).astype(np.float32)
    return per_iter_ns, out, (min(lo), min(hi))


# revision 34
# speedup vs baseline: 3.1187x; 1.0511x over previous
"""AWQ 4-bit quantized linear (out = x @ dequant(qweight).T + bias), 8-core
tensor-parallel on TRN2.

Sharding: out_features split 8 ways (O' = 1024 per core); x replicated; each
core computes out[:, c*1024:(c+1)*1024] and the host concatenates.  No
device collectives.

Per-core layout (same trick as the 1-core baseline, O-sliced): qweight u16
rows are permuted so SBUF tile i holds u16-rows {16p + i : p in [0,128)}.
Input column c = 4r + k = 64p + (4i + k); the AWQ group of column c is
g = p//2, independent of (i,k), so one [128, O'] scale tile
s128[p,o] = scales[o, p//2] serves every dequant op, and the x-transposes
are plain stride-64 column slices of x.

Dequant per tile/plane: nib = qw & (15<<4k) (DVE tensor_scalar, 4x mode),
w = nib * s128 (tensor_tensor, 2x mode).  The 2^{4k} nibble-position factor
is compensated on the x side during PSUM->SBUF eviction (ACT).  A subset of
the w-multiplies runs on the GPSIMD (Pool) engine to unload DVE, which is
otherwise the bottleneck engine.

Zeros + bias fold into one extra K=128 matmul: R[p,t] = sum of raw x over
columns [64p, 64p+64) (recovered from the scaled transposes via identity-
matmul accumulation and a 16^k recombination), paired with
bmat[p,o] = -(s*z)[p//2, o]; row 127 of R is forced to 1.0 and bmat[127] =
bias (row 127's half-group sum is folded into row 126 by matrix A).

Benchmarking: the per-execute axon dispatch costs ~1 ms/core and swamps
wall-clock timing of single executions.  bench() therefore builds a second
Bass module with the whole per-iteration body unrolled K_HI times (each
iteration re-DMAs all inputs from DRAM and re-writes the output) and
measures the slope between the K_LO- and K_HI-iteration programs: the
marginal cost of one extra full computation on-device, i.e. the steady-state
HW execution time per iteration.
"""

import numpy as np
from contextlib import ExitStack

import concourse.bass as bass
import concourse.mybir as mybir
import concourse.tile as tile
from concourse.bass_utils import run_bass_kernel_spmd
from concourse.masks import make_identity

dt = mybir.dt

N_CORES = 8
I = 8192                    # in_features
O = 8192                    # out_features
OSH = O // N_CORES          # 1024 out-features per core
T = 128                     # batch*seq = 4*32
NG = 64                     # groups (group_size 128)
NR = 64                     # residue tiles (r64 = 4i + k)
NS = OSH // 512             # 512-wide matmul chunks per out block (2)

_CACHE = {}

_KORD = (3, 0, 1, 2)        # k=3 (v-plane, no AND) first so DVE starts early


def _emit_iteration(nc, tc, P):
    """Emit one full computation: load everything from DRAM, dequantize,
    matmul, correct zeros+bias, store out.

    qw tiles are processed in PAIRS (one [128, 2*OSH] SBUF tile covering
    original tiles 2j and 2j+1) so each DVE dequant op handles 2*OSH
    elements, halving per-instruction overhead.  The scale operand is a
    doubled tile s128d = s128|s128.  DVE emission order is chosen to avoid
    head-of-line blocking on the in-order engine: the k=3 multiplies of the
    first two pairs (which need only qw + scales) run before the y-plane
    subtracts (which need the x transposes)."""
    qw_d, x_d, aux_d, out_d = P["qw_d"], P["x_d"], P["aux_d"], P["out_d"]
    ident = P["ident"]

    # s128d first on the scalar queue: it unblocks the early k=3 DVE
    # multiplies; x follows (feeds the PE transposes); the bias/correction
    # tables are only needed late.
    s128 = P["cst_p"].tile([128, OSH], dt.float16, tag="s128")
    nc.scalar.dma_start(s128[:], aux_d[:, 0:OSH])
    x_sb = P["x_p"].tile([T, I], dt.float16, tag="x")
    nc.scalar.dma_start(x_sb[:], x_d[:, :])
    # amat4: four copies of the A matrix pre-scaled by 16^k; the R matmuls
    # for plane k use amat4[:, 128k:128(k+1)] so all 64 accumulate into ONE
    # psum slot and no 16^k recombination pass is needed.
    amat4 = P["sm_p"].tile([128, 512], dt.float16, tag="amat4")
    nc.scalar.dma_start(amat4[:], aux_d[:, 2 * OSH : 2 * OSH + 512])
    cmat = P["sm_p"].tile([128, T], dt.float16, tag="cmat")
    nc.scalar.dma_start(cmat[:], aux_d[:, 2 * OSH + 512 : 2 * OSH + 640])
    bmat = P["cst_p"].tile([128, OSH], dt.float16, tag="bmat")
    nc.scalar.dma_start(bmat[:], aux_d[:, OSH : 2 * OSH])

    # ---- early DVE work: k=3 dequant of ALL quads needs only qw+scales,
    # so DVE is busy while the x transposes are still in flight ----
    def bq(ap):        # view a [128, 4*OSH] quad op as [128, 4, OSH]
        return ap.rearrange("p (a b) -> p a b", a=4)

    s128b = s128[:].unsqueeze(1).broadcast_to([128, 4, OSH])
    qwts, w3s = {}, {}
    for j in range(4):
        qwt = P["qwt_p"].tile([128, 4 * OSH], dt.uint16, tag="qwt")
        for h in range(4):
            r0 = 512 * j + 128 * h
            nc.sync.dma_start(qwt[:, OSH * h : OSH * (h + 1)],
                              qw_d[r0 : r0 + 128, :])
        qwts[j] = qwt
        w3 = P["w3_p"].tile([128, 4 * OSH], dt.float16, tag="w3")
        nc.vector.tensor_tensor(out=bq(w3[:]), in0=bq(qwt[:]), in1=s128b,
                                op=mybir.AluOpType.mult)
        w3s[j] = w3

    # ---- preamble: 64 transposes, 4-per-PSUM-tile, evicted into one
    # [128, 2048] tile per nibble plane k (quarter q at 512*q).
    # k=3 first: its transposes feed both the y-subtracts and the early
    # k=3 matmuls. ----
    x_r = x_sb.rearrange("t (p r) -> t r p", r=NR)
    xts, yts = {}, {}
    for k in _KORD:
        xt = P["xt_p"].tile([128, 4 * 4 * T], dt.float16, tag=f"xt{k}")
        for q in range(4):
            ps = P["pst_p"].tile([128, 4 * T], dt.float16, tag="tp")
            for m in range(4):
                r64 = 4 * (4 * q + m) + k
                nc.tensor.transpose(
                    ps[:, T * m : T * (m + 1)], x_r[:, r64, :], ident[:]
                )
            if k == 0:
                nc.scalar.copy(xt[:, 512 * q : 512 * (q + 1)], ps[:])
            else:
                nc.scalar.mul(xt[:, 512 * q : 512 * (q + 1)], ps[:],
                              float(2.0 ** (-4 * k)))
        xts[k] = xt

    # V-plane trick: plane k=3 is the FULL u16 value (w = v*s, no AND),
    # paired with a3; planes k<3 pair with y_k = a_k - a_3 so the extra
    # terms telescope away exactly.
    for k in range(3):
        y = P["xt_p"].tile([128, 4 * 4 * T], dt.float16, tag=f"y{k}")
        nc.vector.tensor_tensor(out=y[:], in0=xts[k][:], in1=xts[3][:],
                                op=mybir.AluOpType.subtract)
        yts[k] = y

    def xsl(k, i):
        q, m = i // 4, i % 4
        src = xts[3] if k == 3 else yts[k]
        return src[:, 512 * q + T * m : 512 * q + T * (m + 1)]

    # ---- all k=3 matmuls up front (w3 + xts[3] are ready first) ----
    out_ps = P["pso_p"].tile([T, OSH], dt.float32, tag="out")
    for j in range(4):
        for h in range(4):
            i = 4 * j + h
            for ns in range(NS):
                nc.tensor.matmul(
                    out_ps[:, 512 * ns : 512 * (ns + 1)],
                    xsl(3, i),
                    w3s[j][:, OSH * h + 512 * ns : OSH * h + 512 * (ns + 1)],
                    start=(j == 0 and h == 0),
                    stop=False,
                )

    # ---- half-group sums of raw x -> R (PE): R = sum_k 16^k A @ a_k ----
    psum_x = P["psx_p"].tile([128, T], dt.float32, tag="psx")
    for k in range(4):
        for q in range(4):
            for m in range(4):
                nc.tensor.matmul(
                    psum_x[:],
                    amat4[:, 128 * k : 128 * (k + 1)],
                    xts[k][:, 512 * q + T * m : 512 * q + T * (m + 1)],
                    start=(k == 0 and q == 0 and m == 0),
                    stop=(k == 3 and q == 3 and m == 3),
                )

    def emit_quad(j):
        qwt = qwts[j]
        for k in (0, 1, 2):
            nib = P["nib_p"].tile([128, 4 * OSH], dt.uint16, tag="nib")
            nc.vector.tensor_scalar(
                out=nib[:], in0=qwt[:], scalar1=15 << (4 * k),
                scalar2=None, op0=mybir.AluOpType.bitwise_and,
            )
            w = P["w_p"].tile([128, 4 * OSH], dt.float16, tag="w")
            nc.vector.tensor_tensor(out=bq(w[:]), in0=bq(nib[:]), in1=s128b,
                                    op=mybir.AluOpType.mult)
            for h in range(4):
                i = 4 * j + h
                for ns in range(NS):
                    nc.tensor.matmul(
                        out_ps[:, 512 * ns : 512 * (ns + 1)],
                        xsl(k, i),
                        w[:, OSH * h + 512 * ns : OSH * h + 512 * (ns + 1)],
                        start=False,
                        stop=False,
                    )

    for j in range(3):
        emit_quad(j)

    # row 127 (zeroed by A) becomes the bias row: rmat = psum_x + C
    rmat = P["sm_p"].tile([128, T], dt.float16, tag="rmat")
    nc.vector.tensor_tensor(
        out=rmat[:], in0=psum_x[:], in1=cmat[:], op=mybir.AluOpType.add
    )

    emit_quad(3)

    # zeros + bias correction
    for ns in range(NS):
        nc.tensor.matmul(
            out_ps[:, 512 * ns : 512 * (ns + 1)],
            rmat[:],
            bmat[:, 512 * ns : 512 * (ns + 1)],
            start=False, stop=(ns == NS - 1),
        )
    osb = P["osb_p"].tile([T, OSH], dt.float32, tag="osb")
    nc.scalar.copy(osb[:], out_ps[:])
    nc.scalar.dma_start(out_d[:, :], osb[:])


def _build_nc(n_iters=1, hw_loop=False, unroll=1):
    nc = bass.Bass()
    qw_d = nc.dram_tensor("qw", [2048, OSH], dt.uint16, kind="ExternalInput")
    x_d = nc.dram_tensor("xin", [T, I], dt.float16, kind="ExternalInput")
    # aux cols: 0:OSH s128 | OSH:2*OSH bmat | +512 amat4 | +128 cmat
    aux_d = nc.dram_tensor("aux", [128, 2 * OSH + 640], dt.float16,
                           kind="ExternalInput")
    out_d = nc.dram_tensor("out", [T, OSH], dt.float32, kind="ExternalOutput")

    with tile.TileContext(nc) as tc:
        with ExitStack() as ctx:
            P = {
                "qw_d": qw_d, "x_d": x_d, "aux_d": aux_d, "out_d": out_d,
                "x_p": ctx.enter_context(tc.tile_pool(name="x", bufs=1)),
                "cst_p": ctx.enter_context(tc.tile_pool(name="cst", bufs=2)),
                "sm_p": ctx.enter_context(tc.tile_pool(name="sm", bufs=2)),
                "xt_p": ctx.enter_context(tc.tile_pool(name="xt", bufs=2)),
                "qwt_p": ctx.enter_context(tc.tile_pool(name="qwt", bufs=5)),
                "nib_p": ctx.enter_context(tc.tile_pool(name="nib", bufs=2)),
                "w_p": ctx.enter_context(tc.tile_pool(name="w", bufs=3)),
                "w3_p": ctx.enter_context(tc.tile_pool(name="w3", bufs=5)),
                "osb_p": ctx.enter_context(tc.tile_pool(name="osb", bufs=1)),
                "sgl_p": ctx.enter_context(tc.tile_pool(name="sgl", bufs=1)),
                "pst_p": ctx.enter_context(
                    tc.tile_pool(name="pst", bufs=3, space="PSUM")),
                "psx_p": ctx.enter_context(
                    tc.tile_pool(name="psx", bufs=1, space="PSUM")),
                "pso_p": ctx.enter_context(
                    tc.tile_pool(name="pso", bufs=1, space="PSUM")),
            }
            ident = P["sgl_p"].tile([128, 128], dt.float16, tag="ident")
            make_identity(nc, ident[:])
            P["ident"] = ident
            if hw_loop:
                assert n_iters % unroll == 0
                with tc.For_i(0, n_iters // unroll):
                    for _ in range(unroll):
                        _emit_iteration(nc, tc, P)
            else:
                for _ in range(n_iters):
                    _emit_iteration(nc, tc, P)

    _split_excess_waits(nc)
    nc.finalize()
    return nc


_SPLIT_TYPES = {
    "InstTensorScalarPtr", "InstTensorTensor", "InstActivation", "InstMatmult",
    "InstDMACopy", "InstDmaTransposeAnt", "InstMemSet", "InstTensorCopy",
    "InstTensorReduce", "InstDrain", "InstMemset", "InstNoOp",
}

_ENG_MAP = {
    "DVE": "vector", "Activation": "scalar", "PE": "tensor",
    "Pool": "gpsimd", "SP": "sync",
}


def _split_excess_waits(nc):
    """walrus accepts at most one sync-wait per (non-drain) instruction in
    this build; move excess waits onto same-engine ENGINE_NOPs inserted just
    before the instruction."""
    for bb in nc.main_func.blocks:
        insts = list(bb.instructions)
        need = []  # (idx, inst, extra_waits)
        for idx, ins in enumerate(insts):
            if type(ins).__name__ not in _SPLIT_TYPES:
                continue
            si = ins.sync_info
            w = list(si.on_wait) if si else []
            if len(w) > 1:
                need.append((idx, ins, w))
        if not need:
            continue
        created = {}
        for idx, ins, w in need:
            eng = _ENG_MAP.get(ins.engine.name if ins.engine else "", "vector")
            nops = []
            for extra in w[:-1]:
                bi = getattr(nc, eng).nop()
                nop = bi.ins
                nop.sync_info = mybir.SyncInfo(on_wait=[extra], on_update=[])
                nops.append(nop)
            ins.sync_info = mybir.SyncInfo(
                on_wait=[w[-1]], on_update=list(ins.sync_info.on_update))
            created[idx] = nops
        nop_names = {n.name for nops in created.values() for n in nops}
        for bb2 in nc.main_func.blocks:
            cur = [i for i in bb2.instructions if i.name not in nop_names]
            if bb2.name == bb.name:
                out = []
                for idx, ins in enumerate(insts):
                    if idx in created:
                        out.extend(created[idx])
                    out.append(ins)
                bb2.instructions = out
            elif len(cur) != len(list(bb2.instructions)):
                bb2.instructions = cur


def _prep_in_maps(x, qweight, scales, qzeros, bias):
    """Host prep: repack qweight to the permuted-u16 layout and build the
    per-core aux tables.  Cached on input identity."""
    key = (id(x), id(qweight), id(scales), id(qzeros), id(bias))
    cached = _CACHE.get("prep")
    if cached is not None and cached[0] == key:
        return cached[1]

    x = np.asarray(x)
    qweight = np.asarray(qweight)
    scales = np.asarray(scales)
    qzeros = np.asarray(qzeros)
    bias = np.asarray(bias)

    x2 = np.ascontiguousarray(x.reshape(T, I))
    if x2.dtype != np.float16:
        x2 = x2.astype(np.float16)

    # qw tile i, partition p <- u16 row 16p+i of qweight.view(u16).T
    qw16 = qweight.view(np.uint16)              # [O, 2048] (nibble c=4r+k in row o)
    qwp = np.ascontiguousarray(
        qw16.reshape(O, 128, 16).transpose(2, 1, 0)
    ).reshape(2048, O)

    scT = scales.T.astype(np.float16)           # [64, O]
    rep = np.arange(128) // 2
    s128 = np.ascontiguousarray(scT[rep])       # [128, O]

    # zeros: znib[g, o] = nibble g%4 of u16 word g//4 of qzeros row o
    qz16 = qzeros.view(np.uint16)               # [O, 16]
    g = np.arange(NG)
    znib = (qz16.T[g // 4] >> (4 * (g % 4))[:, None]).astype(np.uint16) & 15
    bm_half = -(scT.astype(np.float32) * znib.astype(np.float32))  # [64, O]
    bmat = bm_half[rep].astype(np.float16)
    bmat[127] = bias.astype(np.float16)

    # A matrix: identity, col 126 also sums row 127, col 127 dead.
    # amat4[:, 128k:128(k+1)] = 16^k * A compensates the 2^-4k scaling of
    # the plane-k transposes, so the 64 R matmuls share one psum slot.
    amat = np.eye(128, dtype=np.float32)
    amat[127, 127] = 0.0
    amat[127, 126] = 1.0
    amat4 = np.concatenate(
        [amat * float(16 ** k) for k in range(4)], axis=1
    ).astype(np.float16)
    # C: row 127 ones (bias row of R)
    cmat = np.zeros((128, 128), np.float16)
    cmat[127, :T] = 1.0

    maps = []
    for c in range(N_CORES):
        osl = slice(OSH * c, OSH * (c + 1))
        aux = np.concatenate(
            [s128[:, osl], bmat[:, osl], amat4, cmat], axis=1
        ).astype(np.float16)
        maps.append({
            "qw": np.ascontiguousarray(qwp[:, osl]),
            "xin": x2,
            "aux": np.ascontiguousarray(aux),
        })
    _CACHE["prep"] = (key, maps)
    return maps


def _get_nc(n_iters=1, hw_loop=False, unroll=1):
    key = ("nc", n_iters, hw_loop, unroll)
    if key not in _CACHE:
        _CACHE[key] = _build_nc(n_iters, hw_loop, unroll)
    return _CACHE[key]


def _gather(results):
    out = np.concatenate(
        [np.asarray(results[c]["out"]) for c in range(N_CORES)], axis=1
    )
    return np.ascontiguousarray(out.reshape(4, 32, O).astype(np.float32))


def run(inputs, trace=False, trace_cores=None):
    nc = _get_nc(1)
    maps = _prep_in_maps(**inputs)
    res = run_bass_kernel_spmd(nc, maps, list(range(N_CORES)), trace=trace,
                               trace_cores=trace_cores)
    return _gather(res.results), res


def kernel(**inputs) -> np.ndarray:
    out, _ = run(inputs, trace=False)
    return out


K_LO = 16
K_HI = 816


def _build_sharded(nc):
    import jax
    from jax.sharding import Mesh, PartitionSpec
    from jax.experimental.shard_map import shard_map
    from concourse import bass2jax
    import concourse.mybir as mb

    partition_name = nc.partition_id_tensor.name if nc.partition_id_tensor else None
    in_names, out_names, out_avals, zero_outs = [], [], [], []
    for alloc in nc.m.functions[0].allocations:
        if not isinstance(alloc, mb.MemoryLocationSet):
            continue
        name = alloc.memorylocations[0].name
        if alloc.kind == "ExternalInput":
            if name != partition_name:
                in_names.append(name)
        elif alloc.kind == "ExternalOutput":
            out_names.append(name)
            shape = tuple(alloc.tensor_shape)
            dtype = mb.dt.np(alloc.dtype)
            out_avals.append(jax.core.ShapedArray(shape, dtype))
            zero_outs.append(np.zeros(shape, dtype))
    n_params = len(in_names)
    in_names_all = in_names + out_names
    if partition_name is not None:
        in_names_all.append(partition_name)

    def _body(*args):
        operands = list(args)
        if partition_name is not None:
            operands.append(bass2jax.partition_id_tensor())
        outs = bass2jax._bass_exec_p.bind(
            *operands,
            out_avals=tuple(out_avals),
            in_names=tuple(in_names_all),
            out_names=tuple(out_names),
            lowering_input_output_aliases=(),
            sim_require_finite=True,
            sim_require_nnan=True,
            nc=nc,
        )
        return tuple(outs)

    devices = jax.devices()[:N_CORES]
    mesh = Mesh(np.asarray(devices), ("core",))
    n_outs = len(out_names)
    sharded = jax.jit(
        shard_map(
            _body, mesh=mesh,
            in_specs=(PartitionSpec("core"),) * (n_params + n_outs),
            out_specs=(PartitionSpec("core"),) * n_outs,
            check_rep=False,
        ),
        keep_unused=True,
    )
    return sharded, in_names, zero_outs


def bench(inputs, n_rep=8):
    """Time the K_LO- and K_HI-iteration unrolled programs; the slope
    isolates steady-state per-iteration device time from the (large, noisy)
    per-execute axon dispatch constant."""
    import time
    import jax
    from concourse import bass2jax

    bass2jax.install_neuronx_cc_hook()
    maps = _prep_in_maps(**inputs)

    runners = {}
    for k_it in (K_LO, K_HI):
        nc = _get_nc(k_it, hw_loop=True, unroll=16)
        sharded, in_names, zero_outs = _build_sharded(nc)
        concat_in = [
            np.concatenate([np.asarray(maps[c][nm]) for c in range(N_CORES)],
                           axis=0)
            for nm in in_names
        ]
        concat_zeros = [
            np.zeros((N_CORES * z.shape[0], *z.shape[1:]), z.dtype)
            for z in zero_outs
        ]
        args_dev = [jax.device_put(a) for a in concat_in + concat_zeros]
        outs = sharded(*args_dev)
        jax.block_until_ready(outs)
        runners[k_it] = (sharded, args_dev, outs)

    def timed(k_it):
        sharded, args_dev, _ = runners[k_it]
        o = sharded(*args_dev)
        jax.block_until_ready(o)
        t0 = time.time()
        o = sharded(*args_dev)
        jax.block_until_ready(o)
        return time.time() - t0

    lo = [timed(K_LO) for _ in range(n_rep)]
    hi = [timed(K_HI) for _ in range(n_rep)]
    per_iter_ns = (min(hi) - min(lo)) / (K_HI - K_LO) * 1e9

    outs = runners[K_HI][2]
    full = np.asarray(outs[0])          # [N_CORES*T, OSH] concat along axis0
    parts = [full[c * T : (c + 1) * T] for c in range(N_CORES)]
    out = np.concatenate(parts, axis=1).reshape(4, 32, O).astype(np.float32)
    return per_iter_ns, out, (min(lo), min(hi))


# revision 37
# speedup vs baseline: 3.1227x; 1.0013x over previous
"""AWQ 4-bit quantized linear (out = x @ dequant(qweight).T + bias), 8-core
tensor-parallel on TRN2.

Sharding: out_features split 8 ways (O' = 1024 per core); x replicated; each
core computes out[:, c*1024:(c+1)*1024] and the host concatenates.  No
device collectives.

Per-core layout (same trick as the 1-core baseline, O-sliced): qweight u16
rows are permuted so SBUF tile i holds u16-rows {16p + i : p in [0,128)}.
Input column c = 4r + k = 64p + (4i + k); the AWQ group of column c is
g = p//2, independent of (i,k), so one [128, O'] scale tile
s128[p,o] = scales[o, p//2] serves every dequant op, and the x-transposes
are plain stride-64 column slices of x.

Dequant per tile/plane: nib = qw & (15<<4k) (DVE tensor_scalar, 4x mode),
w = nib * s128 (tensor_tensor, 2x mode).  The 2^{4k} nibble-position factor
is compensated on the x side during PSUM->SBUF eviction (ACT).  A subset of
the w-multiplies runs on the GPSIMD (Pool) engine to unload DVE, which is
otherwise the bottleneck engine.

Zeros + bias fold into one extra K=128 matmul: R[p,t] = sum of raw x over
columns [64p, 64p+64) (recovered from the scaled transposes via identity-
matmul accumulation and a 16^k recombination), paired with
bmat[p,o] = -(s*z)[p//2, o]; row 127 of R is forced to 1.0 and bmat[127] =
bias (row 127's half-group sum is folded into row 126 by matrix A).

Benchmarking: the per-execute axon dispatch costs ~1 ms/core and swamps
wall-clock timing of single executions.  bench() therefore builds a second
Bass module with the whole per-iteration body unrolled K_HI times (each
iteration re-DMAs all inputs from DRAM and re-writes the output) and
measures the slope between the K_LO- and K_HI-iteration programs: the
marginal cost of one extra full computation on-device, i.e. the steady-state
HW execution time per iteration.
"""

import numpy as np
from contextlib import ExitStack

import concourse.bass as bass
import concourse.mybir as mybir
import concourse.tile as tile
from concourse.bass_utils import run_bass_kernel_spmd
from concourse.masks import make_identity

dt = mybir.dt

N_CORES = 8
I = 8192                    # in_features
O = 8192                    # out_features
OSH = O // N_CORES          # 1024 out-features per core
T = 128                     # batch*seq = 4*32
NG = 64                     # groups (group_size 128)
NR = 64                     # residue tiles (r64 = 4i + k)
NS = OSH // 512             # 512-wide matmul chunks per out block (2)

_CACHE = {}

_KORD = (3, 0, 1, 2)        # k=3 (v-plane, no AND) first so DVE starts early


def _emit_iteration(nc, tc, P):
    """Emit one full computation: load everything from DRAM, dequantize,
    matmul, correct zeros+bias, store out.

    qw tiles are processed in PAIRS (one [128, 2*OSH] SBUF tile covering
    original tiles 2j and 2j+1) so each DVE dequant op handles 2*OSH
    elements, halving per-instruction overhead.  The scale operand is a
    doubled tile s128d = s128|s128.  DVE emission order is chosen to avoid
    head-of-line blocking on the in-order engine: the k=3 multiplies of the
    first two pairs (which need only qw + scales) run before the y-plane
    subtracts (which need the x transposes)."""
    qw_d, x_d, aux_d, out_d = P["qw_d"], P["x_d"], P["aux_d"], P["out_d"]
    ident = P["ident"]

    # s128d first on the scalar queue: it unblocks the early k=3 DVE
    # multiplies; x follows (feeds the PE transposes); the bias/correction
    # tables are only needed late.
    s128 = P["cst_p"].tile([128, OSH], dt.float16, tag="s128")
    nc.scalar.dma_start(s128[:], aux_d[:, 0:OSH])
    x_sb = P["x_p"].tile([T, I], dt.float16, tag="x")
    nc.scalar.dma_start(x_sb[:], x_d[:, :])
    # amat4: four copies of the A matrix pre-scaled by 16^k; the R matmuls
    # for plane k use amat4[:, 128k:128(k+1)] so all 64 accumulate into ONE
    # psum slot and no 16^k recombination pass is needed.
    amat4 = P["sm_p"].tile([128, 512], dt.float16, tag="amat4")
    nc.scalar.dma_start(amat4[:], aux_d[:, 2 * OSH : 2 * OSH + 512])
    cmat = P["sm_p"].tile([128, T], dt.float16, tag="cmat")
    nc.scalar.dma_start(cmat[:], aux_d[:, 2 * OSH + 512 : 2 * OSH + 640])
    bmat = P["cst_p"].tile([128, OSH], dt.float16, tag="bmat")
    nc.scalar.dma_start(bmat[:], aux_d[:, OSH : 2 * OSH])

    # ---- early DVE work: k=3 dequant of ALL quads needs only qw+scales,
    # so DVE is busy while the x transposes are still in flight ----
    def bq(ap):        # view a [128, 4*OSH] quad op as [128, 4, OSH]
        return ap.rearrange("p (a b) -> p a b", a=4)

    s128b = s128[:].unsqueeze(1).broadcast_to([128, 4, OSH])
    qwts, w3s = {}, {}
    for j in range(4):
        qwt = P["qwt_p"].tile([128, 4 * OSH], dt.uint16, tag="qwt")
        for h in range(4):
            r0 = 512 * j + 128 * h
            nc.sync.dma_start(qwt[:, OSH * h : OSH * (h + 1)],
                              qw_d[r0 : r0 + 128, :])
        qwts[j] = qwt
        w3 = P["w3_p"].tile([128, 4 * OSH], dt.float16, tag="w3")
        nc.vector.tensor_tensor(out=bq(w3[:]), in0=bq(qwt[:]), in1=s128b,
                                op=mybir.AluOpType.mult)
        w3s[j] = w3

    # ---- preamble: 64 transposes, 4-per-PSUM-tile, evicted into one
    # [128, 2048] tile per nibble plane k (quarter q at 512*q).
    # k=3 first: its transposes feed both the y-subtracts and the early
    # k=3 matmuls. ----
    x_r = x_sb.rearrange("t (p r) -> t r p", r=NR)
    xts, yts = {}, {}
    for k in _KORD:
        xt = P["xt_p"].tile([128, 4 * 4 * T], dt.float16, tag=f"xt{k}")
        for q in range(4):
            ps = P["pst_p"].tile([128, 4 * T], dt.float16, tag="tp")
            for m in range(4):
                r64 = 4 * (4 * q + m) + k
                nc.tensor.transpose(
                    ps[:, T * m : T * (m + 1)], x_r[:, r64, :], ident[:]
                )
            if k == 0:
                nc.scalar.copy(xt[:, 512 * q : 512 * (q + 1)], ps[:])
            else:
                nc.scalar.mul(xt[:, 512 * q : 512 * (q + 1)], ps[:],
                              float(2.0 ** (-4 * k)))
        xts[k] = xt

    # V-plane trick: plane k=3 is the FULL u16 value (w = v*s, no AND),
    # paired with a3; planes k<3 pair with y_k = a_k - a_3 so the extra
    # terms telescope away exactly.
    for k in range(3):
        y = P["xt_p"].tile([128, 4 * 4 * T], dt.float16, tag=f"y{k}")
        nc.vector.tensor_tensor(out=y[:], in0=xts[k][:], in1=xts[3][:],
                                op=mybir.AluOpType.subtract)
        yts[k] = y

    def xsl(k, i):
        q, m = i // 4, i % 4
        src = xts[3] if k == 3 else yts[k]
        return src[:, 512 * q + T * m : 512 * q + T * (m + 1)]

    # ---- all k=3 matmuls up front (w3 + xts[3] are ready first) ----
    out_ps = P["pso_p"].tile([T, OSH], dt.float32, tag="out")
    for j in range(4):
        for h in range(4):
            i = 4 * j + h
            for ns in range(NS):
                nc.tensor.matmul(
                    out_ps[:, 512 * ns : 512 * (ns + 1)],
                    xsl(3, i),
                    w3s[j][:, OSH * h + 512 * ns : OSH * h + 512 * (ns + 1)],
                    start=(j == 0 and h == 0),
                    stop=False,
                )

    # ---- half-group sums of raw x -> R (PE): R = sum_k 16^k A @ a_k ----
    psum_x = P["psx_p"].tile([128, T], dt.float32, tag="psx")
    for k in range(4):
        for q in range(4):
            for m in range(4):
                nc.tensor.matmul(
                    psum_x[:],
                    amat4[:, 128 * k : 128 * (k + 1)],
                    xts[k][:, 512 * q + T * m : 512 * q + T * (m + 1)],
                    start=(k == 0 and q == 0 and m == 0),
                    stop=(k == 3 and q == 3 and m == 3),
                )

    def emit_quad(j):
        qwt = qwts[j]
        for k in (0, 1, 2):
            nib = P["nib_p"].tile([128, 4 * OSH], dt.uint16, tag="nib")
            nc.vector.tensor_scalar(
                out=nib[:], in0=qwt[:], scalar1=15 << (4 * k),
                scalar2=None, op0=mybir.AluOpType.bitwise_and,
            )
            w = P["w_p"].tile([128, 4 * OSH], dt.float16, tag="w")
            nc.vector.tensor_tensor(out=bq(w[:]), in0=bq(nib[:]), in1=s128b,
                                    op=mybir.AluOpType.mult)
            for h in range(4):
                i = 4 * j + h
                for ns in range(NS):
                    nc.tensor.matmul(
                        out_ps[:, 512 * ns : 512 * (ns + 1)],
                        xsl(k, i),
                        w[:, OSH * h + 512 * ns : OSH * h + 512 * (ns + 1)],
                        start=False,
                        stop=False,
                    )

    for j in range(3):
        emit_quad(j)

    # row 127 (zeroed by A) becomes the bias row: rmat = psum_x + C
    rmat = P["sm_p"].tile([128, T], dt.float16, tag="rmat")
    nc.vector.tensor_tensor(
        out=rmat[:], in0=psum_x[:], in1=cmat[:], op=mybir.AluOpType.add
    )

    emit_quad(3)

    # zeros + bias correction
    for ns in range(NS):
        nc.tensor.matmul(
            out_ps[:, 512 * ns : 512 * (ns + 1)],
            rmat[:],
            bmat[:, 512 * ns : 512 * (ns + 1)],
            start=False, stop=(ns == NS - 1),
        )
    osb = P["osb_p"].tile([T, OSH], dt.float32, tag="osb")
    nc.scalar.copy(osb[:], out_ps[:])
    nc.scalar.dma_start(out_d[:, :], osb[:])


def _build_nc(n_iters=1, hw_loop=False, unroll=1):
    nc = bass.Bass()
    qw_d = nc.dram_tensor("qw", [2048, OSH], dt.uint16, kind="ExternalInput")
    x_d = nc.dram_tensor("xin", [T, I], dt.float16, kind="ExternalInput")
    # aux cols: 0:OSH s128 | OSH:2*OSH bmat | +512 amat4 | +128 cmat
    aux_d = nc.dram_tensor("aux", [128, 2 * OSH + 640], dt.float16,
                           kind="ExternalInput")
    out_d = nc.dram_tensor("out", [T, OSH], dt.float32, kind="ExternalOutput")

    with tile.TileContext(nc) as tc:
        with ExitStack() as ctx:
            P = {
                "qw_d": qw_d, "x_d": x_d, "aux_d": aux_d, "out_d": out_d,
                "x_p": ctx.enter_context(tc.tile_pool(name="x", bufs=1)),
                "cst_p": ctx.enter_context(tc.tile_pool(name="cst", bufs=2)),
                "sm_p": ctx.enter_context(tc.tile_pool(name="sm", bufs=2)),
                "xt_p": ctx.enter_context(tc.tile_pool(name="xt", bufs=2)),
                "qwt_p": ctx.enter_context(tc.tile_pool(name="qwt", bufs=5)),
                "nib_p": ctx.enter_context(tc.tile_pool(name="nib", bufs=2)),
                "w_p": ctx.enter_context(tc.tile_pool(name="w", bufs=3)),
                "w3_p": ctx.enter_context(tc.tile_pool(name="w3", bufs=5)),
                "osb_p": ctx.enter_context(tc.tile_pool(name="osb", bufs=1)),
                "sgl_p": ctx.enter_context(tc.tile_pool(name="sgl", bufs=1)),
                "pst_p": ctx.enter_context(
                    tc.tile_pool(name="pst", bufs=3, space="PSUM")),
                "psx_p": ctx.enter_context(
                    tc.tile_pool(name="psx", bufs=1, space="PSUM")),
                "pso_p": ctx.enter_context(
                    tc.tile_pool(name="pso", bufs=1, space="PSUM")),
            }
            ident = P["sgl_p"].tile([128, 128], dt.float16, tag="ident")
            make_identity(nc, ident[:])
            P["ident"] = ident
            if hw_loop:
                assert n_iters % unroll == 0
                with tc.For_i(0, n_iters // unroll):
                    for _ in range(unroll):
                        _emit_iteration(nc, tc, P)
            else:
                for _ in range(n_iters):
                    _emit_iteration(nc, tc, P)

    _split_excess_waits(nc)
    nc.finalize()
    return nc


_SPLIT_TYPES = {
    "InstTensorScalarPtr", "InstTensorTensor", "InstActivation", "InstMatmult",
    "InstDMACopy", "InstDmaTransposeAnt", "InstMemSet", "InstTensorCopy",
    "InstTensorReduce", "InstDrain", "InstMemset", "InstNoOp",
}

_ENG_MAP = {
    "DVE": "vector", "Activation": "scalar", "PE": "tensor",
    "Pool": "gpsimd", "SP": "sync",
}


def _split_excess_waits(nc):
    """walrus accepts at most one sync-wait per (non-drain) instruction in
    this build; move excess waits onto same-engine ENGINE_NOPs inserted just
    before the instruction."""
    for bb in nc.main_func.blocks:
        insts = list(bb.instructions)
        need = []  # (idx, inst, extra_waits)
        for idx, ins in enumerate(insts):
            if type(ins).__name__ not in _SPLIT_TYPES:
                continue
            si = ins.sync_info
            w = list(si.on_wait) if si else []
            if len(w) > 1:
                need.append((idx, ins, w))
        if not need:
            continue
        created = {}
        for idx, ins, w in need:
            eng = _ENG_MAP.get(ins.engine.name if ins.engine else "", "vector")
            nops = []
            for extra in w[:-1]:
                bi = getattr(nc, eng).nop()
                nop = bi.ins
                nop.sync_info = mybir.SyncInfo(on_wait=[extra], on_update=[])
                nops.append(nop)
            ins.sync_info = mybir.SyncInfo(
                on_wait=[w[-1]], on_update=list(ins.sync_info.on_update))
            created[idx] = nops
        nop_names = {n.name for nops in created.values() for n in nops}
        for bb2 in nc.main_func.blocks:
            cur = [i for i in bb2.instructions if i.name not in nop_names]
            if bb2.name == bb.name:
                out = []
                for idx, ins in enumerate(insts):
                    if idx in created:
                        out.extend(created[idx])
                    out.append(ins)
                bb2.instructions = out
            elif len(cur) != len(list(bb2.instructions)):
                bb2.instructions = cur


def _prep_in_maps(x, qweight, scales, qzeros, bias):
    """Host prep: repack qweight to the permuted-u16 layout and build the
    per-core aux tables.  Cached on input identity."""
    key = (id(x), id(qweight), id(scales), id(qzeros), id(bias))
    cached = _CACHE.get("prep")
    if cached is not None and cached[0] == key:
        return cached[1]

    x = np.asarray(x)
    qweight = np.asarray(qweight)
    scales = np.asarray(scales)
    qzeros = np.asarray(qzeros)
    bias = np.asarray(bias)

    x2 = np.ascontiguousarray(x.reshape(T, I))
    if x2.dtype != np.float16:
        x2 = x2.astype(np.float16)

    # qw tile i, partition p <- u16 row 16p+i of qweight.view(u16).T
    qw16 = qweight.view(np.uint16)              # [O, 2048] (nibble c=4r+k in row o)
    qwp = np.ascontiguousarray(
        qw16.reshape(O, 128, 16).transpose(2, 1, 0)
    ).reshape(2048, O)

    scT = scales.T.astype(np.float16)           # [64, O]
    rep = np.arange(128) // 2
    s128 = np.ascontiguousarray(scT[rep])       # [128, O]

    # zeros: znib[g, o] = nibble g%4 of u16 word g//4 of qzeros row o
    qz16 = qzeros.view(np.uint16)               # [O, 16]
    g = np.arange(NG)
    znib = (qz16.T[g // 4] >> (4 * (g % 4))[:, None]).astype(np.uint16) & 15
    bm_half = -(scT.astype(np.float32) * znib.astype(np.float32))  # [64, O]
    bmat = bm_half[rep].astype(np.float16)
    bmat[127] = bias.astype(np.float16)

    # A matrix: identity, col 126 also sums row 127, col 127 dead.
    # amat4[:, 128k:128(k+1)] = 16^k * A compensates the 2^-4k scaling of
    # the plane-k transposes, so the 64 R matmuls share one psum slot.
    amat = np.eye(128, dtype=np.float32)
    amat[127, 127] = 0.0
    amat[127, 126] = 1.0
    amat4 = np.concatenate(
        [amat * float(16 ** k) for k in range(4)], axis=1
    ).astype(np.float16)
    # C: row 127 ones (bias row of R)
    cmat = np.zeros((128, 128), np.float16)
    cmat[127, :T] = 1.0

    maps = []
    for c in range(N_CORES):
        osl = slice(OSH * c, OSH * (c + 1))
        aux = np.concatenate(
            [s128[:, osl], bmat[:, osl], amat4, cmat], axis=1
        ).astype(np.float16)
        maps.append({
            "qw": np.ascontiguousarray(qwp[:, osl]),
            "xin": x2,
            "aux": np.ascontiguousarray(aux),
        })
    _CACHE["prep"] = (key, maps)
    return maps


def _get_nc(n_iters=1, hw_loop=False, unroll=1):
    key = ("nc", n_iters, hw_loop, unroll)
    if key not in _CACHE:
        _CACHE[key] = _build_nc(n_iters, hw_loop, unroll)
    return _CACHE[key]


def _gather(results):
    out = np.concatenate(
        [np.asarray(results[c]["out"]) for c in range(N_CORES)], axis=1
    )
    return np.ascontiguousarray(out.reshape(4, 32, O).astype(np.float32))


def run(inputs, trace=False, trace_cores=None):
    nc = _get_nc(1)
    maps = _prep_in_maps(**inputs)
    res = run_bass_kernel_spmd(nc, maps, list(range(N_CORES)), trace=trace,
                               trace_cores=trace_cores)
    return _gather(res.results), res


def kernel(**inputs) -> np.ndarray:
    out, _ = run(inputs, trace=False)
    return out


K_LO = 32
K_HI = 1632


def _build_sharded(nc):
    import jax
    from jax.sharding import Mesh, PartitionSpec
    from jax.experimental.shard_map import shard_map
    from concourse import bass2jax
    import concourse.mybir as mb

    partition_name = nc.partition_id_tensor.name if nc.partition_id_tensor else None
    in_names, out_names, out_avals, zero_outs = [], [], [], []
    for alloc in nc.m.functions[0].allocations:
        if not isinstance(alloc, mb.MemoryLocationSet):
            continue
        name = alloc.memorylocations[0].name
        if alloc.kind == "ExternalInput":
            if name != partition_name:
                in_names.append(name)
        elif alloc.kind == "ExternalOutput":
            out_names.append(name)
            shape = tuple(alloc.tensor_shape)
            dtype = mb.dt.np(alloc.dtype)
            out_avals.append(jax.core.ShapedArray(shape, dtype))
            zero_outs.append(np.zeros(shape, dtype))
    n_params = len(in_names)
    in_names_all = in_names + out_names
    if partition_name is not None:
        in_names_all.append(partition_name)

    def _body(*args):
        operands = list(args)
        if partition_name is not None:
            operands.append(bass2jax.partition_id_tensor())
        outs = bass2jax._bass_exec_p.bind(
            *operands,
            out_avals=tuple(out_avals),
            in_names=tuple(in_names_all),
            out_names=tuple(out_names),
            lowering_input_output_aliases=(),
            sim_require_finite=True,
            sim_require_nnan=True,
            nc=nc,
        )
        return tuple(outs)

    devices = jax.devices()[:N_CORES]
    mesh = Mesh(np.asarray(devices), ("core",))
    n_outs = len(out_names)
    sharded = jax.jit(
        shard_map(
            _body, mesh=mesh,
            in_specs=(PartitionSpec("core"),) * (n_params + n_outs),
            out_specs=(PartitionSpec("core"),) * n_outs,
            check_rep=False,
        ),
        keep_unused=True,
    )
    return sharded, in_names, zero_outs


def bench(inputs, n_rep=10):
    """Time the K_LO- and K_HI-iteration unrolled programs; the slope
    isolates steady-state per-iteration device time from the (large, noisy)
    per-execute axon dispatch constant."""
    import time
    import jax
    from concourse import bass2jax

    bass2jax.install_neuronx_cc_hook()
    maps = _prep_in_maps(**inputs)

    runners = {}
    for k_it in (K_LO, K_HI):
        nc = _get_nc(k_it, hw_loop=True, unroll=32)
        sharded, in_names, zero_outs = _build_sharded(nc)
        concat_in = [
            np.concatenate([np.asarray(maps[c][nm]) for c in range(N_CORES)],
                           axis=0)
            for nm in in_names
        ]
        concat_zeros = [
            np.zeros((N_CORES * z.shape[0], *z.shape[1:]), z.dtype)
            for z in zero_outs
        ]
        args_dev = [jax.device_put(a) for a in concat_in + concat_zeros]
        outs = sharded(*args_dev)
        jax.block_until_ready(outs)
        runners[k_it] = (sharded, args_dev, outs)

    def timed(k_it):
        sharded, args_dev, _ = runners[k_it]
        o = sharded(*args_dev)
        jax.block_until_ready(o)
        t0 = time.time()
        o = sharded(*args_dev)
        jax.block_until_ready(o)
        return time.time() - t0

    lo = [timed(K_LO) for _ in range(n_rep)]
    hi = [timed(K_HI) for _ in range(n_rep)]
    per_iter_ns = (min(hi) - min(lo)) / (K_HI - K_LO) * 1e9

    outs = runners[K_HI][2]
    full = np.asarray(outs[0])          # [N_CORES*T, OSH] concat along axis0
    parts = [full[c * T : (c + 1) * T] for c in range(N_CORES)]
    out = np.concatenate(parts, axis=1).reshape(4, 32, O).astype(np.float32)
    return per_iter_ns, out, (min(lo), min(hi))


# revision 44
# speedup vs baseline: 3.2143x; 1.0293x over previous
"""AWQ 4-bit quantized linear (out = x @ dequant(qweight).T + bias), 8-core
tensor-parallel on TRN2.

Sharding: out_features split 8 ways (O' = 1024 per core); x replicated; each
core computes out[:, c*1024:(c+1)*1024] and the host concatenates.  No
device collectives.

Per-core layout (same trick as the 1-core baseline, O-sliced): qweight u16
rows are permuted so SBUF tile i holds u16-rows {16p + i : p in [0,128)}.
Input column c = 4r + k = 64p + (4i + k); the AWQ group of column c is
g = p//2, independent of (i,k), so one [128, O'] scale tile
s128[p,o] = scales[o, p//2] serves every dequant op, and the x-transposes
are plain stride-64 column slices of x.

Dequant per tile/plane: nib = qw & (15<<4k) (DVE tensor_scalar, 4x mode),
w = nib * s128 (tensor_tensor, 2x mode).  The 2^{4k} nibble-position factor
is compensated on the x side during PSUM->SBUF eviction (ACT).  A subset of
the w-multiplies runs on the GPSIMD (Pool) engine to unload DVE, which is
otherwise the bottleneck engine.

Zeros + bias fold into one extra K=128 matmul: R[p,t] = sum of raw x over
columns [64p, 64p+64) (recovered from the scaled transposes via identity-
matmul accumulation and a 16^k recombination), paired with
bmat[p,o] = -(s*z)[p//2, o]; row 127 of R is forced to 1.0 and bmat[127] =
bias (row 127's half-group sum is folded into row 126 by matrix A).

Benchmarking: the per-execute axon dispatch costs ~1 ms/core and swamps
wall-clock timing of single executions.  bench() therefore builds a second
Bass module with the whole per-iteration body unrolled K_HI times (each
iteration re-DMAs all inputs from DRAM and re-writes the output) and
measures the slope between the K_LO- and K_HI-iteration programs: the
marginal cost of one extra full computation on-device, i.e. the steady-state
HW execution time per iteration.
"""

import numpy as np
from contextlib import ExitStack

import concourse.bass as bass
import concourse.mybir as mybir
import concourse.tile as tile
from concourse.bass_utils import run_bass_kernel_spmd
from concourse.masks import make_identity

dt = mybir.dt

N_CORES = 8
I = 8192                    # in_features
O = 8192                    # out_features
OSH = O // N_CORES          # 1024 out-features per core
T = 128                     # batch*seq = 4*32
NG = 64                     # groups (group_size 128)
NR = 64                     # residue tiles (r64 = 4i + k)
NS = OSH // 512             # 512-wide matmul chunks per out block (2)

_CACHE = {}

_KORD = (3, 0, 1, 2)        # k=3 (v-plane, no AND) first so DVE starts early


def _emit_iteration(nc, tc, P):
    """Emit one full computation: load everything from DRAM, dequantize,
    matmul, correct zeros+bias, store out.

    qw tiles are processed in PAIRS (one [128, 2*OSH] SBUF tile covering
    original tiles 2j and 2j+1) so each DVE dequant op handles 2*OSH
    elements, halving per-instruction overhead.  The scale operand is a
    doubled tile s128d = s128|s128.  DVE emission order is chosen to avoid
    head-of-line blocking on the in-order engine: the k=3 multiplies of the
    first two pairs (which need only qw + scales) run before the y-plane
    subtracts (which need the x transposes)."""
    qw_d, x_d, aux_d, out_d = P["qw_d"], P["x_d"], P["aux_d"], P["out_d"]
    ident = P["ident"]

    # s128d first on the scalar queue: it unblocks the early k=3 DVE
    # multiplies; x follows (feeds the PE transposes); the bias/correction
    # tables are only needed late.
    s128 = P["cst_p"].tile([128, OSH], dt.float16, tag="s128")
    nc.scalar.dma_start(s128[:], aux_d[:, 0:OSH])
    x_sb = P["x_p"].tile([T, I], dt.float16, tag="x")
    nc.scalar.dma_start(x_sb[:], x_d[:, :])
    # amat4: four copies of the A matrix pre-scaled so the 64 R matmuls all
    # accumulate into ONE psum slot: 16^k*A for the k<3 y-planes, 4369*A
    # for the a3 plane (compensating sum_k 16^k a_k = sum_{k<3} 16^k y_k +
    # 4369*a_3).
    amat4 = P["sm_p"].tile([128, 512], dt.float16, tag="amat4")
    nc.scalar.dma_start(amat4[:], aux_d[:, 2 * OSH : 2 * OSH + 512])
    cmat = P["sm_p"].tile([128, T], dt.float16, tag="cmat")
    nc.scalar.dma_start(cmat[:], aux_d[:, 2 * OSH + 512 : 2 * OSH + 640])
    # idents: scaled identity matrices for the y-plane transpose-matmuls:
    # cols 0:128 = 2^-4 I, 128:256 = 2^-8 I, 256:384 = -2^-12 I
    idents = P["sm_p"].tile([128, 384], dt.float16, tag="idents")
    nc.scalar.dma_start(idents[:], aux_d[:, 2 * OSH + 640 : 2 * OSH + 1024])
    bmat = P["cst_p"].tile([128, OSH], dt.float16, tag="bmat")
    nc.scalar.dma_start(bmat[:], aux_d[:, OSH : 2 * OSH])

    # ---- early DVE work: k=3 dequant of ALL quads needs only qw+scales,
    # so DVE is busy while the x transposes are still in flight ----
    def bq(ap):        # view a [128, 4*OSH] quad op as [128, 4, OSH]
        return ap.rearrange("p (a b) -> p a b", a=4)

    s128b = s128[:].unsqueeze(1).broadcast_to([128, 4, OSH])
    qwts, w3s = {}, {}
    for j in range(4):
        qwt = P["qwt_p"].tile([128, 4 * OSH], dt.uint16, tag="qwt")
        for h in range(4):
            r0 = 512 * j + 128 * h
            nc.sync.dma_start(qwt[:, OSH * h : OSH * (h + 1)],
                              qw_d[r0 : r0 + 128, :])
        qwts[j] = qwt
        w3 = P["w3_p"].tile([128, 4 * OSH], dt.float16, tag="w3")
        nc.vector.tensor_tensor(out=bq(w3[:]), in0=bq(qwt[:]), in1=s128b,
                                op=mybir.AluOpType.mult)
        w3s[j] = w3

    # ---- preamble: plane k=3 via plain transposes (scaled on eviction);
    # planes k<3 produced DIRECTLY as y_k = 2^-4k T_k - 2^-12 T_3 on PE,
    # by pairs of scaled-identity matmuls accumulating in PSUM (the V-plane
    # subtraction costs no DVE time this way). ----
    x_r = x_sb.rearrange("t (p r) -> t r p", r=NR)
    xts, yts = {}, {}
    xt3 = P["xt_p"].tile([128, 4 * 4 * T], dt.float16, tag="xt3")
    for q in range(4):
        ps = P["pst_p"].tile([128, 4 * T], dt.float16, tag="tp")
        for m in range(4):
            r64 = 4 * (4 * q + m) + 3
            nc.tensor.transpose(
                ps[:, T * m : T * (m + 1)], x_r[:, r64, :], ident[:]
            )
        nc.scalar.mul(xt3[:, 512 * q : 512 * (q + 1)], ps[:],
                      float(2.0 ** -12))
    xts[3] = xt3

    for k in range(3):
        y = P["xt_p"].tile([128, 4 * 4 * T], dt.float16, tag=f"y{k}")
        for q in range(4):
            ps = P["psty_p"].tile([128, 4 * T], dt.float32, tag="ty")
            for m in range(4):
                r64k = 4 * (4 * q + m) + k
                r643 = 4 * (4 * q + m) + 3
                if k == 0:
                    rhs_k = ident[:]
                else:
                    rhs_k = idents[:, 128 * (k - 1) : 128 * k]
                nc.tensor.matmul(
                    ps[:, T * m : T * (m + 1)], x_r[:, r64k, :], rhs_k,
                    start=True, stop=False,
                )
                nc.tensor.matmul(
                    ps[:, T * m : T * (m + 1)], x_r[:, r643, :],
                    idents[:, 256:384],
                    start=False, stop=True,
                )
            nc.scalar.copy(y[:, 512 * q : 512 * (q + 1)], ps[:])
        yts[k] = y

    def xsl(k, i):
        q, m = i // 4, i % 4
        src = xts[3] if k == 3 else yts[k]
        return src[:, 512 * q + T * m : 512 * q + T * (m + 1)]

    # ---- all k=3 matmuls up front (w3 + xts[3] are ready first) ----
    out_ps = P["pso_p"].tile([T, OSH], dt.float32, tag="out")
    for j in range(4):
        for h in range(4):
            i = 4 * j + h
            for ns in range(NS):
                nc.tensor.matmul(
                    out_ps[:, 512 * ns : 512 * (ns + 1)],
                    xsl(3, i),
                    w3s[j][:, OSH * h + 512 * ns : OSH * h + 512 * (ns + 1)],
                    start=(j == 0 and h == 0),
                    stop=False,
                )

    # ---- half-group sums of raw x -> R (PE):
    # R = sum_{k<3} 16^k A @ y_k + 4369 A @ a_3 ----
    psum_x = P["psx_p"].tile([128, T], dt.float32, tag="psx")
    for k in range(4):
        src = yts[k] if k < 3 else xts[3]
        for q in range(4):
            for m in range(4):
                nc.tensor.matmul(
                    psum_x[:],
                    amat4[:, 128 * k : 128 * (k + 1)],
                    src[:, 512 * q + T * m : 512 * q + T * (m + 1)],
                    start=(k == 0 and q == 0 and m == 0),
                    stop=(k == 3 and q == 3 and m == 3),
                )

    def emit_quad(j):
        qwt = qwts[j]
        for k in (0, 1, 2):
            nib = P["nib_p"].tile([128, 4 * OSH], dt.uint16, tag="nib")
            nc.vector.tensor_scalar(
                out=nib[:], in0=qwt[:], scalar1=15 << (4 * k),
                scalar2=None, op0=mybir.AluOpType.bitwise_and,
            )
            w = P["w_p"].tile([128, 4 * OSH], dt.float16, tag="w")
            nc.vector.tensor_tensor(out=bq(w[:]), in0=bq(nib[:]), in1=s128b,
                                    op=mybir.AluOpType.mult)
            for h in range(4):
                i = 4 * j + h
                for ns in range(NS):
                    nc.tensor.matmul(
                        out_ps[:, 512 * ns : 512 * (ns + 1)],
                        xsl(k, i),
                        w[:, OSH * h + 512 * ns : OSH * h + 512 * (ns + 1)],
                        start=False,
                        stop=False,
                    )

    for j in range(3):
        emit_quad(j)

    # row 127 (zeroed by A) becomes the bias row: rmat = psum_x + C
    rmat = P["sm_p"].tile([128, T], dt.float16, tag="rmat")
    nc.vector.tensor_tensor(
        out=rmat[:], in0=psum_x[:], in1=cmat[:], op=mybir.AluOpType.add
    )

    emit_quad(3)

    # zeros + bias correction
    for ns in range(NS):
        nc.tensor.matmul(
            out_ps[:, 512 * ns : 512 * (ns + 1)],
            rmat[:],
            bmat[:, 512 * ns : 512 * (ns + 1)],
            start=False, stop=(ns == NS - 1),
        )
    osb = P["osb_p"].tile([T, OSH], dt.float32, tag="osb")
    nc.scalar.copy(osb[:], out_ps[:])
    nc.scalar.dma_start(out_d[:, :], osb[:])


def _build_nc(n_iters=1, hw_loop=False, unroll=1):
    nc = bass.Bass()
    qw_d = nc.dram_tensor("qw", [2048, OSH], dt.uint16, kind="ExternalInput")
    x_d = nc.dram_tensor("xin", [T, I], dt.float16, kind="ExternalInput")
    # aux cols: 0:OSH s128 | OSH:2*OSH bmat | +512 amat4 | +128 cmat
    # | +384 scaled identities
    aux_d = nc.dram_tensor("aux", [128, 2 * OSH + 1024], dt.float16,
                           kind="ExternalInput")
    out_d = nc.dram_tensor("out", [T, OSH], dt.float32, kind="ExternalOutput")

    with tile.TileContext(nc) as tc:
        with ExitStack() as ctx:
            P = {
                "qw_d": qw_d, "x_d": x_d, "aux_d": aux_d, "out_d": out_d,
                "x_p": ctx.enter_context(tc.tile_pool(name="x", bufs=1)),
                "cst_p": ctx.enter_context(tc.tile_pool(name="cst", bufs=2)),
                "sm_p": ctx.enter_context(tc.tile_pool(name="sm", bufs=2)),
                "xt_p": ctx.enter_context(tc.tile_pool(name="xt", bufs=2)),
                "qwt_p": ctx.enter_context(tc.tile_pool(name="qwt", bufs=5)),
                "nib_p": ctx.enter_context(tc.tile_pool(name="nib", bufs=2)),
                "w_p": ctx.enter_context(tc.tile_pool(name="w", bufs=3)),
                "w3_p": ctx.enter_context(tc.tile_pool(name="w3", bufs=5)),
                "osb_p": ctx.enter_context(tc.tile_pool(name="osb", bufs=1)),
                "sgl_p": ctx.enter_context(tc.tile_pool(name="sgl", bufs=1)),
                "pst_p": ctx.enter_context(
                    tc.tile_pool(name="pst", bufs=2, space="PSUM")),
                "psty_p": ctx.enter_context(
                    tc.tile_pool(name="psty", bufs=3, space="PSUM")),
                "psx_p": ctx.enter_context(
                    tc.tile_pool(name="psx", bufs=1, space="PSUM")),
                "pso_p": ctx.enter_context(
                    tc.tile_pool(name="pso", bufs=1, space="PSUM")),
            }
            ident = P["sgl_p"].tile([128, 128], dt.float16, tag="ident")
            make_identity(nc, ident[:])
            P["ident"] = ident
            if hw_loop:
                assert n_iters % unroll == 0
                with tc.For_i(0, n_iters // unroll):
                    for _ in range(unroll):
                        _emit_iteration(nc, tc, P)
            else:
                for _ in range(n_iters):
                    _emit_iteration(nc, tc, P)

    _split_excess_waits(nc)
    nc.finalize()
    return nc


_SPLIT_TYPES = {
    "InstTensorScalarPtr", "InstTensorTensor", "InstActivation", "InstMatmult",
    "InstDMACopy", "InstDmaTransposeAnt", "InstMemSet", "InstTensorCopy",
    "InstTensorReduce", "InstDrain", "InstMemset", "InstNoOp",
}

_ENG_MAP = {
    "DVE": "vector", "Activation": "scalar", "PE": "tensor",
    "Pool": "gpsimd", "SP": "sync",
}


def _split_excess_waits(nc):
    """walrus accepts at most one sync-wait per (non-drain) instruction in
    this build; move excess waits onto same-engine ENGINE_NOPs inserted just
    before the instruction."""
    for bb in nc.main_func.blocks:
        insts = list(bb.instructions)
        need = []  # (idx, inst, extra_waits)
        for idx, ins in enumerate(insts):
            if type(ins).__name__ not in _SPLIT_TYPES:
                continue
            si = ins.sync_info
            w = list(si.on_wait) if si else []
            if len(w) > 1:
                need.append((idx, ins, w))
        if not need:
            continue
        created = {}
        for idx, ins, w in need:
            eng = _ENG_MAP.get(ins.engine.name if ins.engine else "", "vector")
            nops = []
            for extra in w[:-1]:
                bi = getattr(nc, eng).nop()
                nop = bi.ins
                nop.sync_info = mybir.SyncInfo(on_wait=[extra], on_update=[])
                nops.append(nop)
            ins.sync_info = mybir.SyncInfo(
                on_wait=[w[-1]], on_update=list(ins.sync_info.on_update))
            created[idx] = nops
        nop_names = {n.name for nops in created.values() for n in nops}
        for bb2 in nc.main_func.blocks:
            cur = [i for i in bb2.instructions if i.name not in nop_names]
            if bb2.name == bb.name:
                out = []
                for idx, ins in enumerate(insts):
                    if idx in created:
                        out.extend(created[idx])
                    out.append(ins)
                bb2.instructions = out
            elif len(cur) != len(list(bb2.instructions)):
                bb2.instructions = cur


def _prep_in_maps(x, qweight, scales, qzeros, bias):
    """Host prep: repack qweight to the permuted-u16 layout and build the
    per-core aux tables.  Cached on input identity."""
    key = (id(x), id(qweight), id(scales), id(qzeros), id(bias))
    cached = _CACHE.get("prep")
    if cached is not None and cached[0] == key:
        return cached[1]

    x = np.asarray(x)
    qweight = np.asarray(qweight)
    scales = np.asarray(scales)
    qzeros = np.asarray(qzeros)
    bias = np.asarray(bias)

    x2 = np.ascontiguousarray(x.reshape(T, I))
    if x2.dtype != np.float16:
        x2 = x2.astype(np.float16)

    # qw tile i, partition p <- u16 row 16p+i of qweight.view(u16).T
    qw16 = qweight.view(np.uint16)              # [O, 2048] (nibble c=4r+k in row o)
    qwp = np.ascontiguousarray(
        qw16.reshape(O, 128, 16).transpose(2, 1, 0)
    ).reshape(2048, O)

    scT = scales.T.astype(np.float16)           # [64, O]
    rep = np.arange(128) // 2
    s128 = np.ascontiguousarray(scT[rep])       # [128, O]

    # zeros: znib[g, o] = nibble g%4 of u16 word g//4 of qzeros row o
    qz16 = qzeros.view(np.uint16)               # [O, 16]
    g = np.arange(NG)
    znib = (qz16.T[g // 4] >> (4 * (g % 4))[:, None]).astype(np.uint16) & 15
    bm_half = -(scT.astype(np.float32) * znib.astype(np.float32))  # [64, O]
    bmat = bm_half[rep].astype(np.float16)
    bmat[127] = bias.astype(np.float16)

    # A matrix: identity, col 126 also sums row 127, col 127 dead.
    # R accumulates over the y-planes: sum_k 16^k a_k =
    # sum_{k<3} 16^k y_k + (1+16+256+4096) a_3, so amat4 holds 16^k*A for
    # k<3 and 4369*A for the a_3 plane.
    amat = np.eye(128, dtype=np.float32)
    amat[127, 127] = 0.0
    amat[127, 126] = 1.0
    amat4 = np.concatenate(
        [amat * float(16 ** k) for k in range(3)] + [amat * 4369.0], axis=1
    ).astype(np.float16)
    # C: row 127 ones (bias row of R)
    cmat = np.zeros((128, 128), np.float16)
    cmat[127, :T] = 1.0
    # scaled identities for the y-plane matmuls: 2^-4 I | 2^-8 I | -2^-12 I
    eye = np.eye(128, dtype=np.float32)
    idents = np.concatenate(
        [eye * (2.0 ** -4), eye * (2.0 ** -8), eye * -(2.0 ** -12)], axis=1
    ).astype(np.float16)

    maps = []
    for c in range(N_CORES):
        osl = slice(OSH * c, OSH * (c + 1))
        aux = np.concatenate(
            [s128[:, osl], bmat[:, osl], amat4, cmat, idents], axis=1
        ).astype(np.float16)
        maps.append({
            "qw": np.ascontiguousarray(qwp[:, osl]),
            "xin": x2,
            "aux": np.ascontiguousarray(aux),
        })
    _CACHE["prep"] = (key, maps)
    return maps


def _get_nc(n_iters=1, hw_loop=False, unroll=1):
    key = ("nc", n_iters, hw_loop, unroll)
    if key not in _CACHE:
        _CACHE[key] = _build_nc(n_iters, hw_loop, unroll)
    return _CACHE[key]


def _gather(results):
    out = np.concatenate(
        [np.asarray(results[c]["out"]) for c in range(N_CORES)], axis=1
    )
    return np.ascontiguousarray(out.reshape(4, 32, O).astype(np.float32))


def run(inputs, trace=False, trace_cores=None):
    nc = _get_nc(1)
    maps = _prep_in_maps(**inputs)
    res = run_bass_kernel_spmd(nc, maps, list(range(N_CORES)), trace=trace,
                               trace_cores=trace_cores)
    return _gather(res.results), res


def kernel(**inputs) -> np.ndarray:
    out, _ = run(inputs, trace=False)
    return out


K_LO = 32
K_HI = 1632


def _build_sharded(nc):
    import jax
    from jax.sharding import Mesh, PartitionSpec
    from jax.experimental.shard_map import shard_map
    from concourse import bass2jax
    import concourse.mybir as mb

    partition_name = nc.partition_id_tensor.name if nc.partition_id_tensor else None
    in_names, out_names, out_avals, zero_outs = [], [], [], []
    for alloc in nc.m.functions[0].allocations:
        if not isinstance(alloc, mb.MemoryLocationSet):
            continue
        name = alloc.memorylocations[0].name
        if alloc.kind == "ExternalInput":
            if name != partition_name:
                in_names.append(name)
        elif alloc.kind == "ExternalOutput":
            out_names.append(name)
            shape = tuple(alloc.tensor_shape)
            dtype = mb.dt.np(alloc.dtype)
            out_avals.append(jax.core.ShapedArray(shape, dtype))
            zero_outs.append(np.zeros(shape, dtype))
    n_params = len(in_names)
    in_names_all = in_names + out_names
    if partition_name is not None:
        in_names_all.append(partition_name)

    def _body(*args):
        operands = list(args)
        if partition_name is not None:
            operands.append(bass2jax.partition_id_tensor())
        outs = bass2jax._bass_exec_p.bind(
            *operands,
            out_avals=tuple(out_avals),
            in_names=tuple(in_names_all),
            out_names=tuple(out_names),
            lowering_input_output_aliases=(),
            sim_require_finite=True,
            sim_require_nnan=True,
            nc=nc,
        )
        return tuple(outs)

    devices = jax.devices()[:N_CORES]
    mesh = Mesh(np.asarray(devices), ("core",))
    n_outs = len(out_names)
    sharded = jax.jit(
        shard_map(
            _body, mesh=mesh,
            in_specs=(PartitionSpec("core"),) * (n_params + n_outs),
            out_specs=(PartitionSpec("core"),) * n_outs,
            check_rep=False,
        ),
        keep_unused=True,
    )
    return sharded, in_names, zero_outs


def bench(inputs, n_rep=10):
    """Time the K_LO- and K_HI-iteration unrolled programs; the slope
    isolates steady-state per-iteration device time from the (large, noisy)
    per-execute axon dispatch constant."""
    import time
    import jax
    from concourse import bass2jax

    bass2jax.install_neuronx_cc_hook()
    maps = _prep_in_maps(**inputs)

    runners = {}
    for k_it in (K_LO, K_HI):
        nc = _get_nc(k_it, hw_loop=True, unroll=32)
        sharded, in_names, zero_outs = _build_sharded(nc)
        concat_in = [
            np.concatenate([np.asarray(maps[c][nm]) for c in range(N_CORES)],
                           axis=0)
            for nm in in_names
        ]
        concat_zeros = [
            np.zeros((N_CORES * z.shape[0], *z.shape[1:]), z.dtype)
            for z in zero_outs
        ]
        args_dev = [jax.device_put(a) for a in concat_in + concat_zeros]
        outs = sharded(*args_dev)
        jax.block_until_ready(outs)
        runners[k_it] = (sharded, args_dev, outs)

    def timed(k_it):
        sharded, args_dev, _ = runners[k_it]
        o = sharded(*args_dev)
        jax.block_until_ready(o)
        t0 = time.time()
        o = sharded(*args_dev)
        jax.block_until_ready(o)
        return time.time() - t0

    lo = [timed(K_LO) for _ in range(n_rep)]
    hi = [timed(K_HI) for _ in range(n_rep)]
    per_iter_ns = (min(hi) - min(lo)) / (K_HI - K_LO) * 1e9

    outs = runners[K_HI][2]
    full = np.asarray(outs[0])          # [N_CORES*T, OSH] concat along axis0
    parts = [full[c * T : (c + 1) * T] for c in range(N_CORES)]
    out = np.concatenate(parts, axis=1).reshape(4, 32, O).astype(np.float32)
    return per_iter_ns, out, (min(lo), min(hi))
